# revision 12
# baseline (speedup 1.0000x reference)
"""AttnOutputDecoder Trainium2 kernel.

Sharding: data-parallel over batch B=16 across 8 cores (2 batches/core).
Per core: LSTM (gate order i,f,o,g; host-precomputed x@W_ih injected into
PSUM via identity matmul) overlapped with Bahdanau attention scores.
Score tanh-args wh[s]+ws[t] are built as a single fp8 DoubleRow matmul per
256-col chunk: lhsT packs [wh-chunk ; ws-rows] in fp8 (accuracy verified:
adds ~2e-3 rel err), rhs is an exact 0/1 selector matrix in fp8, so each
element is written once at 0.5 cyc/row instead of twice at 1.0. Ws_b is
folded into the ws tile copy. Output projection streams Vp_w.T in bf16
(fp8 fails the 2e-2 gate); logits stored bf16; Vp_b added on host.
"""

import numpy as np
import ml_dtypes

import concourse.bass as bass
import concourse.mybir as mybir
import concourse.tile as tile
from concourse import bacc
from concourse import bass_utils

BF16 = ml_dtypes.bfloat16
FP8 = ml_dtypes.float8_e4m3
F32 = mybir.dt.float32
BF = mybir.dt.bfloat16
F8 = mybir.dt.float8e4
AF = mybir.ActivationFunctionType
ALU = mybir.AluOpType
DR = mybir.MatmulPerfMode.DoubleRow

B, T, S, D, V = 16, 64, 128, 512, 32000
NC = 8
BL = B // NC          # local batches per core = 2
R = BL * T            # local rows = 128
G4 = 4 * D            # 2048 gates
KC = D // 128         # 4 contraction chunks
TS = 8                # score t-sub-block
CC = 256              # su DoubleRow column chunk
VBLK = 512
NBLK = (V + VBLK - 1) // VBLK   # 63
NPRE = 24             # prefetched vocab blocks
NSTRB = 5             # streamed-vocab buffer slots (1 block each)
WAITP = 1780          # pacing period hint (ns/step) for score tasks
WAITB = 6500          # pacing base offset (ns)

_cached = {}


def _build_nc(stage=3):
    # stage 1: LSTM only; 2: + scores/softmax/out2; 3: full (vocab)
    nc = bacc.Bacc("TRN2", target_bir_lowering=False, debug=False,
                   num_devices=NC)

    def din(name, shape, dt):
        return nc.dram_tensor(name, shape, dt, kind="ExternalInput").ap()

    t_ident = din("ident", [128, 128], BF)
    t_h0 = din("h0", [128, KC * BL], BF)
    t_c0 = din("c0", [128, KC * BL], F32)
    t_whha = din("whha", [128, 8 * KC * 128], BF)    # [p,(j<8,kc,g)]
    t_whhb = din("whhb", [128, 8 * KC * 128], BF)    # [p,(j>=8,kc,g)]
    t_xg = din("xg", [128, T * 16 * BL], BF)         # [p,(t,j,b)]
    t_encT = din("encT", [128, KC * BL * S], BF)     # [p,(kc,b,s)]
    t_whT = din("whT", [128, KC * D], BF)            # Wh_w.T re
    t_wsT = din("wsT", [128, KC * D], BF)            # Ws_w.T re
    t_wsb8 = din("wsb8", [TS, D], BF)                # Ws_b row x TS
    t_vt = din("vt", [128, KC], BF)
    t_selv = din("selv", [128, 2 * TS * S], F8)      # DR selectors
    t_enc = din("enc", [128, BL * D], BF)            # [s,(b,d)]
    t_vw1 = din("vw1", [128, KC * D], BF)
    t_vw2 = din("vw2", [128, KC * D], BF)
    t_vb = din("vb", [128, KC], F32)
    t_vpt = din("vpt", [128, KC * V], BF)            # [p,(kc,v)] Vp_w.T re
    t_out = nc.dram_tensor("out", [R, V], BF, kind="ExternalOutput").ap()

    with tile.TileContext(nc) as tc:
        with (
            tc.tile_pool(name="const", bufs=1) as cp,
            tc.tile_pool(name="state", bufs=1) as sp,
            tc.tile_pool(name="gates", bufs=8) as gp,
            tc.tile_pool(name="attn", bufs=2) as ap_,
            tc.tile_pool(name="thp", bufs=2) as thp,
            tc.tile_pool(name="voc", bufs=2) as vp,
            tc.tile_pool(name="lsbp", bufs=2) as lp,
            tc.tile_pool(name="ps_g", bufs=2, space="PSUM") as ppg,
            tc.tile_pool(name="ps_sum", bufs=2, space="PSUM") as pps,
            tc.tile_pool(name="ps_e", bufs=1, space="PSUM") as ppe,
            tc.tile_pool(name="ps_w", bufs=1, space="PSUM") as ppw,
        ):
            # ---- LSTM-critical loads first (serial DMA device) ----
            ident = cp.tile([128, 128], BF)
            nc.sync.dma_start(out=ident[:], in_=t_ident[:])
            h0b = sp.tile([128, KC * BL], BF)
            nc.sync.dma_start(out=h0b[:], in_=t_h0[:])
            c = sp.tile([128, KC * BL], F32)
            nc.sync.dma_start(out=c[:], in_=t_c0[:])
            whha = cp.tile([128, 8 * KC * 128], BF)
            nc.sync.dma_start(out=whha[:], in_=t_whha[:])
            xg = cp.tile([128, T * 16 * BL], BF)
            nc.sync.dma_start(out=xg[:], in_=t_xg[:])
            whhb = cp.tile([128, 8 * KC * 128], BF)
            nc.sync.dma_start(out=whhb[:], in_=t_whhb[:])
            # ---- wh deps, then remaining constants ----
            encT = cp.tile([128, KC * BL * S], BF)
            nc.sync.dma_start(out=encT[:], in_=t_encT[:])
            whT = cp.tile([128, KC * D], BF)
            nc.sync.dma_start(out=whT[:], in_=t_whT[:])
            wsT = cp.tile([128, KC * D], BF)
            nc.sync.dma_start(out=wsT[:], in_=t_wsT[:])
            wsb8 = cp.tile([TS, D], BF)
            nc.sync.dma_start(out=wsb8[:], in_=t_wsb8[:])
            vt = cp.tile([128, KC], BF)
            nc.sync.dma_start(out=vt[:], in_=t_vt[:])
            selv = cp.tile([128, 2 * TS * S], F8)
            nc.sync.dma_start(out=selv[:], in_=t_selv[:])
            enc = cp.tile([128, BL * D], BF)
            nc.sync.dma_start(out=enc[:], in_=t_enc[:])
            vw1 = cp.tile([128, KC * D], BF)
            nc.sync.dma_start(out=vw1[:], in_=t_vw1[:])
            vw2 = cp.tile([128, KC * D], BF)
            nc.sync.dma_start(out=vw2[:], in_=t_vw2[:])
            vb = cp.tile([128, KC], F32)
            nc.sync.dma_start(out=vb[:], in_=t_vb[:])

            encT4 = encT[:].rearrange("p (kc b s) -> p kc b s", kc=KC, b=BL)
            whT4 = whT[:].rearrange("p (kc d) -> p kc d", kc=KC)
            wsT4 = wsT[:].rearrange("p (kc d) -> p kc d", kc=KC)
            selv3 = selv[:].rearrange("p (i n) -> p i n", i=2)

            outT = sp.tile([128, KC * BL * T], BF)   # [p,(kc,b,t)] all h's
            outT4 = outT[:].rearrange("p (kc b t) -> p kc b t", kc=KC, b=BL,
                                      t=T)

            # ---- vocab weight prefetch: chunked large DMAs ----
            vpt4 = t_vpt[:].rearrange("p (kc v) -> p kc v", kc=KC, v=V)
            vpre = cp.tile([128, KC * NPRE * VBLK], BF)
            vpre4 = vpre[:].rearrange("p (kc v) -> p kc v", kc=KC,
                                      v=NPRE * VBLK)
            PCH = 7
            for pc in range(0, NPRE, PCH):
                pe_ = min(NPRE, pc + PCH)
                nc.sync.dma_start(out=vpre4[:, :, pc * VBLK:pe_ * VBLK],
                                  in_=vpt4[:, :, pc * VBLK:pe_ * VBLK])

            # ====== wh[s, d] chunks -> wsu slot0 (fp8), ws_b folded to ws ==
            # wsu[(b,par)]: [128, kc, 2, 128] fp8; slot0 = wh (s-partition),
            # slot1 rows 0..TS = ws rows of t-block (t-partition), rest zero.
            wsu_tiles = {}
            for b_ in range(BL):
                for par in range(2):
                    w_ = sp.tile([128, KC, 2, 128], F8, tag=f"wsu{b_}_{par}")
                    wsu_tiles[(b_, par)] = w_
                    nc.gpsimd.memset(w_[:, :, 1, :], 0)
            for b_ in range(BL):
                whp = ppw.tile([S, D], F32, tag="ws", name=f"whp{b_}")
                for kc in range(KC):
                    nc.tensor.matmul(out=whp[:], lhsT=encT4[:, kc, b_, :],
                                     rhs=whT4[:, kc, :],
                                     start=(kc == 0), stop=(kc == KC - 1))
                whp3 = whp[:].rearrange("p (kc d) -> p kc d", kc=KC)
                for par in range(2):
                    nc.vector.tensor_copy(
                        out=wsu_tiles[(b_, par)][:, :, 0, :], in_=whp3[:, :, :])

            eps = ppe.tile([S, BL * T], F32, tag="eps")   # scores [s,(b,t)]

            # ========== LSTM with interleaved score tasks ==========
            # task_a: fp8 DoubleRow outer-sum + ACT tanh (lags LSTM);
            # task_b: eps dot matmuls, emitted later still.
            pend_a = []
            pend_b = []
            th_tiles = {}

            def emit_a(b, tsub, dc):
                su = pps.tile([128, TS * S], F32, tag="sum",
                              name=f"su{b}_{tsub}_{dc}")
                wsu = wsu_tiles[(b, tsub % 2)]
                for cc in range(TS * S // CC):
                    nc.tensor.matmul(out=su[:, cc * CC:(cc + 1) * CC],
                                     lhsT=wsu[:, dc, :, :],
                                     rhs=selv3[:, :, cc * CC:(cc + 1) * CC],
                                     start=True, stop=True, perf_mode=DR)
                th = thp.tile([128, TS * S], BF, tag=f"th{dc}",
                              name=f"th{b}_{tsub}_{dc}")
                nc.scalar.activation(out=th[:], in_=su[:], func=AF.Tanh)
                th_tiles[(b, tsub, dc)] = th
                if dc == KC - 1:
                    pend_b.append((b, tsub))

            def emit_b(b, tsub, half):
                for tl in range(half * (TS // 2), (half + 1) * (TS // 2)):
                    t = tsub * TS + tl
                    for d2 in range(KC):
                        nc.tensor.matmul(
                            out=eps[:, b * T + t: b * T + t + 1],
                            lhsT=th_tiles[(b, tsub, d2)][:,
                                tl * S:(tl + 1) * S],
                            rhs=vt[:, d2:d2 + 1],
                            start=(d2 == 0), stop=(d2 == KC - 1))

            def pump():
                if pend_a:
                    emit_a(*pend_a.pop(0))
                if pend_b:
                    b, tsub = pend_b[0]
                    half = pump.half
                    emit_b(b, tsub, half)
                    if half == 1:
                        pend_b.pop(0)
                    pump.half = 1 - half
            pump.half = 0

            from contextlib import ExitStack
            for t in range(T):
                _hp = ExitStack()
                _hp.enter_context(tc.high_priority())
                gps = ppg.tile([128, 16 * BL], F32, tag="gps",
                               name=f"gps{t}")
                nc.tensor.matmul(out=gps[:], lhsT=ident[:],
                                 rhs=xg[:, t * 32:(t + 1) * 32],
                                 start=True, stop=False,
                                 skip_group_check=True)
                for j in range(16):
                    wt = whha if j < 8 else whhb
                    jr = j % 8
                    for kc in range(KC):
                        if t == 0:
                            hsrc = h0b[:, kc * BL:(kc + 1) * BL]
                        else:
                            hsrc = outT4[:, kc, :, t - 1]
                        nc.tensor.matmul(
                            out=gps[:, j * BL:(j + 1) * BL],
                            lhsT=wt[:, (jr * KC + kc) * 128:
                                    (jr * KC + kc + 1) * 128],
                            rhs=hsrc, start=False, stop=(kc == KC - 1),
                            skip_group_check=True)
                sio = gp.tile([128, 16 * BL], F32, tag="sio")
                nc.scalar.activation(out=sio[:, 0:12 * BL],
                                     in_=gps[:, 0:12 * BL], func=AF.Sigmoid)
                nc.scalar.activation(out=sio[:, 12 * BL:16 * BL],
                                     in_=gps[:, 12 * BL:16 * BL],
                                     func=AF.Tanh)
                t1 = gp.tile([128, KC * BL], F32, tag="t1")
                t2 = gp.tile([128, KC * BL], F32, tag="t2")
                nc.vector.tensor_mul(out=t1[:], in0=sio[:, 4 * BL:8 * BL],
                                     in1=c[:])
                nc.vector.tensor_mul(out=t2[:], in0=sio[:, 0:4 * BL],
                                     in1=sio[:, 12 * BL:16 * BL])
                nc.vector.tensor_add(out=c[:], in0=t1[:], in1=t2[:])
                tc_ = gp.tile([128, KC * BL], F32, tag="tc")
                nc.scalar.activation(out=tc_[:], in_=c[:], func=AF.Tanh)
                nc.vector.tensor_mul(out=outT4[:, :, :, t],
                                     in0=sio[:, 8 * BL:12 * BL], in1=tc_[:])
                _hp.close()

                with tc.tile_wait_until((WAITB + WAITP * t) / 1e6):
                    pump()

                if stage >= 2 and t % TS == TS - 1:
                    tsub = t // TS
                    tc.tile_set_cur_wait((WAITB + WAITP * t) / 1e6)
                    for b in range(BL):
                        wps = ppw.tile([TS, D], F32, tag="ws",
                                       name=f"wps{b}_{tsub}")
                        for kc in range(KC):
                            nc.tensor.matmul(
                                out=wps[:],
                                lhsT=outT[:, (kc * BL + b) * T + tsub * TS:
                                          (kc * BL + b) * T + tsub * TS + TS],
                                rhs=wsT4[:, kc, :],
                                start=(kc == 0), stop=(kc == KC - 1))
                        wps3 = wps[:].rearrange("p (kc d) -> p kc d", kc=KC)
                        wsb3 = wsb8[:].rearrange("p (kc d) -> p kc d", kc=KC)
                        nc.vector.tensor_add(
                            out=wsu_tiles[(b, tsub % 2)][0:TS, :, 1, :],
                            in0=wps3[:, :, :], in1=wsb3[:, :, :])
                    tc.cur_wait_ts = 0
                    pend_a.extend((b, tsub, dc) for b in range(BL)
                                  for dc in range(KC))

            while pend_a or pend_b:
                pump()

            # ============ softmax, context, out2 ============
            ctxT = sp.tile([128, BL * KC * T], BF)   # [p,(b,dc,t)]
            o2T = sp.tile([128, KC * BL * T], BF)    # [p,(ec,b,t)]
            _hp2 = ExitStack()
            _hp2.enter_context(tc.high_priority())
            for b in range(BL if stage >= 2 else 0):
                ebf = ap_.tile([S, T], BF, tag="ebf")
                nc.scalar.activation(out=ebf[:],
                                     in_=eps[:, b * T:(b + 1) * T],
                                     func=AF.Exp)
                etp = ppw.tile([T, S], BF, tag="ws", name=f"etp{b}")
                nc.tensor.transpose(out=etp[:], in_=ebf[:],
                                    identity=ident[:, :])
                ssum = ap_.tile([T, 1], F32, tag="ssum")
                nc.vector.tensor_reduce(out=ssum[:], in_=etp[:],
                                        axis=mybir.AxisListType.X, op=ALU.add)
                rsum = ap_.tile([T, 1], F32, tag="rsum")
                nc.vector.reciprocal(out=rsum[:], in_=ssum[:])
                abf = ap_.tile([T, S], BF, tag="abf")
                nc.vector.tensor_scalar_mul(out=abf[:], in0=etp[:],
                                            scalar1=rsum[:])
                atp = ppw.tile([S, T], BF, tag="ws", name=f"atp{b}")
                nc.tensor.transpose(out=atp[:], in_=abf[:],
                                    identity=ident[0:T, 0:T])
                atb = ap_.tile([S, T], BF, tag="atb")
                nc.vector.tensor_copy(out=atb[:], in_=atp[:])
                for dc in range(KC):
                    cps = ppg.tile([128, T], F32, tag="gps",
                                   name=f"cps{b}_{dc}")
                    nc.tensor.matmul(out=cps[:],
                                     lhsT=enc[:, b * D + dc * 128:
                                              b * D + (dc + 1) * 128],
                                     rhs=atb[:], start=True, stop=True)
                    nc.vector.tensor_copy(
                        out=ctxT[:, (b * KC + dc) * T:(b * KC + dc + 1) * T],
                        in_=cps[:])
                for ec in range(KC):
                    ops = ppg.tile([128, T], F32, tag="gps",
                                   name=f"ops{b}_{ec}")
                    for kc in range(KC):
                        nc.tensor.matmul(
                            out=ops[:],
                            lhsT=vw1[:, kc * D + ec * 128:
                                     kc * D + (ec + 1) * 128],
                            rhs=ctxT[:, (b * KC + kc) * T:
                                     (b * KC + kc + 1) * T],
                            start=(kc == 0), stop=False)
                    for kc in range(KC):
                        nc.tensor.matmul(
                            out=ops[:],
                            lhsT=vw2[:, kc * D + ec * 128:
                                     kc * D + (ec + 1) * 128],
                            rhs=outT[:, (kc * BL + b) * T:
                                     (kc * BL + b + 1) * T],
                            start=False, stop=(kc == KC - 1))
                    nc.vector.tensor_scalar(
                        out=o2T[:, (ec * BL + b) * T:(ec * BL + b) * T + T],
                        in0=ops[:], scalar1=vb[:, ec:ec + 1], scalar2=None,
                        op0=ALU.add)

            # ================= vocab projection (bf16) ==========
            _hp2.close()
            o2r = o2T[:].rearrange("p (e c) -> p e c", e=KC, c=128)
            # Recycle dead phase-1 SBUF slots as stream buffers for the tail
            # vocab blocks: their DMAs fire as soon as the old tiles' readers
            # retire, moving DMA out of the saturated vocab tail.
            NRG = 4                      # blocks in the whh-slot group
            RS1 = ["xg", "whT", "wsT", "vw1", "vw2"]
            # 2KB slots that free when the last score task retires; pairs
            # stage one block each as two [128,2,512] half-tiles
            RS2 = [("th0", "th1"), ("th2", "th3"), ("th0", "th1"),
                   ("th2", "th3"), ("selv", "encT")]
            rg0 = NBLK - NRG - len(RS1)  # whh group covers rg0..rg0+3
            rs2_0 = rg0 - len(RS2)
            rs3_0 = rs2_0 - 2            # wsu-quad + outT/ctxT/wsb8 blocks
            rsrc = {}
            if stage >= 3:
                # whha+whhb slots: 8 KB each -> two blocks each
                for gi, wtag in enumerate(("whha", "whhb")):
                    ib_g = rg0 + gi * 2
                    vg = cp.tile([128, KC, 2 * VBLK], BF, tag=wtag,
                                 name=f"vgrp{gi}")
                    nc.sync.dma_start(
                        out=vg[:, :, :],
                        in_=vpt4[:, :, ib_g * VBLK:(ib_g + 2) * VBLK])
                    rsrc[ib_g] = (vg, 0)
                    rsrc[ib_g + 1] = (vg, VBLK)
                # quad block from the four 1KB wsu slots (kc0..3)
                ibx = rs3_0
                vqs = []
                for b_ in range(BL):
                    for par in range(2):
                        vq = sp.tile([128, 1, VBLK], BF,
                                     tag=f"wsu{b_}_{par}",
                                     name=f"vq{b_}_{par}")
                        kcq = b_ * 2 + par
                        nc.sync.dma_start(
                            out=vq[:, :, :],
                            in_=vpt4[:, kcq:kcq + 1,
                                     ibx * VBLK:(ibx + 1) * VBLK])
                        vqs.append((vq, 0))
                rsrc[ibx] = ("quad", vqs)
                # quad block from outT (1KB) + ctxT (1KB) + enc (2KB) slots
                iby = rs3_0 + 1
                vq_o = sp.tile([128, 1, VBLK], BF, tag="outT", name="vqo")
                nc.sync.dma_start(out=vq_o[:, :, :],
                                  in_=vpt4[:, 0:1, iby * VBLK:
                                           (iby + 1) * VBLK])
                vq_c = sp.tile([128, 1, VBLK], BF, tag="ctxT", name="vqc")
                nc.sync.dma_start(out=vq_c[:, :, :],
                                  in_=vpt4[:, 1:2, iby * VBLK:
                                           (iby + 1) * VBLK])
                vq_e2 = cp.tile([128, 2, VBLK], BF, tag="enc", name="vqe2")
                nc.sync.dma_start(out=vq_e2[:, :, :],
                                  in_=vpt4[:, 2:4, iby * VBLK:
                                           (iby + 1) * VBLK])
                rsrc[iby] = ("quad", [(vq_o, 0), (vq_c, 0),
                                      (vq_e2, 0), (vq_e2, 1)])
                for i, (tga, tgb) in enumerate(RS2):
                    ib_r = rs2_0 + i
                    v0r = ib_r * VBLK
                    pa = thp if tga.startswith("th") else cp
                    pb = thp if tgb.startswith("th") else cp
                    ta = pa.tile([128, 2, VBLK], BF, tag=tga,
                                 name=f"vspl{ib_r}a")
                    nc.sync.dma_start(out=ta[:, :, :],
                                      in_=vpt4[:, 0:2, v0r:v0r + VBLK])
                    tb = pb.tile([128, 2, VBLK], BF, tag=tgb,
                                 name=f"vspl{ib_r}b")
                    nc.sync.dma_start(out=tb[:, :, :],
                                      in_=vpt4[:, 2:4, v0r:v0r + VBLK])
                    rsrc[ib_r] = ("split", ta, tb)
                for i, tg in enumerate(RS1):
                    ib_r = rg0 + NRG + i
                    v0r = ib_r * VBLK
                    wr = min(VBLK, V - v0r)
                    vrt = cp.tile([128, KC, VBLK], BF, tag=tg,
                                  name=f"vrt{ib_r}")
                    nc.sync.dma_start(out=vrt[:, :, :wr],
                                      in_=vpt4[:, :, v0r:v0r + wr])
                    rsrc[ib_r] = (vrt, 0)
            # consumption order: alternate streamed/prefetched so stream DMAs
            # never stall; recycled-slot blocks go last (data arrives
            # mid-phase)
            tail0 = rs3_0 - (rs3_0 % 4)   # align tail to lsb store groups
            order = []
            si, pi = NPRE, 0
            while si < tail0 or pi < NPRE:
                if si < tail0:
                    order.append(si)
                    si += 1
                if pi < NPRE:
                    order.append(pi)
                    pi += 1
            order += list(range(tail0, NBLK))
            lsb_tiles = {}
            if stage < 3:
                order = []
            for nb, ib in enumerate(order):
                v0 = ib * VBLK
                w = min(VBLK, V - v0)
                if ib < NPRE:
                    def rhs_of(kc, v0=v0, w=w):
                        return vpre4[:, kc, v0:v0 + w]
                elif ib in rsrc:
                    ent = rsrc[ib]
                    if ent[0] == "split":
                        def rhs_of(kc, ta=ent[1], tb=ent[2], w=w):
                            return (ta if kc < 2 else tb)[:, kc % 2, 0:w]
                    elif ent[0] == "quad":
                        def rhs_of(kc, lst=ent[1], w=w):
                            t_, ix = lst[kc]
                            return t_[:, ix, 0:w]
                    else:
                        def rhs_of(kc, vs3=ent[0], voff=ent[1], w=w):
                            return vs3[:, kc, voff:voff + w]
                else:
                    vst = vp.tile([128, KC, VBLK], BF, tag="vs", bufs=NSTRB,
                                  name=f"vst{ib}")
                    nc.sync.dma_start(out=vst[:, :, :w],
                                      in_=vpt4[:, :, v0:v0 + w])
                    def rhs_of(kc, vst=vst, w=w):
                        return vst[:, kc, 0:w]
                grp = ib // 4
                if grp not in lsb_tiles:
                    lsb_tiles[grp] = [lp.tile([128, 4 * VBLK], BF, tag="lsb",
                                              name=f"lsb{grp}"), 0]
                lsb_e = lsb_tiles[grp]
                lps = ppg.tile([128, VBLK], F32, tag="gps", name=f"lps{ib}")
                for kc in range(KC):
                    nc.tensor.matmul(out=lps[:, :w],
                                     lhsT=o2r[:, kc, :],
                                     rhs=rhs_of(kc),
                                     start=(kc == 0), stop=(kc == KC - 1))
                dst = lsb_e[0][:, (ib % 4) * VBLK:(ib % 4) * VBLK + w]
                if nb % 2 == 0:
                    nc.scalar.copy(out=dst, in_=lps[:, :w])
                else:
                    nc.vector.tensor_copy(out=dst, in_=lps[:, :w])
                lsb_e[1] += 1
                nblk_grp = min(4, NBLK - grp * 4)
                if lsb_e[1] == nblk_grp:
                    gv0 = grp * 4 * VBLK
                    wlen = min(4 * VBLK, V - gv0)
                    nc.sync.dma_start(out=t_out[:, gv0:gv0 + wlen],
                                      in_=lsb_e[0][:, :wlen])

    nc.compile()
    return nc


def _prep_in_maps(inputs):
    inp = {k: np.asarray(v) for k, v in inputs.items()}
    words = inp["words"].astype(np.int64)
    enc = inp["encoder_output"].astype(np.float32)
    pre_h, cell = inp["pre_h"], inp["cell"]
    emb = inp["emb"]
    W_ih, W_hh = inp["W_ih"], inp["W_hh"]
    b_ih, b_hh = inp["b_ih"], inp["b_hh"]
    Wh_w = inp["Wh_w"]
    Ws_w, Ws_b = inp["Ws_w"], inp["Ws_b"]
    vt_w = inp["vt_w"]
    V_w, V_b = inp["V_w"], inp["V_b"]
    Vp_w, Vp_b = inp["Vp_w"], inp["Vp_b"]

    def re_lhsT(m, dt=BF16):  # [512, N] -> [128, 4*N] chunk-major
        n = m.shape[1]
        return np.ascontiguousarray(
            m.reshape(4, 128, n).transpose(1, 0, 2).reshape(128, 4 * n)
        ).astype(dt)

    # gate reorder (i,f,g,o) -> (i,f,o,g)
    perm = np.r_[0:512, 512:1024, 1536:2048, 1024:1536]
    W_ih_p, W_hh_p = W_ih[perm], W_hh[perm]
    b2 = (b_ih + b_hh)[perm].astype(np.float32)

    whh_re = re_lhsT(np.ascontiguousarray(W_hh_p.T))     # [p,(kc,g)]
    # -> j-major [p,(j,kc,128)]
    whh_j = np.ascontiguousarray(
        whh_re.reshape(128, KC, 16, 128).transpose(0, 2, 1, 3)
        .reshape(128, 16 * KC * 128))
    whha_re = np.ascontiguousarray(whh_j[:, :8 * KC * 128])
    whhb_re = np.ascontiguousarray(whh_j[:, 8 * KC * 128:])
    whT_re = re_lhsT(np.ascontiguousarray(Wh_w.T))
    wsT_re = re_lhsT(np.ascontiguousarray(Ws_w.T))
    vw1_re = re_lhsT(np.ascontiguousarray(V_w[:, :D].T))
    vw2_re = re_lhsT(np.ascontiguousarray(V_w[:, D:].T))
    vpt_re = re_lhsT(np.ascontiguousarray(Vp_w.T))
    wsb8_re = np.tile(Ws_b.reshape(1, D), (TS, 1)).astype(BF16)
    vb_re = np.ascontiguousarray(V_b.reshape(4, 128).T).astype(np.float32)
    vt_re = np.ascontiguousarray(vt_w.reshape(4, 128).T).astype(BF16)
    ident_re = np.eye(128, dtype=np.float32).astype(BF16)
    # DR selector: slot0[p,(tl,s)] = [p==s], slot1[p,(tl,s)] = [p==tl]
    sel0 = np.tile(np.eye(128, dtype=np.float32), (1, TS))
    sel1 = np.zeros((128, TS * S), dtype=np.float32)
    for tl in range(TS):
        sel1[tl, tl * S:(tl + 1) * S] = 1.0
    selv_re = np.concatenate([sel0, sel1], axis=1).astype(FP8)

    x_all = emb[words]                                   # [B,T,D]
    xg_all = x_all @ W_ih_p.T.astype(np.float32) + b2    # [B,T,4D]

    in_maps = []
    for k in range(NC):
        bs = slice(k * BL, (k + 1) * BL)
        xgl = xg_all[bs]                                 # [2,T,2048]
        xg_re = np.ascontiguousarray(
            xgl.reshape(BL, T, 16, 128).transpose(3, 1, 2, 0)
            .reshape(128, T * 16 * BL)).astype(BF16)     # [p,(t,j,b)]
        h0 = np.ascontiguousarray(
            pre_h[bs].reshape(BL, 4, 128).transpose(2, 1, 0)
            .reshape(128, 4 * BL)).astype(BF16)
        c0 = np.ascontiguousarray(
            cell[bs].reshape(BL, 4, 128).transpose(2, 1, 0)
            .reshape(128, 4 * BL)).astype(np.float32)
        encl = enc[bs]                                   # [2,S,D]
        encT_re = np.ascontiguousarray(
            encl.reshape(BL, S, 4, 128).transpose(3, 2, 0, 1)
            .reshape(128, 4 * BL * S)).astype(BF16)
        enc_re = np.ascontiguousarray(
            encl.transpose(1, 0, 2).reshape(S, BL * D)).astype(BF16)
        in_maps.append({
            "xg": xg_re, "whha": whha_re, "whhb": whhb_re, "h0": h0,
            "c0": c0, "encT": encT_re, "enc": enc_re, "whT": whT_re,
            "wsT": wsT_re, "wsb8": wsb8_re, "vt": vt_re, "selv": selv_re,
            "vw1": vw1_re, "vw2": vw2_re, "vb": vb_re, "vpt": vpt_re,
            "ident": ident_re,
        })
    return in_maps


def kernel(**inputs):
    in_maps = _prep_in_maps(inputs)
    if "nc" not in _cached:
        _cached["nc"] = _build_nc()
    res = bass_utils.run_bass_kernel_spmd(_cached["nc"], in_maps,
                                          core_ids=list(range(NC)))
    vpb = np.asarray(inputs["Vp_b"]).astype(np.float32)
    outs = [np.asarray(res.results[k]["out"]).astype(np.float32)
            .reshape(BL, T, V) for k in range(NC)]
    return np.concatenate(outs, axis=0) + vpb[None, None, :]


if __name__ == "__main__":
    pass


# revision 14
# speedup vs baseline: 1.0972x; 1.0972x over previous
"""AttnOutputDecoder Trainium2 kernel.

Sharding: data-parallel over batch B=16 across 8 cores (2 batches/core).
Per core: LSTM (gate order i,f,o,g; host-precomputed x@W_ih injected into
PSUM via identity matmul) overlapped with Bahdanau attention scores.
Score tanh-args wh[s]+ws[t] are built as a single fp8 DoubleRow matmul per
256-col chunk: lhsT packs [wh-chunk ; ws-rows] in fp8 (accuracy verified:
adds ~2e-3 rel err), rhs is an exact 0/1 selector matrix in fp8, so each
element is written once at 0.5 cyc/row instead of twice at 1.0. Ws_b is
folded into the ws tile copy. Output projection streams Vp_w.T in bf16
(fp8 fails the 2e-2 gate); logits stored bf16; Vp_b added on host.
"""

import numpy as np
import ml_dtypes

import concourse.bass as bass
import concourse.mybir as mybir
import concourse.tile as tile
from concourse import bacc
from concourse import bass_utils

BF16 = ml_dtypes.bfloat16
FP8 = ml_dtypes.float8_e4m3
F32 = mybir.dt.float32
BF = mybir.dt.bfloat16
F8 = mybir.dt.float8e4
AF = mybir.ActivationFunctionType
ALU = mybir.AluOpType
DR = mybir.MatmulPerfMode.DoubleRow

B, T, S, D, V = 16, 64, 128, 512, 32000
NC = 8
BL = B // NC          # local batches per core = 2
R = BL * T            # local rows = 128
G4 = 4 * D            # 2048 gates
KC = D // 128         # 4 contraction chunks
TS = 8                # score t-sub-block
CC = 256              # su DoubleRow column chunk
VBLK = 512
NBLK = (V + VBLK - 1) // VBLK   # 63
NPRE = 24             # prefetched vocab blocks
NSTRB = 6             # streamed-vocab buffer slots (1 block each)
WAITP = 2000          # pacing period hint (ns/step) for score tasks
WAITB = 7000          # pacing base offset (ns)

_cached = {}


def _build_nc(stage=3):
    # stage 1: LSTM only; 2: + scores/softmax/out2; 3: full (vocab)
    nc = bacc.Bacc("TRN2", target_bir_lowering=False, debug=False,
                   num_devices=NC)

    def din(name, shape, dt):
        return nc.dram_tensor(name, shape, dt, kind="ExternalInput").ap()

    t_ident = din("ident", [128, 128], BF)
    t_h0 = din("h0", [128, KC * BL], BF)
    t_c0 = din("c0", [128, KC * BL], F32)
    t_whha = din("whha", [128, 8 * KC * 128], BF)    # [p,(j<8,kc,g)]
    t_whhb = din("whhb", [128, 8 * KC * 128], BF)    # [p,(j>=8,kc,g)]
    t_xg = din("xg", [128, T * 16 * BL], BF)         # [p,(t,j,b)]
    t_encT = din("encT", [128, KC * BL * S], BF)     # [p,(kc,b,s)]
    t_whT = din("whT", [128, KC * D], BF)            # Wh_w.T re
    t_wsT = din("wsT", [128, KC * D], BF)            # Ws_w.T re
    t_wsb8 = din("wsb8", [TS, D], BF)                # Ws_b row x TS
    t_vt = din("vt", [128, KC], BF)
    t_selv = din("selv", [128, 2 * TS * S], F8)      # DR selectors
    t_enc = din("enc", [128, BL * D], BF)            # [s,(b,d)]
    t_vw1 = din("vw1", [128, KC * D], BF)
    t_vw2 = din("vw2", [128, KC * D], BF)
    t_vb = din("vb", [128, KC], F32)
    t_vpt = din("vpt", [128, KC * V], BF)            # [p,(kc,v)] Vp_w.T re
    t_out = nc.dram_tensor("out", [R, V], BF, kind="ExternalOutput").ap()

    with tile.TileContext(nc) as tc:
        with (
            tc.tile_pool(name="const", bufs=1) as cp,
            tc.tile_pool(name="state", bufs=1) as sp,
            tc.tile_pool(name="gates", bufs=8) as gp,
            tc.tile_pool(name="attn", bufs=2) as ap_,
            tc.tile_pool(name="thp", bufs=2) as thp,
            tc.tile_pool(name="voc", bufs=2) as vp,
            tc.tile_pool(name="lsbp", bufs=3) as lp,
            tc.tile_pool(name="ps_g", bufs=2, space="PSUM") as ppg,
            tc.tile_pool(name="ps_sum", bufs=2, space="PSUM") as pps,
            tc.tile_pool(name="ps_e", bufs=1, space="PSUM") as ppe,
            tc.tile_pool(name="ps_w", bufs=1, space="PSUM") as ppw,
        ):
            # ---- LSTM-critical loads first (serial DMA device) ----
            ident = cp.tile([128, 128], BF)
            nc.sync.dma_start(out=ident[:], in_=t_ident[:])
            h0b = sp.tile([128, KC * BL], BF)
            nc.sync.dma_start(out=h0b[:], in_=t_h0[:])
            c = sp.tile([128, KC * BL], F32)
            nc.sync.dma_start(out=c[:], in_=t_c0[:])
            whha = cp.tile([128, 8 * KC * 128], BF)
            nc.sync.dma_start(out=whha[:], in_=t_whha[:])
            xg = cp.tile([128, T * 16 * BL], BF)
            nc.sync.dma_start(out=xg[:], in_=t_xg[:])
            whhb = cp.tile([128, 8 * KC * 128], BF)
            nc.sync.dma_start(out=whhb[:], in_=t_whhb[:])
            # ---- wh deps, then remaining constants ----
            encT = cp.tile([128, KC * BL * S], BF)
            nc.sync.dma_start(out=encT[:], in_=t_encT[:])
            whT = cp.tile([128, KC * D], BF)
            nc.sync.dma_start(out=whT[:], in_=t_whT[:])
            wsT = cp.tile([128, KC * D], BF)
            nc.sync.dma_start(out=wsT[:], in_=t_wsT[:])
            wsb8 = cp.tile([TS, D], BF)
            nc.sync.dma_start(out=wsb8[:], in_=t_wsb8[:])
            vt = cp.tile([128, KC], BF)
            nc.sync.dma_start(out=vt[:], in_=t_vt[:])
            selv = cp.tile([128, 2 * TS * S], F8)
            nc.sync.dma_start(out=selv[:], in_=t_selv[:])
            enc = cp.tile([128, BL * D], BF)
            nc.sync.dma_start(out=enc[:], in_=t_enc[:])
            vw1 = cp.tile([128, KC * D], BF)
            nc.sync.dma_start(out=vw1[:], in_=t_vw1[:])
            vw2 = cp.tile([128, KC * D], BF)
            nc.sync.dma_start(out=vw2[:], in_=t_vw2[:])
            vb = cp.tile([128, KC], F32)
            nc.sync.dma_start(out=vb[:], in_=t_vb[:])

            encT4 = encT[:].rearrange("p (kc b s) -> p kc b s", kc=KC, b=BL)
            whT4 = whT[:].rearrange("p (kc d) -> p kc d", kc=KC)
            wsT4 = wsT[:].rearrange("p (kc d) -> p kc d", kc=KC)
            selv3 = selv[:].rearrange("p (i n) -> p i n", i=2)

            outT = sp.tile([128, KC * BL * T], BF)   # [p,(kc,b,t)] all h's
            outT4 = outT[:].rearrange("p (kc b t) -> p kc b t", kc=KC, b=BL,
                                      t=T)

            # ---- vocab weight prefetch: chunked large DMAs ----
            vpt4 = t_vpt[:].rearrange("p (kc v) -> p kc v", kc=KC, v=V)
            vpre = cp.tile([128, KC * NPRE * VBLK], BF)
            vpre4 = vpre[:].rearrange("p (kc v) -> p kc v", kc=KC,
                                      v=NPRE * VBLK)
            PCH = 7
            for pc in range(0, NPRE, PCH):
                pe_ = min(NPRE, pc + PCH)
                nc.sync.dma_start(out=vpre4[:, :, pc * VBLK:pe_ * VBLK],
                                  in_=vpt4[:, :, pc * VBLK:pe_ * VBLK])

            # ====== wh[s, d] chunks -> wsu slot0 (fp8), ws_b folded to ws ==
            # wsu[(b,par)]: [128, kc, 2, 128] fp8; slot0 = wh (s-partition),
            # slot1 rows 0..TS = ws rows of t-block (t-partition), rest zero.
            wsu_tiles = {}
            for b_ in range(BL):
                for par in range(2):
                    w_ = sp.tile([128, KC, 2, 128], F8, tag=f"wsu{b_}_{par}")
                    wsu_tiles[(b_, par)] = w_
                    nc.gpsimd.memset(w_[:, :, 1, :], 0)
            for b_ in range(BL):
                whp = ppw.tile([S, D], F32, tag="ws", name=f"whp{b_}")
                for kc in range(KC):
                    nc.tensor.matmul(out=whp[:], lhsT=encT4[:, kc, b_, :],
                                     rhs=whT4[:, kc, :],
                                     start=(kc == 0), stop=(kc == KC - 1))
                whp3 = whp[:].rearrange("p (kc d) -> p kc d", kc=KC)
                for par in range(2):
                    nc.vector.tensor_copy(
                        out=wsu_tiles[(b_, par)][:, :, 0, :], in_=whp3[:, :, :])

            eps = ppe.tile([S, BL * T], F32, tag="eps")   # scores [s,(b,t)]

            # ========== LSTM with interleaved score tasks ==========
            # task_a: fp8 DoubleRow outer-sum + ACT tanh (lags LSTM);
            # task_b: eps dot matmuls, emitted later still.
            pend_a = []
            pend_b = []
            th_tiles = {}

            def emit_a(b, tsub, dc):
                su = pps.tile([128, TS * S], F32, tag="sum",
                              name=f"su{b}_{tsub}_{dc}")
                wsu = wsu_tiles[(b, tsub % 2)]
                for cc in range(TS * S // CC):
                    nc.tensor.matmul(out=su[:, cc * CC:(cc + 1) * CC],
                                     lhsT=wsu[:, dc, :, :],
                                     rhs=selv3[:, :, cc * CC:(cc + 1) * CC],
                                     start=True, stop=True, perf_mode=DR)
                th = thp.tile([128, TS * S], BF, tag=f"th{dc}",
                              name=f"th{b}_{tsub}_{dc}")
                nc.scalar.activation(out=th[:], in_=su[:], func=AF.Tanh)
                th_tiles[(b, tsub, dc)] = th
                if dc == KC - 1:
                    pend_b.append((b, tsub))

            def emit_b(b, tsub, half):
                for tl in range(half * (TS // 2), (half + 1) * (TS // 2)):
                    t = tsub * TS + tl
                    for d2 in range(KC):
                        nc.tensor.matmul(
                            out=eps[:, b * T + t: b * T + t + 1],
                            lhsT=th_tiles[(b, tsub, d2)][:,
                                tl * S:(tl + 1) * S],
                            rhs=vt[:, d2:d2 + 1],
                            start=(d2 == 0), stop=(d2 == KC - 1))

            def pump():
                if pend_a:
                    emit_a(*pend_a.pop(0))
                if pend_b:
                    b, tsub = pend_b[0]
                    half = pump.half
                    emit_b(b, tsub, half)
                    if half == 1:
                        pend_b.pop(0)
                    pump.half = 1 - half
            pump.half = 0

            from contextlib import ExitStack
            for t in range(T):
                _hp = ExitStack()
                _hp.enter_context(tc.high_priority())
                gps = ppg.tile([128, 16 * BL], F32, tag="gps",
                               name=f"gps{t}")
                nc.tensor.matmul(out=gps[:], lhsT=ident[:],
                                 rhs=xg[:, t * 32:(t + 1) * 32],
                                 start=True, stop=False,
                                 skip_group_check=True)
                for j in range(16):
                    wt = whha if j < 8 else whhb
                    jr = j % 8
                    for kc in range(KC):
                        if t == 0:
                            hsrc = h0b[:, kc * BL:(kc + 1) * BL]
                        else:
                            hsrc = outT4[:, kc, :, t - 1]
                        nc.tensor.matmul(
                            out=gps[:, j * BL:(j + 1) * BL],
                            lhsT=wt[:, (jr * KC + kc) * 128:
                                    (jr * KC + kc + 1) * 128],
                            rhs=hsrc, start=False, stop=(kc == KC - 1),
                            skip_group_check=True)
                sio = gp.tile([128, 16 * BL], F32, tag="sio")
                nc.scalar.activation(out=sio[:, 0:12 * BL],
                                     in_=gps[:, 0:12 * BL], func=AF.Sigmoid)
                nc.scalar.activation(out=sio[:, 12 * BL:16 * BL],
                                     in_=gps[:, 12 * BL:16 * BL],
                                     func=AF.Tanh)
                t1 = gp.tile([128, KC * BL], F32, tag="t1")
                t2 = gp.tile([128, KC * BL], F32, tag="t2")
                nc.vector.tensor_mul(out=t1[:], in0=sio[:, 4 * BL:8 * BL],
                                     in1=c[:])
                nc.vector.tensor_mul(out=t2[:], in0=sio[:, 0:4 * BL],
                                     in1=sio[:, 12 * BL:16 * BL])
                nc.vector.tensor_add(out=c[:], in0=t1[:], in1=t2[:])
                tc_ = gp.tile([128, KC * BL], F32, tag="tc")
                nc.scalar.activation(out=tc_[:], in_=c[:], func=AF.Tanh)
                nc.vector.tensor_mul(out=outT4[:, :, :, t],
                                     in0=sio[:, 8 * BL:12 * BL], in1=tc_[:])
                _hp.close()

                with tc.tile_wait_until((WAITB + WAITP * t) / 1e6):
                    pump()

                if stage >= 2 and t % TS == TS - 1:
                    tsub = t // TS
                    tc.tile_set_cur_wait((WAITB + WAITP * t) / 1e6)
                    for b in range(BL):
                        wps = ppw.tile([TS, D], F32, tag="ws",
                                       name=f"wps{b}_{tsub}")
                        for kc in range(KC):
                            nc.tensor.matmul(
                                out=wps[:],
                                lhsT=outT[:, (kc * BL + b) * T + tsub * TS:
                                          (kc * BL + b) * T + tsub * TS + TS],
                                rhs=wsT4[:, kc, :],
                                start=(kc == 0), stop=(kc == KC - 1))
                        wps3 = wps[:].rearrange("p (kc d) -> p kc d", kc=KC)
                        wsb3 = wsb8[:].rearrange("p (kc d) -> p kc d", kc=KC)
                        nc.vector.tensor_add(
                            out=wsu_tiles[(b, tsub % 2)][0:TS, :, 1, :],
                            in0=wps3[:, :, :], in1=wsb3[:, :, :])
                    tc.cur_wait_ts = 0
                    pend_a.extend((b, tsub, dc) for b in range(BL)
                                  for dc in range(KC))

            while pend_a or pend_b:
                pump()

            # ============ softmax, context, out2 ============
            ctxT = sp.tile([128, BL * KC * T], BF)   # [p,(b,dc,t)]
            o2T = sp.tile([128, KC * BL * T], BF)    # [p,(ec,b,t)]
            _hp2 = ExitStack()
            _hp2.enter_context(tc.high_priority())
            for b in range(BL if stage >= 2 else 0):
                ebf = ap_.tile([S, T], BF, tag="ebf")
                nc.scalar.activation(out=ebf[:],
                                     in_=eps[:, b * T:(b + 1) * T],
                                     func=AF.Exp)
                etp = ppw.tile([T, S], BF, tag="ws", name=f"etp{b}")
                nc.tensor.transpose(out=etp[:], in_=ebf[:],
                                    identity=ident[:, :])
                ssum = ap_.tile([T, 1], F32, tag="ssum")
                nc.vector.tensor_reduce(out=ssum[:], in_=etp[:],
                                        axis=mybir.AxisListType.X, op=ALU.add)
                rsum = ap_.tile([T, 1], F32, tag="rsum")
                nc.vector.reciprocal(out=rsum[:], in_=ssum[:])
                abf = ap_.tile([T, S], BF, tag="abf")
                nc.vector.tensor_scalar_mul(out=abf[:], in0=etp[:],
                                            scalar1=rsum[:])
                atp = ppw.tile([S, T], BF, tag="ws", name=f"atp{b}")
                nc.tensor.transpose(out=atp[:], in_=abf[:],
                                    identity=ident[0:T, 0:T])
                atb = ap_.tile([S, T], BF, tag="atb")
                nc.vector.tensor_copy(out=atb[:], in_=atp[:])
                for dc in range(KC):
                    cps = ppg.tile([128, T], F32, tag="gps",
                                   name=f"cps{b}_{dc}")
                    nc.tensor.matmul(out=cps[:],
                                     lhsT=enc[:, b * D + dc * 128:
                                              b * D + (dc + 1) * 128],
                                     rhs=atb[:], start=True, stop=True)
                    nc.vector.tensor_copy(
                        out=ctxT[:, (b * KC + dc) * T:(b * KC + dc + 1) * T],
                        in_=cps[:])
                for ec in range(KC):
                    ops = ppg.tile([128, T], F32, tag="gps",
                                   name=f"ops{b}_{ec}")
                    for kc in range(KC):
                        nc.tensor.matmul(
                            out=ops[:],
                            lhsT=vw1[:, kc * D + ec * 128:
                                     kc * D + (ec + 1) * 128],
                            rhs=ctxT[:, (b * KC + kc) * T:
                                     (b * KC + kc + 1) * T],
                            start=(kc == 0), stop=False)
                    for kc in range(KC):
                        nc.tensor.matmul(
                            out=ops[:],
                            lhsT=vw2[:, kc * D + ec * 128:
                                     kc * D + (ec + 1) * 128],
                            rhs=outT[:, (kc * BL + b) * T:
                                     (kc * BL + b + 1) * T],
                            start=False, stop=(kc == KC - 1))
                    nc.vector.tensor_scalar(
                        out=o2T[:, (ec * BL + b) * T:(ec * BL + b) * T + T],
                        in0=ops[:], scalar1=vb[:, ec:ec + 1], scalar2=None,
                        op0=ALU.add)

            # ================= vocab projection (bf16) ==========
            _hp2.close()
            o2r = o2T[:].rearrange("p (e c) -> p e c", e=KC, c=128)
            # Recycle dead phase-1 SBUF slots as stream buffers for the tail
            # vocab blocks: their DMAs fire as soon as the old tiles' readers
            # retire, moving DMA out of the saturated vocab tail.
            NRG = 4                      # blocks in the whh-slot group
            RS1 = ["xg", "whT", "wsT", "vw1", "vw2"]
            # 2KB slots that free when the last score task retires; pairs
            # stage one block each as two [128,2,512] half-tiles
            RS2 = [("th0", "th1"), ("th2", "th3"), ("th0", "th1"),
                   ("th2", "th3"), ("selv", "encT")]
            rg0 = NBLK - NRG - len(RS1)  # whh group covers rg0..rg0+3
            rs2_0 = rg0 - len(RS2)
            rs3_0 = rs2_0 - 2            # wsu-quad + outT/ctxT/wsb8 blocks
            rsrc = {}
            if stage >= 3:
                # whha+whhb slots: 8 KB each -> two blocks each
                for gi, wtag in enumerate(("whha", "whhb")):
                    ib_g = rg0 + gi * 2
                    vg = cp.tile([128, KC, 2 * VBLK], BF, tag=wtag,
                                 name=f"vgrp{gi}")
                    nc.sync.dma_start(
                        out=vg[:, :, :],
                        in_=vpt4[:, :, ib_g * VBLK:(ib_g + 2) * VBLK])
                    rsrc[ib_g] = (vg, 0)
                    rsrc[ib_g + 1] = (vg, VBLK)
                # quad block from the four 1KB wsu slots (kc0..3)
                ibx = rs3_0
                vqs = []
                for b_ in range(BL):
                    for par in range(2):
                        vq = sp.tile([128, 1, VBLK], BF,
                                     tag=f"wsu{b_}_{par}",
                                     name=f"vq{b_}_{par}")
                        kcq = b_ * 2 + par
                        nc.sync.dma_start(
                            out=vq[:, :, :],
                            in_=vpt4[:, kcq:kcq + 1,
                                     ibx * VBLK:(ibx + 1) * VBLK])
                        vqs.append((vq, 0))
                rsrc[ibx] = ("quad", vqs)
                # quad block from outT (1KB) + ctxT (1KB) + enc (2KB) slots
                iby = rs3_0 + 1
                vq_o = sp.tile([128, 1, VBLK], BF, tag="outT", name="vqo")
                nc.sync.dma_start(out=vq_o[:, :, :],
                                  in_=vpt4[:, 0:1, iby * VBLK:
                                           (iby + 1) * VBLK])
                vq_c = sp.tile([128, 1, VBLK], BF, tag="ctxT", name="vqc")
                nc.sync.dma_start(out=vq_c[:, :, :],
                                  in_=vpt4[:, 1:2, iby * VBLK:
                                           (iby + 1) * VBLK])
                vq_e2 = cp.tile([128, 2, VBLK], BF, tag="enc", name="vqe2")
                nc.sync.dma_start(out=vq_e2[:, :, :],
                                  in_=vpt4[:, 2:4, iby * VBLK:
                                           (iby + 1) * VBLK])
                rsrc[iby] = ("quad", [(vq_o, 0), (vq_c, 0),
                                      (vq_e2, 0), (vq_e2, 1)])
                for i, (tga, tgb) in enumerate(RS2):
                    ib_r = rs2_0 + i
                    v0r = ib_r * VBLK
                    pa = thp if tga.startswith("th") else cp
                    pb = thp if tgb.startswith("th") else cp
                    ta = pa.tile([128, 2, VBLK], BF, tag=tga,
                                 name=f"vspl{ib_r}a")
                    nc.sync.dma_start(out=ta[:, :, :],
                                      in_=vpt4[:, 0:2, v0r:v0r + VBLK])
                    tb = pb.tile([128, 2, VBLK], BF, tag=tgb,
                                 name=f"vspl{ib_r}b")
                    nc.sync.dma_start(out=tb[:, :, :],
                                      in_=vpt4[:, 2:4, v0r:v0r + VBLK])
                    rsrc[ib_r] = ("split", ta, tb)
                for i, tg in enumerate(RS1):
                    ib_r = rg0 + NRG + i
                    v0r = ib_r * VBLK
                    wr = min(VBLK, V - v0r)
                    vrt = cp.tile([128, KC, VBLK], BF, tag=tg,
                                  name=f"vrt{ib_r}")
                    nc.sync.dma_start(out=vrt[:, :, :wr],
                                      in_=vpt4[:, :, v0r:v0r + wr])
                    rsrc[ib_r] = (vrt, 0)
            # consumption order: alternate streamed/prefetched so stream DMAs
            # never stall; recycled-slot blocks go last (data arrives
            # mid-phase)
            tail0 = rs3_0 - (rs3_0 % 4)   # align tail to lsb store groups
            order = []
            si, pi = NPRE, 0
            while si < tail0 or pi < NPRE:
                if si < tail0:
                    order.append(si)
                    si += 1
                if pi < NPRE:
                    order.append(pi)
                    pi += 1
            order += list(range(tail0, NBLK))
            lsb_tiles = {}
            if stage < 3:
                order = []
            for nb, ib in enumerate(order):
                v0 = ib * VBLK
                w = min(VBLK, V - v0)
                if ib < NPRE:
                    def rhs_of(kc, v0=v0, w=w):
                        return vpre4[:, kc, v0:v0 + w]
                elif ib in rsrc:
                    ent = rsrc[ib]
                    if ent[0] == "split":
                        def rhs_of(kc, ta=ent[1], tb=ent[2], w=w):
                            return (ta if kc < 2 else tb)[:, kc % 2, 0:w]
                    elif ent[0] == "quad":
                        def rhs_of(kc, lst=ent[1], w=w):
                            t_, ix = lst[kc]
                            return t_[:, ix, 0:w]
                    else:
                        def rhs_of(kc, vs3=ent[0], voff=ent[1], w=w):
                            return vs3[:, kc, voff:voff + w]
                else:
                    vst = vp.tile([128, KC, VBLK], BF, tag="vs", bufs=NSTRB,
                                  name=f"vst{ib}")
                    nc.sync.dma_start(out=vst[:, :, :w],
                                      in_=vpt4[:, :, v0:v0 + w])
                    def rhs_of(kc, vst=vst, w=w):
                        return vst[:, kc, 0:w]
                grp = ib // 4
                if grp not in lsb_tiles:
                    lsb_tiles[grp] = [lp.tile([128, 4 * VBLK], BF, tag="lsb",
                                              name=f"lsb{grp}"), 0]
                lsb_e = lsb_tiles[grp]
                lps = ppg.tile([128, VBLK], F32, tag="gps", name=f"lps{ib}")
                for kc in range(KC):
                    nc.tensor.matmul(out=lps[:, :w],
                                     lhsT=o2r[:, kc, :],
                                     rhs=rhs_of(kc),
                                     start=(kc == 0), stop=(kc == KC - 1))
                dst = lsb_e[0][:, (ib % 4) * VBLK:(ib % 4) * VBLK + w]
                if nb % 2 == 0:
                    nc.scalar.copy(out=dst, in_=lps[:, :w])
                else:
                    nc.vector.tensor_copy(out=dst, in_=lps[:, :w])
                lsb_e[1] += 1
                nblk_grp = min(4, NBLK - grp * 4)
                if lsb_e[1] == nblk_grp:
                    gv0 = grp * 4 * VBLK
                    wlen = min(4 * VBLK, V - gv0)
                    nc.sync.dma_start(out=t_out[:, gv0:gv0 + wlen],
                                      in_=lsb_e[0][:, :wlen])

    nc.compile()
    return nc


def _prep_in_maps(inputs):
    inp = {k: np.asarray(v) for k, v in inputs.items()}
    words = inp["words"].astype(np.int64)
    enc = inp["encoder_output"].astype(np.float32)
    pre_h, cell = inp["pre_h"], inp["cell"]
    emb = inp["emb"]
    W_ih, W_hh = inp["W_ih"], inp["W_hh"]
    b_ih, b_hh = inp["b_ih"], inp["b_hh"]
    Wh_w = inp["Wh_w"]
    Ws_w, Ws_b = inp["Ws_w"], inp["Ws_b"]
    vt_w = inp["vt_w"]
    V_w, V_b = inp["V_w"], inp["V_b"]
    Vp_w, Vp_b = inp["Vp_w"], inp["Vp_b"]

    def re_lhsT(m, dt=BF16):  # [512, N] -> [128, 4*N] chunk-major
        n = m.shape[1]
        return np.ascontiguousarray(
            m.reshape(4, 128, n).transpose(1, 0, 2).reshape(128, 4 * n)
        ).astype(dt)

    # gate reorder (i,f,g,o) -> (i,f,o,g)
    perm = np.r_[0:512, 512:1024, 1536:2048, 1024:1536]
    W_ih_p, W_hh_p = W_ih[perm], W_hh[perm]
    b2 = (b_ih + b_hh)[perm].astype(np.float32)

    whh_re = re_lhsT(np.ascontiguousarray(W_hh_p.T))     # [p,(kc,g)]
    # -> j-major [p,(j,kc,128)]
    whh_j = np.ascontiguousarray(
        whh_re.reshape(128, KC, 16, 128).transpose(0, 2, 1, 3)
        .reshape(128, 16 * KC * 128))
    whha_re = np.ascontiguousarray(whh_j[:, :8 * KC * 128])
    whhb_re = np.ascontiguousarray(whh_j[:, 8 * KC * 128:])
    whT_re = re_lhsT(np.ascontiguousarray(Wh_w.T))
    wsT_re = re_lhsT(np.ascontiguousarray(Ws_w.T))
    vw1_re = re_lhsT(np.ascontiguousarray(V_w[:, :D].T))
    vw2_re = re_lhsT(np.ascontiguousarray(V_w[:, D:].T))
    vpt_re = re_lhsT(np.ascontiguousarray(Vp_w.T))
    wsb8_re = np.tile(Ws_b.reshape(1, D), (TS, 1)).astype(BF16)
    vb_re = np.ascontiguousarray(V_b.reshape(4, 128).T).astype(np.float32)
    vt_re = np.ascontiguousarray(vt_w.reshape(4, 128).T).astype(BF16)
    ident_re = np.eye(128, dtype=np.float32).astype(BF16)
    # DR selector: slot0[p,(tl,s)] = [p==s], slot1[p,(tl,s)] = [p==tl]
    sel0 = np.tile(np.eye(128, dtype=np.float32), (1, TS))
    sel1 = np.zeros((128, TS * S), dtype=np.float32)
    for tl in range(TS):
        sel1[tl, tl * S:(tl + 1) * S] = 1.0
    selv_re = np.concatenate([sel0, sel1], axis=1).astype(FP8)

    x_all = emb[words]                                   # [B,T,D]
    xg_all = x_all @ W_ih_p.T.astype(np.float32) + b2    # [B,T,4D]

    in_maps = []
    for k in range(NC):
        bs = slice(k * BL, (k + 1) * BL)
        xgl = xg_all[bs]                                 # [2,T,2048]
        xg_re = np.ascontiguousarray(
            xgl.reshape(BL, T, 16, 128).transpose(3, 1, 2, 0)
            .reshape(128, T * 16 * BL)).astype(BF16)     # [p,(t,j,b)]
        h0 = np.ascontiguousarray(
            pre_h[bs].reshape(BL, 4, 128).transpose(2, 1, 0)
            .reshape(128, 4 * BL)).astype(BF16)
        c0 = np.ascontiguousarray(
            cell[bs].reshape(BL, 4, 128).transpose(2, 1, 0)
            .reshape(128, 4 * BL)).astype(np.float32)
        encl = enc[bs]                                   # [2,S,D]
        encT_re = np.ascontiguousarray(
            encl.reshape(BL, S, 4, 128).transpose(3, 2, 0, 1)
            .reshape(128, 4 * BL * S)).astype(BF16)
        enc_re = np.ascontiguousarray(
            encl.transpose(1, 0, 2).reshape(S, BL * D)).astype(BF16)
        in_maps.append({
            "xg": xg_re, "whha": whha_re, "whhb": whhb_re, "h0": h0,
            "c0": c0, "encT": encT_re, "enc": enc_re, "whT": whT_re,
            "wsT": wsT_re, "wsb8": wsb8_re, "vt": vt_re, "selv": selv_re,
            "vw1": vw1_re, "vw2": vw2_re, "vb": vb_re, "vpt": vpt_re,
            "ident": ident_re,
        })
    return in_maps


def kernel(**inputs):
    in_maps = _prep_in_maps(inputs)
    if "nc" not in _cached:
        _cached["nc"] = _build_nc()
    res = bass_utils.run_bass_kernel_spmd(_cached["nc"], in_maps,
                                          core_ids=list(range(NC)))
    vpb = np.asarray(inputs["Vp_b"]).astype(np.float32)
    outs = [np.asarray(res.results[k]["out"]).astype(np.float32)
            .reshape(BL, T, V) for k in range(NC)]
    return np.concatenate(outs, axis=0) + vpb[None, None, :]


if __name__ == "__main__":
    pass


# revision 22
# speedup vs baseline: 1.1772x; 1.0728x over previous
"""AttnOutputDecoder Trainium2 kernel.

Sharding: data-parallel over batch B=16 across 8 cores (2 batches/core).
Per core: LSTM (gate order i,f,o,g; host-precomputed x@W_ih injected into
PSUM via identity matmul) overlapped with Bahdanau attention scores.
Score tanh-args wh[s]+ws[t] are built as a single fp8 DoubleRow matmul per
256-col chunk: lhsT packs [wh-chunk ; ws-rows] in fp8 (accuracy verified:
adds ~2e-3 rel err), rhs is an exact 0/1 selector matrix in fp8, so each
element is written once at 0.5 cyc/row instead of twice at 1.0. Ws_b is
folded into the ws tile copy. Output projection streams Vp_w.T in bf16
(fp8 fails the 2e-2 gate); logits stored bf16; Vp_b added on host.
"""

import numpy as np
import ml_dtypes

import concourse.bass as bass
import concourse.mybir as mybir
import concourse.tile as tile
from concourse import bacc
from concourse import bass_utils

BF16 = ml_dtypes.bfloat16
FP8 = ml_dtypes.float8_e4m3
F32 = mybir.dt.float32
BF = mybir.dt.bfloat16
F8 = mybir.dt.float8e4
AF = mybir.ActivationFunctionType
ALU = mybir.AluOpType
DR = mybir.MatmulPerfMode.DoubleRow

B, T, S, D, V = 16, 64, 128, 512, 32000
NC = 8
BL = B // NC          # local batches per core = 2
R = BL * T            # local rows = 128
G4 = 4 * D            # 2048 gates
KC = D // 128         # 4 contraction chunks
TS = 8                # score t-sub-block
CC = 256              # su DoubleRow column chunk
VBLK = 512
NBLK = (V + VBLK - 1) // VBLK   # 63
NPRE = 24             # prefetched vocab blocks
NSTRB = 6             # streamed-vocab buffer slots (1 block each)
WAITP = 2400          # pacing period hint (ns/step) for score tasks
WAITB = 9500          # pacing base offset (ns)
WAITD = 700           # extra tanh release delay past its su matmuls (ns)

_cached = {}


def _build_nc(stage=3):
    # stage 1: LSTM only; 2: + scores/softmax/out2; 3: full (vocab)
    nc = bacc.Bacc("TRN2", target_bir_lowering=False, debug=False,
                   num_devices=NC)

    def din(name, shape, dt):
        return nc.dram_tensor(name, shape, dt, kind="ExternalInput").ap()

    t_ident = din("ident", [128, 128], BF)
    t_h0 = din("h0", [128, KC * BL], BF)
    t_c0 = din("c0", [128, KC * BL], F32)
    t_whha = din("whha", [128, 8 * KC * 128], BF)    # [p,(j<8,kc,g)]
    t_whhb = din("whhb", [128, 8 * KC * 128], BF)    # [p,(j>=8,kc,g)]
    t_xg = din("xg", [128, T * 16 * BL], BF)         # [p,(t,j,b)]
    t_encT = din("encT", [128, KC * BL * S], BF)     # [p,(kc,b,s)]
    t_whT = din("whT", [128, KC * D], BF)            # Wh_w.T re
    t_wsT = din("wsT", [128, KC * D], BF)            # Ws_w.T re
    t_wsb8 = din("wsb8", [TS, D], BF)                # Ws_b row x TS
    t_vt = din("vt", [128, KC], BF)
    t_selv = din("selv", [128, 2 * TS * S], F8)      # DR selectors
    t_enc = din("enc", [128, BL * D], BF)            # [s,(b,d)]
    t_vw1 = din("vw1", [128, KC * D], BF)
    t_vw2 = din("vw2", [128, KC * D], BF)
    t_vb = din("vb", [128, KC], F32)
    t_vpt = din("vpt", [128, KC * V], BF)            # [p,(kc,v)] Vp_w.T re
    t_out = nc.dram_tensor("out", [R, V], BF, kind="ExternalOutput").ap()

    with tile.TileContext(nc) as tc:
        with (
            tc.tile_pool(name="const", bufs=1) as cp,
            tc.tile_pool(name="state", bufs=1) as sp,
            tc.tile_pool(name="gates", bufs=8) as gp,
            tc.tile_pool(name="attn", bufs=2) as ap_,
            tc.tile_pool(name="thp", bufs=2) as thp,
            tc.tile_pool(name="voc", bufs=2) as vp,
            tc.tile_pool(name="lsbp", bufs=3) as lp,
            tc.tile_pool(name="ps_g", bufs=2, space="PSUM") as ppg,
            tc.tile_pool(name="ps_sum", bufs=2, space="PSUM") as pps,
            tc.tile_pool(name="ps_e", bufs=1, space="PSUM") as ppe,
            tc.tile_pool(name="ps_w", bufs=1, space="PSUM") as ppw,
        ):
            # ---- LSTM-critical loads first (serial DMA device) ----
            ident = cp.tile([128, 128], BF)
            nc.sync.dma_start(out=ident[:], in_=t_ident[:])
            h0b = sp.tile([128, KC * BL], BF)
            nc.sync.dma_start(out=h0b[:], in_=t_h0[:])
            c = sp.tile([128, KC * BL], F32)
            nc.sync.dma_start(out=c[:], in_=t_c0[:])
            whha = cp.tile([128, 8 * KC * 128], BF)
            nc.sync.dma_start(out=whha[:], in_=t_whha[:])
            xg = cp.tile([128, T * 16 * BL], BF)
            nc.sync.dma_start(out=xg[:], in_=t_xg[:])
            whhb = cp.tile([128, 8 * KC * 128], BF)
            nc.sync.dma_start(out=whhb[:], in_=t_whhb[:])
            # ---- wh deps, then remaining constants ----
            encT = cp.tile([128, KC * BL * S], BF)
            nc.sync.dma_start(out=encT[:], in_=t_encT[:])
            whT = cp.tile([128, KC * D], BF)
            nc.sync.dma_start(out=whT[:], in_=t_whT[:])
            wsT = cp.tile([128, KC * D], BF)
            nc.sync.dma_start(out=wsT[:], in_=t_wsT[:])
            wsb8 = cp.tile([TS, D], BF)
            nc.sync.dma_start(out=wsb8[:], in_=t_wsb8[:])
            vt = cp.tile([128, KC], BF)
            nc.sync.dma_start(out=vt[:], in_=t_vt[:])
            selv = cp.tile([128, 2 * TS * S], F8)
            nc.sync.dma_start(out=selv[:], in_=t_selv[:])
            enc = cp.tile([128, BL * D], BF)
            nc.sync.dma_start(out=enc[:], in_=t_enc[:])
            vw1 = cp.tile([128, KC * D], BF)
            nc.sync.dma_start(out=vw1[:], in_=t_vw1[:])
            vw2 = cp.tile([128, KC * D], BF)
            nc.sync.dma_start(out=vw2[:], in_=t_vw2[:])
            vb = cp.tile([128, KC], F32)
            nc.sync.dma_start(out=vb[:], in_=t_vb[:])

            encT4 = encT[:].rearrange("p (kc b s) -> p kc b s", kc=KC, b=BL)
            whT4 = whT[:].rearrange("p (kc d) -> p kc d", kc=KC)
            wsT4 = wsT[:].rearrange("p (kc d) -> p kc d", kc=KC)
            selv3 = selv[:].rearrange("p (i n) -> p i n", i=2)

            outT = sp.tile([128, KC * BL * T], BF)   # [p,(kc,b,t)] all h's
            outT4 = outT[:].rearrange("p (kc b t) -> p kc b t", kc=KC, b=BL,
                                      t=T)

            # ---- vocab weight prefetch: chunked large DMAs ----
            vpt4 = t_vpt[:].rearrange("p (kc v) -> p kc v", kc=KC, v=V)
            vpre = cp.tile([128, KC * NPRE * VBLK], BF)
            vpre4 = vpre[:].rearrange("p (kc v) -> p kc v", kc=KC,
                                      v=NPRE * VBLK)
            PCH = 7
            for pc in range(0, NPRE, PCH):
                pe_ = min(NPRE, pc + PCH)
                nc.sync.dma_start(out=vpre4[:, :, pc * VBLK:pe_ * VBLK],
                                  in_=vpt4[:, :, pc * VBLK:pe_ * VBLK])

            # ====== wh[s, d] chunks -> wsu slot0 (fp8), ws_b folded to ws ==
            # wsu[(b,par)]: [128, kc, 2, 128] fp8; slot0 = wh (s-partition),
            # slot1 rows 0..TS = ws rows of t-block (t-partition), rest zero.
            wsu_tiles = {}
            for b_ in range(BL):
                for par in range(2):
                    w_ = sp.tile([128, KC, 2, 128], F8, tag=f"wsu{b_}_{par}")
                    wsu_tiles[(b_, par)] = w_
                    nc.gpsimd.memset(w_[:, :, 1, :], 0)
            for b_ in range(BL):
                whp = ppw.tile([S, D], F32, tag="ws", name=f"whp{b_}")
                for kc in range(KC):
                    nc.tensor.matmul(out=whp[:], lhsT=encT4[:, kc, b_, :],
                                     rhs=whT4[:, kc, :],
                                     start=(kc == 0), stop=(kc == KC - 1))
                whp3 = whp[:].rearrange("p (kc d) -> p kc d", kc=KC)
                for par in range(2):
                    nc.vector.tensor_copy(
                        out=wsu_tiles[(b_, par)][:, :, 0, :], in_=whp3[:, :, :])

            eps = ppe.tile([S, BL * T], F32, tag="eps")   # scores [s,(b,t)]

            # ========== LSTM with interleaved score tasks ==========
            # task_a: fp8 DoubleRow outer-sum + ACT tanh (lags LSTM);
            # task_b: eps dot matmuls, emitted later still.
            pend_a = []
            pend_b = []
            th_tiles = {}

            su_tiles = {}

            def emit_su(b, tsub, dc):
                # PE part of a score task: no h-dependency, runs in PE idle
                su = pps.tile([128, TS * S], F32, tag="sum",
                              name=f"su{b}_{tsub}_{dc}")
                wsu = wsu_tiles[(b, tsub % 2)]
                for cc in range(TS * S // CC):
                    nc.tensor.matmul(out=su[:, cc * CC:(cc + 1) * CC],
                                     lhsT=wsu[:, dc, :, :],
                                     rhs=selv3[:, :, cc * CC:(cc + 1) * CC],
                                     start=True, stop=True, perf_mode=DR)
                su_tiles[(b, tsub, dc)] = su
                pend_th.append((b, tsub, dc))

            def emit_th(b, tsub, dc):
                th = thp.tile([128, TS * S], BF, tag=f"th{dc}",
                              name=f"th{b}_{tsub}_{dc}")
                nc.scalar.activation(out=th[:],
                                     in_=su_tiles.pop((b, tsub, dc))[:],
                                     func=AF.Tanh)
                th_tiles[(b, tsub, dc)] = th
                if dc == KC - 1:
                    pend_b.append((b, tsub))

            def emit_b(b, tsub, half):
                for tl in range(half * (TS // 2), (half + 1) * (TS // 2)):
                    t = tsub * TS + tl
                    for d2 in range(KC):
                        nc.tensor.matmul(
                            out=eps[:, b * T + t: b * T + t + 1],
                            lhsT=th_tiles[(b, tsub, d2)][:,
                                tl * S:(tl + 1) * S],
                            rhs=vt[:, d2:d2 + 1],
                            start=(d2 == 0), stop=(d2 == KC - 1))

            pend_th = []

            def pump(hint_ns=None):
                # su one step ahead of its tanh: PE part never blocks ACT
                if pend_th:
                    emit_th(*pend_th.pop(0))
                if pend_a:
                    emit_su(*pend_a.pop(0))
                if pend_b:
                    b, tsub = pend_b[0]
                    half = pump.half
                    emit_b(b, tsub, half)
                    if half == 1:
                        pend_b.pop(0)
                    pump.half = 1 - half
            pump.half = 0

            from contextlib import ExitStack
            for t in range(T):
                _hp = ExitStack()
                _hp.enter_context(tc.high_priority())
                gps = ppg.tile([128, 16 * BL], F32, tag="gps",
                               name=f"gps{t}")
                nc.tensor.matmul(out=gps[:], lhsT=ident[:],
                                 rhs=xg[:, t * 32:(t + 1) * 32],
                                 start=True, stop=False,
                                 skip_group_check=True)
                for j in range(16):
                    wt = whha if j < 8 else whhb
                    jr = j % 8
                    for kc in range(KC):
                        if t == 0:
                            hsrc = h0b[:, kc * BL:(kc + 1) * BL]
                        else:
                            hsrc = outT4[:, kc, :, t - 1]
                        nc.tensor.matmul(
                            out=gps[:, j * BL:(j + 1) * BL],
                            lhsT=wt[:, (jr * KC + kc) * 128:
                                    (jr * KC + kc + 1) * 128],
                            rhs=hsrc, start=False, stop=(kc == KC - 1),
                            skip_group_check=True)
                sio = gp.tile([128, 16 * BL], F32, tag="sio")
                nc.scalar.activation(out=sio[:, 0:12 * BL],
                                     in_=gps[:, 0:12 * BL], func=AF.Sigmoid)
                nc.scalar.activation(out=sio[:, 12 * BL:16 * BL],
                                     in_=gps[:, 12 * BL:16 * BL],
                                     func=AF.Tanh)
                t1 = gp.tile([128, KC * BL], F32, tag="t1")
                t2 = gp.tile([128, KC * BL], F32, tag="t2")
                nc.vector.tensor_mul(out=t1[:], in0=sio[:, 4 * BL:8 * BL],
                                     in1=c[:])
                nc.vector.tensor_mul(out=t2[:], in0=sio[:, 0:4 * BL],
                                     in1=sio[:, 12 * BL:16 * BL])
                nc.vector.tensor_add(out=c[:], in0=t1[:], in1=t2[:])
                tc_ = gp.tile([128, KC * BL], F32, tag="tc")
                nc.scalar.activation(out=tc_[:], in_=c[:], func=AF.Tanh)
                nc.vector.tensor_mul(out=outT4[:, :, :, t],
                                     in0=sio[:, 8 * BL:12 * BL], in1=tc_[:])
                _hp.close()

                with tc.tile_wait_until((WAITB + WAITP * t) / 1e6):
                    pump(hint_ns=WAITB + WAITP * t)

                if stage >= 2 and t % TS == TS - 1:
                    tsub = t // TS
                    tc.tile_set_cur_wait((WAITB + WAITP * t) / 1e6)
                    for b in range(BL):
                        wps = ppw.tile([TS, D], F32, tag="ws",
                                       name=f"wps{b}_{tsub}")
                        for kc in range(KC):
                            nc.tensor.matmul(
                                out=wps[:],
                                lhsT=outT[:, (kc * BL + b) * T + tsub * TS:
                                          (kc * BL + b) * T + tsub * TS + TS],
                                rhs=wsT4[:, kc, :],
                                start=(kc == 0), stop=(kc == KC - 1))
                        wps3 = wps[:].rearrange("p (kc d) -> p kc d", kc=KC)
                        wsb3 = wsb8[:].rearrange("p (kc d) -> p kc d", kc=KC)
                        nc.vector.tensor_add(
                            out=wsu_tiles[(b, tsub % 2)][0:TS, :, 1, :],
                            in0=wps3[:, :, :], in1=wsb3[:, :, :])
                    tc.cur_wait_ts = 0
                    pend_a.extend((b, tsub, dc) for b in range(BL)
                                  for dc in range(KC))

            while pend_a or pend_th or pend_b:
                pump()

            # ============ softmax, context, out2 ============
            ctxT = sp.tile([128, BL * KC * T], BF)   # [p,(b,dc,t)]
            o2T = sp.tile([128, KC * BL * T], BF)    # [p,(ec,b,t)]
            _hp2 = ExitStack()
            _hp2.enter_context(tc.high_priority())
            for b in range(BL if stage >= 2 else 0):
                ebf = ap_.tile([S, T], BF, tag="ebf")
                nc.scalar.activation(out=ebf[:],
                                     in_=eps[:, b * T:(b + 1) * T],
                                     func=AF.Exp)
                etp = ppw.tile([T, S], BF, tag="ws", name=f"etp{b}")
                nc.tensor.transpose(out=etp[:], in_=ebf[:],
                                    identity=ident[:, :])
                ssum = ap_.tile([T, 1], F32, tag="ssum")
                nc.vector.tensor_reduce(out=ssum[:], in_=etp[:],
                                        axis=mybir.AxisListType.X, op=ALU.add)
                rsum = ap_.tile([T, 1], F32, tag="rsum")
                nc.vector.reciprocal(out=rsum[:], in_=ssum[:])
                abf = ap_.tile([T, S], BF, tag="abf")
                nc.vector.tensor_scalar_mul(out=abf[:], in0=etp[:],
                                            scalar1=rsum[:])
                atp = ppw.tile([S, T], BF, tag="ws", name=f"atp{b}")
                nc.tensor.transpose(out=atp[:], in_=abf[:],
                                    identity=ident[0:T, 0:T])
                atb = ap_.tile([S, T], BF, tag="atb")
                nc.vector.tensor_copy(out=atb[:], in_=atp[:])
                for dc in range(KC):
                    cps = ppg.tile([128, T], F32, tag="gps",
                                   name=f"cps{b}_{dc}")
                    nc.tensor.matmul(out=cps[:],
                                     lhsT=enc[:, b * D + dc * 128:
                                              b * D + (dc + 1) * 128],
                                     rhs=atb[:], start=True, stop=True)
                    nc.vector.tensor_copy(
                        out=ctxT[:, (b * KC + dc) * T:(b * KC + dc + 1) * T],
                        in_=cps[:])
                for ec in range(KC):
                    ops = ppg.tile([128, T], F32, tag="gps",
                                   name=f"ops{b}_{ec}")
                    for kc in range(KC):
                        nc.tensor.matmul(
                            out=ops[:],
                            lhsT=vw1[:, kc * D + ec * 128:
                                     kc * D + (ec + 1) * 128],
                            rhs=ctxT[:, (b * KC + kc) * T:
                                     (b * KC + kc + 1) * T],
                            start=(kc == 0), stop=False)
                    for kc in range(KC):
                        nc.tensor.matmul(
                            out=ops[:],
                            lhsT=vw2[:, kc * D + ec * 128:
                                     kc * D + (ec + 1) * 128],
                            rhs=outT[:, (kc * BL + b) * T:
                                     (kc * BL + b + 1) * T],
                            start=False, stop=(kc == KC - 1))
                    nc.vector.tensor_scalar(
                        out=o2T[:, (ec * BL + b) * T:(ec * BL + b) * T + T],
                        in0=ops[:], scalar1=vb[:, ec:ec + 1], scalar2=None,
                        op0=ALU.add)

            # ================= vocab projection (bf16) ==========
            _hp2.close()
            o2r = o2T[:].rearrange("p (e c) -> p e c", e=KC, c=128)
            # Recycle dead phase-1 SBUF slots as stream buffers for the tail
            # vocab blocks: their DMAs fire as soon as the old tiles' readers
            # retire, moving DMA out of the saturated vocab tail.
            NRG = 4                      # blocks in the whh-slot group
            RS1 = ["xg", "whT", "wsT", "vw1", "vw2"]
            # 2KB slots that free when the last score task retires; pairs
            # stage one block each as two [128,2,512] half-tiles
            RS2 = [("th0", "th1"), ("th2", "th3"), ("th0", "th1"),
                   ("th2", "th3"), ("selv", "encT")]
            rg0 = NBLK - NRG - len(RS1)  # whh group covers rg0..rg0+3
            rs2_0 = rg0 - len(RS2)
            rs3_0 = rs2_0 - 2            # wsu-quad + outT/ctxT/wsb8 blocks
            rsrc = {}
            if stage >= 3:
                # whha+whhb slots: 8 KB each -> two blocks each
                for gi, wtag in enumerate(("whha", "whhb")):
                    ib_g = rg0 + gi * 2
                    vg = cp.tile([128, KC, 2 * VBLK], BF, tag=wtag,
                                 name=f"vgrp{gi}")
                    nc.sync.dma_start(
                        out=vg[:, :, :],
                        in_=vpt4[:, :, ib_g * VBLK:(ib_g + 2) * VBLK])
                    rsrc[ib_g] = (vg, 0)
                    rsrc[ib_g + 1] = (vg, VBLK)
                # quad block from the four 1KB wsu slots (kc0..3)
                ibx = rs3_0
                vqs = []
                for b_ in range(BL):
                    for par in range(2):
                        vq = sp.tile([128, 1, VBLK], BF,
                                     tag=f"wsu{b_}_{par}",
                                     name=f"vq{b_}_{par}")
                        kcq = b_ * 2 + par
                        nc.sync.dma_start(
                            out=vq[:, :, :],
                            in_=vpt4[:, kcq:kcq + 1,
                                     ibx * VBLK:(ibx + 1) * VBLK])
                        vqs.append((vq, 0))
                rsrc[ibx] = ("quad", vqs)
                # quad block from outT (1KB) + ctxT (1KB) + enc (2KB) slots
                iby = rs3_0 + 1
                vq_o = sp.tile([128, 1, VBLK], BF, tag="outT", name="vqo")
                nc.sync.dma_start(out=vq_o[:, :, :],
                                  in_=vpt4[:, 0:1, iby * VBLK:
                                           (iby + 1) * VBLK])
                vq_c = sp.tile([128, 1, VBLK], BF, tag="ctxT", name="vqc")
                nc.sync.dma_start(out=vq_c[:, :, :],
                                  in_=vpt4[:, 1:2, iby * VBLK:
                                           (iby + 1) * VBLK])
                vq_e2 = cp.tile([128, 2, VBLK], BF, tag="enc", name="vqe2")
                nc.sync.dma_start(out=vq_e2[:, :, :],
                                  in_=vpt4[:, 2:4, iby * VBLK:
                                           (iby + 1) * VBLK])
                rsrc[iby] = ("quad", [(vq_o, 0), (vq_c, 0),
                                      (vq_e2, 0), (vq_e2, 1)])
                for i, (tga, tgb) in enumerate(RS2):
                    ib_r = rs2_0 + i
                    v0r = ib_r * VBLK
                    pa = thp if tga.startswith("th") else cp
                    pb = thp if tgb.startswith("th") else cp
                    ta = pa.tile([128, 2, VBLK], BF, tag=tga,
                                 name=f"vspl{ib_r}a")
                    nc.sync.dma_start(out=ta[:, :, :],
                                      in_=vpt4[:, 0:2, v0r:v0r + VBLK])
                    tb = pb.tile([128, 2, VBLK], BF, tag=tgb,
                                 name=f"vspl{ib_r}b")
                    nc.sync.dma_start(out=tb[:, :, :],
                                      in_=vpt4[:, 2:4, v0r:v0r + VBLK])
                    rsrc[ib_r] = ("split", ta, tb)
                for i, tg in enumerate(RS1):
                    ib_r = rg0 + NRG + i
                    v0r = ib_r * VBLK
                    wr = min(VBLK, V - v0r)
                    vrt = cp.tile([128, KC, VBLK], BF, tag=tg,
                                  name=f"vrt{ib_r}")
                    nc.sync.dma_start(out=vrt[:, :, :wr],
                                      in_=vpt4[:, :, v0r:v0r + wr])
                    rsrc[ib_r] = (vrt, 0)
            # consumption order: alternate streamed/prefetched so stream DMAs
            # never stall; recycled-slot blocks go last (data arrives
            # mid-phase)
            tail0 = rs3_0 - (rs3_0 % 4)   # align tail to lsb store groups
            order = []
            si, pi = NPRE, 0
            while si < tail0 or pi < NPRE:
                if si < tail0:
                    order.append(si)
                    si += 1
                if pi < NPRE:
                    order.append(pi)
                    pi += 1
            order += list(range(tail0, NBLK))
            lsb_tiles = {}
            if stage < 3:
                order = []
            for nb, ib in enumerate(order):
                v0 = ib * VBLK
                w = min(VBLK, V - v0)
                if ib < NPRE:
                    def rhs_of(kc, v0=v0, w=w):
                        return vpre4[:, kc, v0:v0 + w]
                elif ib in rsrc:
                    ent = rsrc[ib]
                    if ent[0] == "split":
                        def rhs_of(kc, ta=ent[1], tb=ent[2], w=w):
                            return (ta if kc < 2 else tb)[:, kc % 2, 0:w]
                    elif ent[0] == "quad":
                        def rhs_of(kc, lst=ent[1], w=w):
                            t_, ix = lst[kc]
                            return t_[:, ix, 0:w]
                    else:
                        def rhs_of(kc, vs3=ent[0], voff=ent[1], w=w):
                            return vs3[:, kc, voff:voff + w]
                else:
                    vst = vp.tile([128, KC, VBLK], BF, tag="vs", bufs=NSTRB,
                                  name=f"vst{ib}")
                    nc.sync.dma_start(out=vst[:, :, :w],
                                      in_=vpt4[:, :, v0:v0 + w])
                    def rhs_of(kc, vst=vst, w=w):
                        return vst[:, kc, 0:w]
                grp = ib // 4
                if grp not in lsb_tiles:
                    lsb_tiles[grp] = [lp.tile([128, 4 * VBLK], BF, tag="lsb",
                                              name=f"lsb{grp}"), 0]
                lsb_e = lsb_tiles[grp]
                lps = ppg.tile([128, VBLK], F32, tag="gps", name=f"lps{ib}")
                for kc in range(KC):
                    nc.tensor.matmul(out=lps[:, :w],
                                     lhsT=o2r[:, kc, :],
                                     rhs=rhs_of(kc),
                                     start=(kc == 0), stop=(kc == KC - 1))
                dst = lsb_e[0][:, (ib % 4) * VBLK:(ib % 4) * VBLK + w]
                if nb % 2 == 0:
                    nc.scalar.copy(out=dst, in_=lps[:, :w])
                else:
                    nc.vector.tensor_copy(out=dst, in_=lps[:, :w])
                lsb_e[1] += 1
                nblk_grp = min(4, NBLK - grp * 4)
                if lsb_e[1] == nblk_grp:
                    gv0 = grp * 4 * VBLK
                    wlen = min(4 * VBLK, V - gv0)
                    nc.sync.dma_start(out=t_out[:, gv0:gv0 + wlen],
                                      in_=lsb_e[0][:, :wlen])

    nc.compile()
    return nc


def _prep_in_maps(inputs):
    inp = {k: np.asarray(v) for k, v in inputs.items()}
    words = inp["words"].astype(np.int64)
    enc = inp["encoder_output"].astype(np.float32)
    pre_h, cell = inp["pre_h"], inp["cell"]
    emb = inp["emb"]
    W_ih, W_hh = inp["W_ih"], inp["W_hh"]
    b_ih, b_hh = inp["b_ih"], inp["b_hh"]
    Wh_w = inp["Wh_w"]
    Ws_w, Ws_b = inp["Ws_w"], inp["Ws_b"]
    vt_w = inp["vt_w"]
    V_w, V_b = inp["V_w"], inp["V_b"]
    Vp_w, Vp_b = inp["Vp_w"], inp["Vp_b"]

    def re_lhsT(m, dt=BF16):  # [512, N] -> [128, 4*N] chunk-major
        n = m.shape[1]
        return np.ascontiguousarray(
            m.reshape(4, 128, n).transpose(1, 0, 2).reshape(128, 4 * n)
        ).astype(dt)

    # gate reorder (i,f,g,o) -> (i,f,o,g)
    perm = np.r_[0:512, 512:1024, 1536:2048, 1024:1536]
    W_ih_p, W_hh_p = W_ih[perm], W_hh[perm]
    b2 = (b_ih + b_hh)[perm].astype(np.float32)

    whh_re = re_lhsT(np.ascontiguousarray(W_hh_p.T))     # [p,(kc,g)]
    # -> j-major [p,(j,kc,128)]
    whh_j = np.ascontiguousarray(
        whh_re.reshape(128, KC, 16, 128).transpose(0, 2, 1, 3)
        .reshape(128, 16 * KC * 128))
    whha_re = np.ascontiguousarray(whh_j[:, :8 * KC * 128])
    whhb_re = np.ascontiguousarray(whh_j[:, 8 * KC * 128:])
    whT_re = re_lhsT(np.ascontiguousarray(Wh_w.T))
    wsT_re = re_lhsT(np.ascontiguousarray(Ws_w.T))
    vw1_re = re_lhsT(np.ascontiguousarray(V_w[:, :D].T))
    vw2_re = re_lhsT(np.ascontiguousarray(V_w[:, D:].T))
    vpt_re = re_lhsT(np.ascontiguousarray(Vp_w.T))
    wsb8_re = np.tile(Ws_b.reshape(1, D), (TS, 1)).astype(BF16)
    vb_re = np.ascontiguousarray(V_b.reshape(4, 128).T).astype(np.float32)
    vt_re = np.ascontiguousarray(vt_w.reshape(4, 128).T).astype(BF16)
    ident_re = np.eye(128, dtype=np.float32).astype(BF16)
    # DR selector: slot0[p,(tl,s)] = [p==s], slot1[p,(tl,s)] = [p==tl]
    sel0 = np.tile(np.eye(128, dtype=np.float32), (1, TS))
    sel1 = np.zeros((128, TS * S), dtype=np.float32)
    for tl in range(TS):
        sel1[tl, tl * S:(tl + 1) * S] = 1.0
    selv_re = np.concatenate([sel0, sel1], axis=1).astype(FP8)

    x_all = emb[words]                                   # [B,T,D]
    xg_all = x_all @ W_ih_p.T.astype(np.float32) + b2    # [B,T,4D]

    in_maps = []
    for k in range(NC):
        bs = slice(k * BL, (k + 1) * BL)
        xgl = xg_all[bs]                                 # [2,T,2048]
        xg_re = np.ascontiguousarray(
            xgl.reshape(BL, T, 16, 128).transpose(3, 1, 2, 0)
            .reshape(128, T * 16 * BL)).astype(BF16)     # [p,(t,j,b)]
        h0 = np.ascontiguousarray(
            pre_h[bs].reshape(BL, 4, 128).transpose(2, 1, 0)
            .reshape(128, 4 * BL)).astype(BF16)
        c0 = np.ascontiguousarray(
            cell[bs].reshape(BL, 4, 128).transpose(2, 1, 0)
            .reshape(128, 4 * BL)).astype(np.float32)
        encl = enc[bs]                                   # [2,S,D]
        encT_re = np.ascontiguousarray(
            encl.reshape(BL, S, 4, 128).transpose(3, 2, 0, 1)
            .reshape(128, 4 * BL * S)).astype(BF16)
        enc_re = np.ascontiguousarray(
            encl.transpose(1, 0, 2).reshape(S, BL * D)).astype(BF16)
        in_maps.append({
            "xg": xg_re, "whha": whha_re, "whhb": whhb_re, "h0": h0,
            "c0": c0, "encT": encT_re, "enc": enc_re, "whT": whT_re,
            "wsT": wsT_re, "wsb8": wsb8_re, "vt": vt_re, "selv": selv_re,
            "vw1": vw1_re, "vw2": vw2_re, "vb": vb_re, "vpt": vpt_re,
            "ident": ident_re,
        })
    return in_maps


def kernel(**inputs):
    in_maps = _prep_in_maps(inputs)
    if "nc" not in _cached:
        _cached["nc"] = _build_nc()
    res = bass_utils.run_bass_kernel_spmd(_cached["nc"], in_maps,
                                          core_ids=list(range(NC)))
    vpb = np.asarray(inputs["Vp_b"]).astype(np.float32)
    outs = [np.asarray(res.results[k]["out"]).astype(np.float32)
            .reshape(BL, T, V) for k in range(NC)]
    return np.concatenate(outs, axis=0) + vpb[None, None, :]


if __name__ == "__main__":
    pass


# revision 35
# speedup vs baseline: 1.1821x; 1.0042x over previous
"""AttnOutputDecoder Trainium2 kernel.

Sharding: data-parallel over batch B=16 across 8 cores (2 batches/core).
Per core: LSTM (gate order i,f,o,g; host-precomputed x@W_ih injected into
PSUM via identity matmul) overlapped with Bahdanau attention scores.
Score tanh-args wh[s]+ws[t] are built as a single fp8 DoubleRow matmul per
256-col chunk: lhsT packs [wh-chunk ; ws-rows] in fp8 (accuracy verified:
adds ~2e-3 rel err), rhs is an exact 0/1 selector matrix in fp8, so each
element is written once at 0.5 cyc/row instead of twice at 1.0. Ws_b is
folded into the ws tile copy. Output projection streams Vp_w.T in bf16
(fp8 fails the 2e-2 gate); logits stored bf16; Vp_b added on host.
"""

import numpy as np
import ml_dtypes

import concourse.bass as bass
import concourse.mybir as mybir
import concourse.tile as tile
from concourse import bacc
from concourse import bass_utils

BF16 = ml_dtypes.bfloat16
FP8 = ml_dtypes.float8_e4m3
F32 = mybir.dt.float32
BF = mybir.dt.bfloat16
F8 = mybir.dt.float8e4
AF = mybir.ActivationFunctionType
ALU = mybir.AluOpType
DR = mybir.MatmulPerfMode.DoubleRow

B, T, S, D, V = 16, 64, 128, 512, 32000
NC = 8
BL = B // NC          # local batches per core = 2
R = BL * T            # local rows = 128
G4 = 4 * D            # 2048 gates
KC = D // 128         # 4 contraction chunks
TS = 8                # score t-sub-block
CC = 256              # su DoubleRow column chunk
VBLK = 512
NBLK = (V + VBLK - 1) // VBLK   # 63
NPRE = 24             # prefetched vocab blocks
NSTRB = 6             # streamed-vocab buffer slots (1 block each)
WAITP = 2400          # pacing period hint (ns/step) for score tasks
WAITB = 9500          # pacing base offset (ns)
WAITD = 700           # extra tanh release delay past its su matmuls (ns)

_cached = {}


def _build_nc(stage=3):
    # stage 1: LSTM only; 2: + scores/softmax/out2; 3: full (vocab)
    nc = bacc.Bacc("TRN2", target_bir_lowering=False, debug=False,
                   num_devices=NC)

    def din(name, shape, dt):
        return nc.dram_tensor(name, shape, dt, kind="ExternalInput").ap()

    t_ident = din("ident", [128, 128], BF)
    t_h0 = din("h0", [128, KC * BL], BF)
    t_c0 = din("c0", [128, KC * BL], F32)
    t_whha = din("whha", [128, 8 * KC * 128], BF)    # [p,(j<8,kc,g)]
    t_whhb = din("whhb", [128, 8 * KC * 128], BF)    # [p,(j>=8,kc,g)]
    t_xg = din("xg", [128, T * 16 * BL], BF)         # [p,(t,j,b)]
    t_encT = din("encT", [128, KC * BL * S], BF)     # [p,(kc,b,s)]
    t_whT = din("whT", [128, KC * D], BF)            # Wh_w.T re
    t_wsT = din("wsT", [128, KC * D], BF)            # Ws_w.T re
    t_wsb8 = din("wsb8", [TS, D], BF)                # Ws_b row x TS
    t_vt = din("vt", [128, KC], BF)
    t_selv = din("selv", [128, 2 * TS * S], F8)      # DR selectors
    t_enc = din("enc", [128, BL * D], BF)            # [s,(b,d)]
    t_vw1 = din("vw1", [128, KC * D], BF)
    t_vw2 = din("vw2", [128, KC * D], BF)
    t_vb = din("vb", [128, KC], F32)
    t_vpt = din("vpt", [128, KC * V], BF)            # [p,(kc,v)] Vp_w.T re
    t_out = nc.dram_tensor("out", [R, V], BF, kind="ExternalOutput").ap()

    with tile.TileContext(nc) as tc:
        with (
            tc.tile_pool(name="const", bufs=1) as cp,
            tc.tile_pool(name="state", bufs=1) as sp,
            tc.tile_pool(name="gates", bufs=8) as gp,
            tc.tile_pool(name="attn", bufs=2) as ap_,
            tc.tile_pool(name="thp", bufs=2) as thp,
            tc.tile_pool(name="voc", bufs=2) as vp,
            tc.tile_pool(name="lsbp", bufs=3) as lp,
            tc.tile_pool(name="ps_g", bufs=2, space="PSUM") as ppg,
            tc.tile_pool(name="ps_sum", bufs=2, space="PSUM") as pps,
            tc.tile_pool(name="ps_e", bufs=1, space="PSUM") as ppe,
            tc.tile_pool(name="ps_w", bufs=1, space="PSUM") as ppw,
        ):
            # ---- LSTM-critical loads first (serial DMA device) ----
            ident = cp.tile([128, 128], BF)
            nc.sync.dma_start(out=ident[:], in_=t_ident[:])
            h0b = sp.tile([128, KC * BL], BF)
            nc.sync.dma_start(out=h0b[:], in_=t_h0[:])
            c = sp.tile([128, KC * BL], F32)
            nc.sync.dma_start(out=c[:], in_=t_c0[:])
            whha = cp.tile([128, 8 * KC * 128], BF)
            nc.sync.dma_start(out=whha[:], in_=t_whha[:])
            xg = cp.tile([128, T * 16 * BL], BF)
            nc.sync.dma_start(out=xg[:], in_=t_xg[:])
            whhb = cp.tile([128, 8 * KC * 128], BF)
            nc.sync.dma_start(out=whhb[:], in_=t_whhb[:])
            # ---- wh deps, then remaining constants ----
            encT = cp.tile([128, KC * BL * S], BF)
            nc.sync.dma_start(out=encT[:], in_=t_encT[:])
            whT = cp.tile([128, KC * D], BF)
            nc.sync.dma_start(out=whT[:], in_=t_whT[:])
            wsT = cp.tile([128, KC * D], BF)
            nc.sync.dma_start(out=wsT[:], in_=t_wsT[:])
            wsb8 = cp.tile([TS, D], BF)
            nc.sync.dma_start(out=wsb8[:], in_=t_wsb8[:])
            vt = cp.tile([128, KC], BF)
            nc.sync.dma_start(out=vt[:], in_=t_vt[:])
            selv = cp.tile([128, 2 * TS * S], F8)
            nc.sync.dma_start(out=selv[:], in_=t_selv[:])
            enc = cp.tile([128, BL * D], BF)
            nc.sync.dma_start(out=enc[:], in_=t_enc[:])
            vw1 = cp.tile([128, KC * D], BF)
            nc.sync.dma_start(out=vw1[:], in_=t_vw1[:])
            vw2 = cp.tile([128, KC * D], BF)
            nc.sync.dma_start(out=vw2[:], in_=t_vw2[:])
            vb = cp.tile([128, KC], F32)
            nc.sync.dma_start(out=vb[:], in_=t_vb[:])

            encT4 = encT[:].rearrange("p (kc b s) -> p kc b s", kc=KC, b=BL)
            whT4 = whT[:].rearrange("p (kc d) -> p kc d", kc=KC)
            wsT4 = wsT[:].rearrange("p (kc d) -> p kc d", kc=KC)
            selv3 = selv[:].rearrange("p (i n) -> p i n", i=2)

            outT = sp.tile([128, KC * BL * T], BF)   # [p,(kc,b,t)] all h's
            outT4 = outT[:].rearrange("p (kc b t) -> p kc b t", kc=KC, b=BL,
                                      t=T)

            # ---- vocab weight prefetch: chunked large DMAs ----
            vpt4 = t_vpt[:].rearrange("p (kc v) -> p kc v", kc=KC, v=V)
            vpre = cp.tile([128, KC * NPRE * VBLK], BF)
            vpre4 = vpre[:].rearrange("p (kc v) -> p kc v", kc=KC,
                                      v=NPRE * VBLK)
            PCH = 7
            for pc in range(0, NPRE, PCH):
                pe_ = min(NPRE, pc + PCH)
                nc.sync.dma_start(out=vpre4[:, :, pc * VBLK:pe_ * VBLK],
                                  in_=vpt4[:, :, pc * VBLK:pe_ * VBLK])

            # ====== wh[s, d] chunks -> wsu slot0 (fp8), ws_b folded to ws ==
            # wsu[(b,par)]: [128, kc, 2, 128] fp8; slot0 = wh (s-partition),
            # slot1 rows 0..TS = ws rows of t-block (t-partition), rest zero.
            wsu_tiles = {}
            for b_ in range(BL):
                for par in range(2):
                    w_ = sp.tile([128, KC, 2, 128], F8, tag=f"wsu{b_}_{par}")
                    wsu_tiles[(b_, par)] = w_
                    nc.gpsimd.memset(w_[:, :, 1, :], 0)
            for b_ in range(BL):
                whp = ppw.tile([S, D], F32, tag="ws", name=f"whp{b_}")
                for kc in range(KC):
                    nc.tensor.matmul(out=whp[:], lhsT=encT4[:, kc, b_, :],
                                     rhs=whT4[:, kc, :],
                                     start=(kc == 0), stop=(kc == KC - 1))
                whp3 = whp[:].rearrange("p (kc d) -> p kc d", kc=KC)
                for par in range(2):
                    nc.vector.tensor_copy(
                        out=wsu_tiles[(b_, par)][:, :, 0, :], in_=whp3[:, :, :])

            eps = ppe.tile([S, BL * T], F32, tag="eps")   # scores [s,(b,t)]

            # ========== LSTM with interleaved score tasks ==========
            # task_a: fp8 DoubleRow outer-sum + ACT tanh (lags LSTM);
            # task_b: eps dot matmuls, emitted later still.
            pend_a = []
            pend_b = []
            th_tiles = {}

            su_tiles = {}

            def emit_su(b, tsub, dc):
                # PE part of a score task: no h-dependency, runs in PE idle
                su = pps.tile([128, TS * S], F32, tag="sum",
                              name=f"su{b}_{tsub}_{dc}")
                wsu = wsu_tiles[(b, tsub % 2)]
                for cc in range(TS * S // CC):
                    nc.tensor.matmul(out=su[:, cc * CC:(cc + 1) * CC],
                                     lhsT=wsu[:, dc, :, :],
                                     rhs=selv3[:, :, cc * CC:(cc + 1) * CC],
                                     start=True, stop=True, perf_mode=DR)
                su_tiles[(b, tsub, dc)] = su
                pend_th.append((b, tsub, dc))

            HH = TS * S // 2
            thA_done = {}

            def emit_thA(b, tsub, dc):
                # first tanh half: fills the ACT idle gap while DVE runs
                # the cell update, ahead of tanh(c)
                th = thp.tile([128, TS * S], BF, tag=f"th{dc}",
                              name=f"th{b}_{tsub}_{dc}")
                nc.scalar.activation(out=th[:, 0:HH],
                                     in_=su_tiles[(b, tsub, dc)][:, 0:HH],
                                     func=AF.Tanh)
                thA_done[(b, tsub, dc)] = th

            def emit_thB(b, tsub, dc):
                th = thA_done.pop((b, tsub, dc))
                nc.scalar.activation(out=th[:, HH:],
                                     in_=su_tiles.pop((b, tsub, dc))[:, HH:],
                                     func=AF.Tanh)
                th_tiles[(b, tsub, dc)] = th
                if dc == KC - 1:
                    pend_b.append((b, tsub))

            def emit_th(b, tsub, dc):
                # unsplit path for the post-loop drain (fewer overheads)
                th = thp.tile([128, TS * S], BF, tag=f"th{dc}",
                              name=f"th{b}_{tsub}_{dc}")
                nc.scalar.activation(out=th[:],
                                     in_=su_tiles.pop((b, tsub, dc))[:],
                                     func=AF.Tanh)
                th_tiles[(b, tsub, dc)] = th
                if dc == KC - 1:
                    pend_b.append((b, tsub))

            eps_done = [False] * BL

            def emit_b(b, tsub, half):
                for tl in range(half * (TS // 2), (half + 1) * (TS // 2)):
                    t = tsub * TS + tl
                    for d2 in range(KC):
                        nc.tensor.matmul(
                            out=eps[:, b * T + t: b * T + t + 1],
                            lhsT=th_tiles[(b, tsub, d2)][:,
                                tl * S:(tl + 1) * S],
                            rhs=vt[:, d2:d2 + 1],
                            start=(d2 == 0), stop=(d2 == KC - 1))
                if tsub == T // TS - 1 and half == 1:
                    eps_done[b] = True

            pend_th = []

            def pump(hint_ns=None):
                # su one step ahead of its tanh: PE part never blocks ACT.
                # thA was emitted inside the LSTM block; finish with thB.
                if pend_th:
                    head = pend_th[0]
                    if head in thA_done:
                        emit_thB(*pend_th.pop(0))
                    else:
                        emit_th(*pend_th.pop(0))
                if pend_a:
                    emit_su(*pend_a.pop(0))
                if pend_b:
                    b, tsub = pend_b[0]
                    half = pump.half
                    emit_b(b, tsub, half)
                    if half == 1:
                        pend_b.pop(0)
                    pump.half = 1 - half
            pump.half = 0

            from contextlib import ExitStack
            for t in range(T):
                _hp = ExitStack()
                _hp.enter_context(tc.high_priority())
                gps = ppg.tile([128, 16 * BL], F32, tag="gps",
                               name=f"gps{t}")
                nc.tensor.matmul(out=gps[:], lhsT=ident[:],
                                 rhs=xg[:, t * 32:(t + 1) * 32],
                                 start=True, stop=False,
                                 skip_group_check=True)
                for j in range(16):
                    wt = whha if j < 8 else whhb
                    jr = j % 8
                    for kc in range(KC):
                        if t == 0:
                            hsrc = h0b[:, kc * BL:(kc + 1) * BL]
                        else:
                            hsrc = outT4[:, kc, :, t - 1]
                        nc.tensor.matmul(
                            out=gps[:, j * BL:(j + 1) * BL],
                            lhsT=wt[:, (jr * KC + kc) * 128:
                                    (jr * KC + kc + 1) * 128],
                            rhs=hsrc, start=False, stop=(kc == KC - 1),
                            skip_group_check=True)
                sio = gp.tile([128, 16 * BL], F32, tag="sio")
                nc.scalar.activation(out=sio[:, 0:12 * BL],
                                     in_=gps[:, 0:12 * BL], func=AF.Sigmoid)
                nc.scalar.activation(out=sio[:, 12 * BL:16 * BL],
                                     in_=gps[:, 12 * BL:16 * BL],
                                     func=AF.Tanh)
                t1 = gp.tile([128, KC * BL], F32, tag="t1")
                t2 = gp.tile([128, KC * BL], F32, tag="t2")
                nc.vector.tensor_mul(out=t1[:], in0=sio[:, 4 * BL:8 * BL],
                                     in1=c[:])
                nc.vector.tensor_mul(out=t2[:], in0=sio[:, 0:4 * BL],
                                     in1=sio[:, 12 * BL:16 * BL])
                nc.vector.tensor_add(out=c[:], in0=t1[:], in1=t2[:])
                tc_ = gp.tile([128, KC * BL], F32, tag="tc")
                nc.scalar.activation(out=tc_[:], in_=c[:], func=AF.Tanh)
                nc.vector.tensor_mul(out=outT4[:, :, :, t],
                                     in0=sio[:, 8 * BL:12 * BL], in1=tc_[:])
                _hp.close()

                with tc.tile_wait_until((WAITB + WAITP * t) / 1e6):
                    pump(hint_ns=WAITB + WAITP * t)

                if stage >= 2 and t % TS == TS - 1:
                    tsub = t // TS
                    tc.tile_set_cur_wait((WAITB + WAITP * t) / 1e6)
                    for b in range(BL):
                        wps = ppw.tile([TS, D], F32, tag="ws",
                                       name=f"wps{b}_{tsub}")
                        for kc in range(KC):
                            nc.tensor.matmul(
                                out=wps[:],
                                lhsT=outT[:, (kc * BL + b) * T + tsub * TS:
                                          (kc * BL + b) * T + tsub * TS + TS],
                                rhs=wsT4[:, kc, :],
                                start=(kc == 0), stop=(kc == KC - 1))
                        wps3 = wps[:].rearrange("p (kc d) -> p kc d", kc=KC)
                        wsb3 = wsb8[:].rearrange("p (kc d) -> p kc d", kc=KC)
                        nc.vector.tensor_add(
                            out=wsu_tiles[(b, tsub % 2)][0:TS, :, 1, :],
                            in0=wps3[:, :, :], in1=wsb3[:, :, :])
                    tc.cur_wait_ts = 0
                    pend_a.extend((b, tsub, dc) for b in range(BL)
                                  for dc in range(KC))

            # ============ softmax, context, out2 (per batch) ============
            ctxT = sp.tile([128, BL * KC * T], BF)   # [p,(b,dc,t)]
            o2T = sp.tile([128, KC * BL * T], BF)    # [p,(ec,b,t)]

            def sm_b(b):
                _hp2 = ExitStack()
                _hp2.enter_context(tc.high_priority())
                ebf = ap_.tile([S, T], BF, tag="ebf")
                nc.scalar.activation(out=ebf[:],
                                     in_=eps[:, b * T:(b + 1) * T],
                                     func=AF.Exp)
                etp = ppw.tile([T, S], BF, tag="ws", name=f"etp{b}")
                nc.tensor.transpose(out=etp[:], in_=ebf[:],
                                    identity=ident[:, :])
                ssum = ap_.tile([T, 1], F32, tag="ssum")
                nc.vector.tensor_reduce(out=ssum[:], in_=etp[:],
                                        axis=mybir.AxisListType.X, op=ALU.add)
                rsum = ap_.tile([T, 1], F32, tag="rsum")
                nc.vector.reciprocal(out=rsum[:], in_=ssum[:])
                abf = ap_.tile([T, S], BF, tag="abf")
                nc.vector.tensor_scalar_mul(out=abf[:], in0=etp[:],
                                            scalar1=rsum[:])
                atp = ppw.tile([S, T], BF, tag="ws", name=f"atp{b}")
                nc.tensor.transpose(out=atp[:], in_=abf[:],
                                    identity=ident[0:T, 0:T])
                atb = ap_.tile([S, T], BF, tag="atb")
                nc.vector.tensor_copy(out=atb[:], in_=atp[:])
                for dc in range(KC):
                    cps = ppg.tile([128, T], F32, tag="gps",
                                   name=f"cps{b}_{dc}")
                    nc.tensor.matmul(out=cps[:],
                                     lhsT=enc[:, b * D + dc * 128:
                                              b * D + (dc + 1) * 128],
                                     rhs=atb[:], start=True, stop=True)
                    nc.vector.tensor_copy(
                        out=ctxT[:, (b * KC + dc) * T:(b * KC + dc + 1) * T],
                        in_=cps[:])
                for ec in range(KC):
                    ops = ppg.tile([128, T], F32, tag="gps",
                                   name=f"ops{b}_{ec}")
                    for kc in range(KC):
                        nc.tensor.matmul(
                            out=ops[:],
                            lhsT=vw1[:, kc * D + ec * 128:
                                     kc * D + (ec + 1) * 128],
                            rhs=ctxT[:, (b * KC + kc) * T:
                                     (b * KC + kc + 1) * T],
                            start=(kc == 0), stop=False)
                    for kc in range(KC):
                        nc.tensor.matmul(
                            out=ops[:],
                            lhsT=vw2[:, kc * D + ec * 128:
                                     kc * D + (ec + 1) * 128],
                            rhs=outT[:, (kc * BL + b) * T:
                                     (kc * BL + b + 1) * T],
                            start=False, stop=(kc == KC - 1))
                    nc.vector.tensor_scalar(
                        out=o2T[:, (ec * BL + b) * T:(ec * BL + b) * T + T],
                        in0=ops[:], scalar1=vb[:, ec:ec + 1], scalar2=None,
                        op0=ALU.add)
                _hp2.close()

            # drain: all remaining su first (PE), then tanh/eps; each
            # batch's softmax/out2 is emitted the moment its scores finish,
            # overlapping the other batch's tanh drain
            sm_done = set()
            while pend_a:
                emit_su(*pend_a.pop(0))
            while pend_th or pend_b:
                pump()
                if stage >= 2:
                    for b_ in range(BL):
                        if eps_done[b_] and b_ not in sm_done:
                            sm_b(b_)
                            sm_done.add(b_)
            if stage >= 2:
                for b_ in range(BL):
                    if b_ not in sm_done:
                        sm_b(b_)

            # ================= vocab projection (bf16) ==========
            o2r = o2T[:].rearrange("p (e c) -> p e c", e=KC, c=128)
            # Recycle dead phase-1 SBUF slots as stream buffers for the tail
            # vocab blocks: their DMAs fire as soon as the old tiles' readers
            # retire, moving DMA out of the saturated vocab tail.
            GRP2 = ("whha", "whhb", "xg")   # 8KB slots: 2 blocks each
            NRG = 2 * len(GRP2)
            RS1 = ["whT", "wsT", "vw1", "vw2"]
            # 2KB slots that free when the last score task retires; pairs
            # stage one block each as two [128,2,512] half-tiles
            RS2 = [("th0", "th1"), ("th2", "th3"), ("th0", "th1"),
                   ("th2", "th3"), ("selv", "encT")]
            rg0 = NBLK - NRG - len(RS1)  # whh group covers rg0..rg0+3
            rs2_0 = rg0 - len(RS2)
            rs3_0 = rs2_0 - 2            # wsu-quad + outT/ctxT/wsb8 blocks
            rsrc = {}
            if stage >= 3:
                # 8 KB slots -> two blocks each
                for gi, wtag in enumerate(GRP2):
                    ib_g = rg0 + gi * 2
                    vg = cp.tile([128, KC, 2 * VBLK], BF, tag=wtag,
                                 name=f"vgrp{gi}")
                    nc.sync.dma_start(
                        out=vg[:, :, :],
                        in_=vpt4[:, :, ib_g * VBLK:(ib_g + 2) * VBLK])
                    rsrc[ib_g] = (vg, 0)
                    rsrc[ib_g + 1] = (vg, VBLK)
                # quad block from the four 1KB wsu slots (kc0..3)
                ibx = rs3_0
                vqs = []
                for b_ in range(BL):
                    for par in range(2):
                        vq = sp.tile([128, 1, VBLK], BF,
                                     tag=f"wsu{b_}_{par}",
                                     name=f"vq{b_}_{par}")
                        kcq = b_ * 2 + par
                        nc.sync.dma_start(
                            out=vq[:, :, :],
                            in_=vpt4[:, kcq:kcq + 1,
                                     ibx * VBLK:(ibx + 1) * VBLK])
                        vqs.append((vq, 0))
                rsrc[ibx] = ("quad", vqs)
                # quad block from outT (1KB) + ctxT (1KB) + enc (2KB) slots
                iby = rs3_0 + 1
                vq_o = sp.tile([128, 1, VBLK], BF, tag="outT", name="vqo")
                nc.sync.dma_start(out=vq_o[:, :, :],
                                  in_=vpt4[:, 0:1, iby * VBLK:
                                           (iby + 1) * VBLK])
                vq_c = sp.tile([128, 1, VBLK], BF, tag="ctxT", name="vqc")
                nc.sync.dma_start(out=vq_c[:, :, :],
                                  in_=vpt4[:, 1:2, iby * VBLK:
                                           (iby + 1) * VBLK])
                vq_e2 = cp.tile([128, 2, VBLK], BF, tag="enc", name="vqe2")
                nc.sync.dma_start(out=vq_e2[:, :, :],
                                  in_=vpt4[:, 2:4, iby * VBLK:
                                           (iby + 1) * VBLK])
                rsrc[iby] = ("quad", [(vq_o, 0), (vq_c, 0),
                                      (vq_e2, 0), (vq_e2, 1)])
                for i, (tga, tgb) in enumerate(RS2):
                    ib_r = rs2_0 + i
                    v0r = ib_r * VBLK
                    pa = thp if tga.startswith("th") else cp
                    pb = thp if tgb.startswith("th") else cp
                    ta = pa.tile([128, 2, VBLK], BF, tag=tga,
                                 name=f"vspl{ib_r}a")
                    nc.sync.dma_start(out=ta[:, :, :],
                                      in_=vpt4[:, 0:2, v0r:v0r + VBLK])
                    tb = pb.tile([128, 2, VBLK], BF, tag=tgb,
                                 name=f"vspl{ib_r}b")
                    nc.sync.dma_start(out=tb[:, :, :],
                                      in_=vpt4[:, 2:4, v0r:v0r + VBLK])
                    rsrc[ib_r] = ("split", ta, tb)
                for i, tg in enumerate(RS1):
                    ib_r = rg0 + NRG + i
                    v0r = ib_r * VBLK
                    wr = min(VBLK, V - v0r)
                    vrt = cp.tile([128, KC, VBLK], BF, tag=tg,
                                  name=f"vrt{ib_r}")
                    nc.sync.dma_start(out=vrt[:, :, :wr],
                                      in_=vpt4[:, :, v0r:v0r + wr])
                    rsrc[ib_r] = (vrt, 0)
            # consumption order: alternate streamed/prefetched so stream DMAs
            # never stall; recycled-slot blocks go last (data arrives
            # mid-phase)
            tail0 = rs3_0 - (rs3_0 % 4)   # align tail to lsb store groups
            order = []
            si, pi = NPRE, 0
            while si < tail0 or pi < NPRE:
                if si < tail0:
                    order.append(si)
                    si += 1
                if pi < NPRE:
                    order.append(pi)
                    pi += 1
            order += list(range(tail0, NBLK))
            lsb_tiles = {}
            if stage < 3:
                order = []
            for nb, ib in enumerate(order):
                v0 = ib * VBLK
                w = min(VBLK, V - v0)
                if ib < NPRE:
                    def rhs_of(kc, v0=v0, w=w):
                        return vpre4[:, kc, v0:v0 + w]
                elif ib in rsrc:
                    ent = rsrc[ib]
                    if ent[0] == "split":
                        def rhs_of(kc, ta=ent[1], tb=ent[2], w=w):
                            return (ta if kc < 2 else tb)[:, kc % 2, 0:w]
                    elif ent[0] == "quad":
                        def rhs_of(kc, lst=ent[1], w=w):
                            t_, ix = lst[kc]
                            return t_[:, ix, 0:w]
                    else:
                        def rhs_of(kc, vs3=ent[0], voff=ent[1], w=w):
                            return vs3[:, kc, voff:voff + w]
                else:
                    vst = vp.tile([128, KC, VBLK], BF, tag="vs", bufs=NSTRB,
                                  name=f"vst{ib}")
                    nc.sync.dma_start(out=vst[:, :, :w],
                                      in_=vpt4[:, :, v0:v0 + w])
                    def rhs_of(kc, vst=vst, w=w):
                        return vst[:, kc, 0:w]
                grp = ib // 4
                if grp not in lsb_tiles:
                    lsb_tiles[grp] = [lp.tile([128, 4 * VBLK], BF, tag="lsb",
                                              name=f"lsb{grp}"), 0]
                lsb_e = lsb_tiles[grp]
                lps = ppg.tile([128, VBLK], F32, tag="gps", name=f"lps{ib}")
                for kc in range(KC):
                    nc.tensor.matmul(out=lps[:, :w],
                                     lhsT=o2r[:, kc, :],
                                     rhs=rhs_of(kc),
                                     start=(kc == 0), stop=(kc == KC - 1))
                dst = lsb_e[0][:, (ib % 4) * VBLK:(ib % 4) * VBLK + w]
                if nb % 2 == 0:
                    nc.scalar.copy(out=dst, in_=lps[:, :w])
                else:
                    nc.vector.tensor_copy(out=dst, in_=lps[:, :w])
                lsb_e[1] += 1
                nblk_grp = min(4, NBLK - grp * 4)
                if lsb_e[1] == nblk_grp:
                    gv0 = grp * 4 * VBLK
                    wlen = min(4 * VBLK, V - gv0)
                    nc.sync.dma_start(out=t_out[:, gv0:gv0 + wlen],
                                      in_=lsb_e[0][:, :wlen])

    nc.compile()
    return nc


def _prep_in_maps(inputs):
    inp = {k: np.asarray(v) for k, v in inputs.items()}
    words = inp["words"].astype(np.int64)
    enc = inp["encoder_output"].astype(np.float32)
    pre_h, cell = inp["pre_h"], inp["cell"]
    emb = inp["emb"]
    W_ih, W_hh = inp["W_ih"], inp["W_hh"]
    b_ih, b_hh = inp["b_ih"], inp["b_hh"]
    Wh_w = inp["Wh_w"]
    Ws_w, Ws_b = inp["Ws_w"], inp["Ws_b"]
    vt_w = inp["vt_w"]
    V_w, V_b = inp["V_w"], inp["V_b"]
    Vp_w, Vp_b = inp["Vp_w"], inp["Vp_b"]

    def re_lhsT(m, dt=BF16):  # [512, N] -> [128, 4*N] chunk-major
        n = m.shape[1]
        return np.ascontiguousarray(
            m.reshape(4, 128, n).transpose(1, 0, 2).reshape(128, 4 * n)
        ).astype(dt)

    # gate reorder (i,f,g,o) -> (i,f,o,g)
    perm = np.r_[0:512, 512:1024, 1536:2048, 1024:1536]
    W_ih_p, W_hh_p = W_ih[perm], W_hh[perm]
    b2 = (b_ih + b_hh)[perm].astype(np.float32)

    whh_re = re_lhsT(np.ascontiguousarray(W_hh_p.T))     # [p,(kc,g)]
    # -> j-major [p,(j,kc,128)]
    whh_j = np.ascontiguousarray(
        whh_re.reshape(128, KC, 16, 128).transpose(0, 2, 1, 3)
        .reshape(128, 16 * KC * 128))
    whha_re = np.ascontiguousarray(whh_j[:, :8 * KC * 128])
    whhb_re = np.ascontiguousarray(whh_j[:, 8 * KC * 128:])
    whT_re = re_lhsT(np.ascontiguousarray(Wh_w.T))
    wsT_re = re_lhsT(np.ascontiguousarray(Ws_w.T))
    vw1_re = re_lhsT(np.ascontiguousarray(V_w[:, :D].T))
    vw2_re = re_lhsT(np.ascontiguousarray(V_w[:, D:].T))
    vpt_re = re_lhsT(np.ascontiguousarray(Vp_w.T))
    wsb8_re = np.tile(Ws_b.reshape(1, D), (TS, 1)).astype(BF16)
    vb_re = np.ascontiguousarray(V_b.reshape(4, 128).T).astype(np.float32)
    vt_re = np.ascontiguousarray(vt_w.reshape(4, 128).T).astype(BF16)
    ident_re = np.eye(128, dtype=np.float32).astype(BF16)
    # DR selector: slot0[p,(tl,s)] = [p==s], slot1[p,(tl,s)] = [p==tl]
    sel0 = np.tile(np.eye(128, dtype=np.float32), (1, TS))
    sel1 = np.zeros((128, TS * S), dtype=np.float32)
    for tl in range(TS):
        sel1[tl, tl * S:(tl + 1) * S] = 1.0
    selv_re = np.concatenate([sel0, sel1], axis=1).astype(FP8)

    x_all = emb[words]                                   # [B,T,D]
    xg_all = x_all @ W_ih_p.T.astype(np.float32) + b2    # [B,T,4D]

    in_maps = []
    for k in range(NC):
        bs = slice(k * BL, (k + 1) * BL)
        xgl = xg_all[bs]                                 # [2,T,2048]
        xg_re = np.ascontiguousarray(
            xgl.reshape(BL, T, 16, 128).transpose(3, 1, 2, 0)
            .reshape(128, T * 16 * BL)).astype(BF16)     # [p,(t,j,b)]
        h0 = np.ascontiguousarray(
            pre_h[bs].reshape(BL, 4, 128).transpose(2, 1, 0)
            .reshape(128, 4 * BL)).astype(BF16)
        c0 = np.ascontiguousarray(
            cell[bs].reshape(BL, 4, 128).transpose(2, 1, 0)
            .reshape(128, 4 * BL)).astype(np.float32)
        encl = enc[bs]                                   # [2,S,D]
        encT_re = np.ascontiguousarray(
            encl.reshape(BL, S, 4, 128).transpose(3, 2, 0, 1)
            .reshape(128, 4 * BL * S)).astype(BF16)
        enc_re = np.ascontiguousarray(
            encl.transpose(1, 0, 2).reshape(S, BL * D)).astype(BF16)
        in_maps.append({
            "xg": xg_re, "whha": whha_re, "whhb": whhb_re, "h0": h0,
            "c0": c0, "encT": encT_re, "enc": enc_re, "whT": whT_re,
            "wsT": wsT_re, "wsb8": wsb8_re, "vt": vt_re, "selv": selv_re,
            "vw1": vw1_re, "vw2": vw2_re, "vb": vb_re, "vpt": vpt_re,
            "ident": ident_re,
        })
    return in_maps


def kernel(**inputs):
    in_maps = _prep_in_maps(inputs)
    if "nc" not in _cached:
        _cached["nc"] = _build_nc()
    res = bass_utils.run_bass_kernel_spmd(_cached["nc"], in_maps,
                                          core_ids=list(range(NC)))
    vpb = np.asarray(inputs["Vp_b"]).astype(np.float32)
    outs = [np.asarray(res.results[k]["out"]).astype(np.float32)
            .reshape(BL, T, V) for k in range(NC)]
    return np.concatenate(outs, axis=0) + vpb[None, None, :]


if __name__ == "__main__":
    pass


# revision 39
# speedup vs baseline: 1.1873x; 1.0044x over previous
"""AttnOutputDecoder Trainium2 kernel.

Sharding: data-parallel over batch B=16 across 8 cores (2 batches/core).
Per core: LSTM (gate order i,f,o,g; host-precomputed x@W_ih injected into
PSUM via identity matmul) overlapped with Bahdanau attention scores.
Score tanh-args wh[s]+ws[t] are built as a single fp8 DoubleRow matmul per
256-col chunk: lhsT packs [wh-chunk ; ws-rows] in fp8 (accuracy verified:
adds ~2e-3 rel err), rhs is an exact 0/1 selector matrix in fp8, so each
element is written once at 0.5 cyc/row instead of twice at 1.0. Ws_b is
folded into the ws tile copy. Output projection streams Vp_w.T in bf16
(fp8 fails the 2e-2 gate); logits stored bf16; Vp_b added on host.
"""

import numpy as np
import ml_dtypes

import concourse.bass as bass
import concourse.mybir as mybir
import concourse.tile as tile
from concourse import bacc
from concourse import bass_utils

BF16 = ml_dtypes.bfloat16
FP8 = ml_dtypes.float8_e4m3
F32 = mybir.dt.float32
BF = mybir.dt.bfloat16
F8 = mybir.dt.float8e4
AF = mybir.ActivationFunctionType
ALU = mybir.AluOpType
DR = mybir.MatmulPerfMode.DoubleRow

B, T, S, D, V = 16, 64, 128, 512, 32000
NC = 8
BL = B // NC          # local batches per core = 2
R = BL * T            # local rows = 128
G4 = 4 * D            # 2048 gates
KC = D // 128         # 4 contraction chunks
TS = 8                # score t-sub-block
CC = 256              # su DoubleRow column chunk
VBLK = 512
NBLK = (V + VBLK - 1) // VBLK   # 63
NPRE = 24             # prefetched vocab blocks
NSTRB = 6             # streamed-vocab buffer slots (1 block each)
WAITP = 2300          # pacing period hint (ns/step) for score tasks
WAITB = 9500          # pacing base offset (ns)
WAITD = 700           # extra tanh release delay past its su matmuls (ns)

_cached = {}


def _build_nc(stage=3):
    # stage 1: LSTM only; 2: + scores/softmax/out2; 3: full (vocab)
    nc = bacc.Bacc("TRN2", target_bir_lowering=False, debug=False,
                   num_devices=NC)

    def din(name, shape, dt):
        return nc.dram_tensor(name, shape, dt, kind="ExternalInput").ap()

    t_ident = din("ident", [128, 128], BF)
    t_h0 = din("h0", [128, KC * BL], BF)
    t_c0 = din("c0", [128, KC * BL], F32)
    t_whha = din("whha", [128, 8 * KC * 128], BF)    # [p,(j<8,kc,g)]
    t_whhb = din("whhb", [128, 8 * KC * 128], BF)    # [p,(j>=8,kc,g)]
    t_xg = din("xg", [128, T * 16 * BL], BF)         # [p,(t,j,b)]
    t_encT = din("encT", [128, KC * BL * S], BF)     # [p,(kc,b,s)]
    t_whT = din("whT", [128, KC * D], BF)            # Wh_w.T re
    t_wsT = din("wsT", [128, KC * D], BF)            # Ws_w.T re
    t_wsb8 = din("wsb8", [TS, D], BF)                # Ws_b row x TS
    t_vt = din("vt", [128, KC], BF)
    t_selv = din("selv", [128, 2 * TS * S], F8)      # DR selectors
    t_enc = din("enc", [128, BL * D], BF)            # [s,(b,d)]
    t_vw1 = din("vw1", [128, KC * D], BF)
    t_vw2 = din("vw2", [128, KC * D], BF)
    t_vb = din("vb", [128, KC], F32)
    t_vpt = din("vpt", [128, KC * V], BF)            # [p,(kc,v)] Vp_w.T re
    t_out = nc.dram_tensor("out", [R, V], BF, kind="ExternalOutput").ap()

    with tile.TileContext(nc) as tc:
        with (
            tc.tile_pool(name="const", bufs=1) as cp,
            tc.tile_pool(name="state", bufs=1) as sp,
            tc.tile_pool(name="gates", bufs=8) as gp,
            tc.tile_pool(name="attn", bufs=2) as ap_,
            tc.tile_pool(name="thp", bufs=2) as thp,
            tc.tile_pool(name="voc", bufs=2) as vp,
            tc.tile_pool(name="lsbp", bufs=3) as lp,
            tc.tile_pool(name="ps_g", bufs=2, space="PSUM") as ppg,
            tc.tile_pool(name="ps_sum", bufs=2, space="PSUM") as pps,
            tc.tile_pool(name="ps_e", bufs=1, space="PSUM") as ppe,
            tc.tile_pool(name="ps_w", bufs=1, space="PSUM") as ppw,
        ):
            # ---- LSTM-critical loads first (serial DMA device) ----
            ident = cp.tile([128, 128], BF)
            nc.sync.dma_start(out=ident[:], in_=t_ident[:])
            h0b = sp.tile([128, KC * BL], BF)
            nc.sync.dma_start(out=h0b[:], in_=t_h0[:])
            c = sp.tile([128, KC * BL], F32)
            nc.sync.dma_start(out=c[:], in_=t_c0[:])
            whha = cp.tile([128, 8 * KC * 128], BF)
            nc.sync.dma_start(out=whha[:], in_=t_whha[:])
            xg = cp.tile([128, T * 16 * BL], BF)
            nc.sync.dma_start(out=xg[:], in_=t_xg[:])
            whhb = cp.tile([128, 8 * KC * 128], BF)
            nc.sync.dma_start(out=whhb[:], in_=t_whhb[:])
            # ---- wh deps, then remaining constants ----
            encT = cp.tile([128, KC * BL * S], BF)
            nc.sync.dma_start(out=encT[:], in_=t_encT[:])
            whT = cp.tile([128, KC * D], BF)
            nc.sync.dma_start(out=whT[:], in_=t_whT[:])
            wsT = cp.tile([128, KC * D], BF)
            nc.sync.dma_start(out=wsT[:], in_=t_wsT[:])
            wsb8 = cp.tile([TS, D], BF)
            nc.sync.dma_start(out=wsb8[:], in_=t_wsb8[:])
            vt = cp.tile([128, KC], BF)
            nc.sync.dma_start(out=vt[:], in_=t_vt[:])
            selv = cp.tile([128, 2 * TS * S], F8)
            nc.sync.dma_start(out=selv[:], in_=t_selv[:])
            enc = cp.tile([128, BL * D], BF)
            nc.sync.dma_start(out=enc[:], in_=t_enc[:])
            vw1 = cp.tile([128, KC * D], BF)
            nc.sync.dma_start(out=vw1[:], in_=t_vw1[:])
            vw2 = cp.tile([128, KC * D], BF)
            nc.sync.dma_start(out=vw2[:], in_=t_vw2[:])
            vb = cp.tile([128, KC], F32)
            nc.sync.dma_start(out=vb[:], in_=t_vb[:])

            encT4 = encT[:].rearrange("p (kc b s) -> p kc b s", kc=KC, b=BL)
            whT4 = whT[:].rearrange("p (kc d) -> p kc d", kc=KC)
            wsT4 = wsT[:].rearrange("p (kc d) -> p kc d", kc=KC)
            selv3 = selv[:].rearrange("p (i n) -> p i n", i=2)

            outT = sp.tile([128, KC * BL * T], BF)   # [p,(kc,b,t)] all h's
            outT4 = outT[:].rearrange("p (kc b t) -> p kc b t", kc=KC, b=BL,
                                      t=T)

            # ---- vocab weight prefetch: chunked large DMAs ----
            vpt4 = t_vpt[:].rearrange("p (kc v) -> p kc v", kc=KC, v=V)
            vpre = cp.tile([128, KC * NPRE * VBLK], BF)
            vpre4 = vpre[:].rearrange("p (kc v) -> p kc v", kc=KC,
                                      v=NPRE * VBLK)
            PCH = 7
            for pc in range(0, NPRE, PCH):
                pe_ = min(NPRE, pc + PCH)
                nc.sync.dma_start(out=vpre4[:, :, pc * VBLK:pe_ * VBLK],
                                  in_=vpt4[:, :, pc * VBLK:pe_ * VBLK])

            # ====== wh[s, d] chunks -> wsu slot0 (fp8), ws_b folded to ws ==
            # wsu[(b,par)]: [128, kc, 2, 128] fp8; slot0 = wh (s-partition),
            # slot1 rows 0..TS = ws rows of t-block (t-partition), rest zero.
            wsu_tiles = {}
            for b_ in range(BL):
                for par in range(2):
                    w_ = sp.tile([128, KC, 2, 128], F8, tag=f"wsu{b_}_{par}")
                    wsu_tiles[(b_, par)] = w_
                    nc.gpsimd.memset(w_[:, :, 1, :], 0)
            for b_ in range(BL):
                whp = ppw.tile([S, D], F32, tag="ws", name=f"whp{b_}")
                for kc in range(KC):
                    nc.tensor.matmul(out=whp[:], lhsT=encT4[:, kc, b_, :],
                                     rhs=whT4[:, kc, :],
                                     start=(kc == 0), stop=(kc == KC - 1))
                whp3 = whp[:].rearrange("p (kc d) -> p kc d", kc=KC)
                for par in range(2):
                    nc.vector.tensor_copy(
                        out=wsu_tiles[(b_, par)][:, :, 0, :], in_=whp3[:, :, :])

            eps = ppe.tile([S, BL * T], F32, tag="eps")   # scores [s,(b,t)]

            # ========== LSTM with interleaved score tasks ==========
            # task_a: fp8 DoubleRow outer-sum + ACT tanh (lags LSTM);
            # task_b: eps dot matmuls, emitted later still.
            pend_a = []
            pend_b = []
            th_tiles = {}

            su_tiles = {}

            def emit_su(b, tsub, dc):
                # PE part of a score task: no h-dependency, runs in PE idle
                su = pps.tile([128, TS * S], F32, tag="sum",
                              name=f"su{b}_{tsub}_{dc}")
                wsu = wsu_tiles[(b, tsub % 2)]
                for cc in range(TS * S // CC):
                    nc.tensor.matmul(out=su[:, cc * CC:(cc + 1) * CC],
                                     lhsT=wsu[:, dc, :, :],
                                     rhs=selv3[:, :, cc * CC:(cc + 1) * CC],
                                     start=True, stop=True, perf_mode=DR)
                su_tiles[(b, tsub, dc)] = su
                pend_th.append((b, tsub, dc))

            HH = TS * S // 2
            thA_done = {}

            def emit_thA(b, tsub, dc):
                # first tanh half: fills the ACT idle gap while DVE runs
                # the cell update, ahead of tanh(c)
                th = thp.tile([128, TS * S], BF, tag=f"th{dc}",
                              name=f"th{b}_{tsub}_{dc}")
                nc.scalar.activation(out=th[:, 0:HH],
                                     in_=su_tiles[(b, tsub, dc)][:, 0:HH],
                                     func=AF.Tanh)
                thA_done[(b, tsub, dc)] = th

            def emit_thB(b, tsub, dc):
                th = thA_done.pop((b, tsub, dc))
                nc.scalar.activation(out=th[:, HH:],
                                     in_=su_tiles.pop((b, tsub, dc))[:, HH:],
                                     func=AF.Tanh)
                th_tiles[(b, tsub, dc)] = th
                if dc == KC - 1:
                    pend_b.append((b, tsub))

            def emit_th(b, tsub, dc):
                # unsplit path for the post-loop drain (fewer overheads)
                th = thp.tile([128, TS * S], BF, tag=f"th{dc}",
                              name=f"th{b}_{tsub}_{dc}")
                nc.scalar.activation(out=th[:],
                                     in_=su_tiles.pop((b, tsub, dc))[:],
                                     func=AF.Tanh)
                th_tiles[(b, tsub, dc)] = th
                if dc == KC - 1:
                    pend_b.append((b, tsub))

            eps_done = [False] * BL

            def emit_b(b, tsub, half):
                for tl in range(half * (TS // 2), (half + 1) * (TS // 2)):
                    t = tsub * TS + tl
                    for d2 in range(KC):
                        nc.tensor.matmul(
                            out=eps[:, b * T + t: b * T + t + 1],
                            lhsT=th_tiles[(b, tsub, d2)][:,
                                tl * S:(tl + 1) * S],
                            rhs=vt[:, d2:d2 + 1],
                            start=(d2 == 0), stop=(d2 == KC - 1))
                if tsub == T // TS - 1 and half == 1:
                    eps_done[b] = True

            pend_th = []

            def pump(hint_ns=None):
                # su one step ahead of its tanh: PE part never blocks ACT.
                # thA was emitted inside the LSTM block; finish with thB.
                if pend_th:
                    head = pend_th[0]
                    if head in thA_done:
                        emit_thB(*pend_th.pop(0))
                    else:
                        emit_th(*pend_th.pop(0))
                if pend_a:
                    emit_su(*pend_a.pop(0))
                if pend_b:
                    b, tsub = pend_b[0]
                    half = pump.half
                    emit_b(b, tsub, half)
                    if half == 1:
                        pend_b.pop(0)
                    pump.half = 1 - half
            pump.half = 0

            from contextlib import ExitStack
            for t in range(T):
                _hp = ExitStack()
                _hp.enter_context(tc.high_priority())
                gps = ppg.tile([128, 16 * BL], F32, tag="gps",
                               name=f"gps{t}")
                nc.tensor.matmul(out=gps[:], lhsT=ident[:],
                                 rhs=xg[:, t * 32:(t + 1) * 32],
                                 start=True, stop=False,
                                 skip_group_check=True)
                for j in range(16):
                    wt = whha if j < 8 else whhb
                    jr = j % 8
                    for kc in range(KC):
                        if t == 0:
                            hsrc = h0b[:, kc * BL:(kc + 1) * BL]
                        else:
                            hsrc = outT4[:, kc, :, t - 1]
                        nc.tensor.matmul(
                            out=gps[:, j * BL:(j + 1) * BL],
                            lhsT=wt[:, (jr * KC + kc) * 128:
                                    (jr * KC + kc + 1) * 128],
                            rhs=hsrc, start=False, stop=(kc == KC - 1),
                            skip_group_check=True)
                # ONE sigmoid for all four gates (g rows host-doubled):
                # tanh(g) = 2*sigmoid(2g) - 1, absorbed into the DVE ops.
                # Each ACT instruction costs ~400 ns of pipeline hold on
                # top of its work, so fewer/larger ACT ops win.
                sio = gp.tile([128, 16 * BL], F32, tag="sio")
                nc.scalar.activation(out=sio[:],
                                     in_=gps[:], func=AF.Sigmoid)
                t1 = gp.tile([128, KC * BL], F32, tag="t1")
                t2 = gp.tile([128, KC * BL], F32, tag="t2")
                nc.vector.tensor_mul(out=t1[:], in0=sio[:, 4 * BL:8 * BL],
                                     in1=c[:])
                # t2 = (2*sig(2g)) * sig(i)
                nc.vector.scalar_tensor_tensor(
                    out=t2[:], in0=sio[:, 12 * BL:16 * BL], scalar=2.0,
                    in1=sio[:, 0:4 * BL], op0=ALU.mult, op1=ALU.mult)
                t3 = gp.tile([128, KC * BL], F32, tag="t3")
                nc.vector.tensor_add(out=t3[:], in0=t1[:], in1=t2[:])
                nc.vector.tensor_sub(out=c[:], in0=t3[:],
                                     in1=sio[:, 0:4 * BL])
                tc_ = gp.tile([128, KC * BL], F32, tag="tc")
                nc.scalar.activation(out=tc_[:], in_=c[:], func=AF.Tanh)
                nc.vector.tensor_mul(out=outT4[:, :, :, t],
                                     in0=sio[:, 8 * BL:12 * BL], in1=tc_[:])
                _hp.close()

                with tc.tile_wait_until((WAITB + WAITP * t) / 1e6):
                    pump(hint_ns=WAITB + WAITP * t)

                if stage >= 2 and t % TS == TS - 1:
                    tsub = t // TS
                    tc.tile_set_cur_wait((WAITB + WAITP * t) / 1e6)
                    for b in range(BL):
                        wps = ppw.tile([TS, D], F32, tag="ws",
                                       name=f"wps{b}_{tsub}")
                        for kc in range(KC):
                            nc.tensor.matmul(
                                out=wps[:],
                                lhsT=outT[:, (kc * BL + b) * T + tsub * TS:
                                          (kc * BL + b) * T + tsub * TS + TS],
                                rhs=wsT4[:, kc, :],
                                start=(kc == 0), stop=(kc == KC - 1))
                        wps3 = wps[:].rearrange("p (kc d) -> p kc d", kc=KC)
                        wsb3 = wsb8[:].rearrange("p (kc d) -> p kc d", kc=KC)
                        nc.vector.tensor_add(
                            out=wsu_tiles[(b, tsub % 2)][0:TS, :, 1, :],
                            in0=wps3[:, :, :], in1=wsb3[:, :, :])
                    tc.cur_wait_ts = 0
                    pend_a.extend((b, tsub, dc) for b in range(BL)
                                  for dc in range(KC))

            # ============ softmax, context, out2 (per batch) ============
            ctxT = sp.tile([128, BL * KC * T], BF)   # [p,(b,dc,t)]
            o2T = sp.tile([128, KC * BL * T], BF)    # [p,(ec,b,t)]

            def sm_b(b):
                _hp2 = ExitStack()
                _hp2.enter_context(tc.high_priority())
                ebf = ap_.tile([S, T], BF, tag="ebf")
                nc.scalar.activation(out=ebf[:],
                                     in_=eps[:, b * T:(b + 1) * T],
                                     func=AF.Exp)
                etp = ppw.tile([T, S], BF, tag="ws", name=f"etp{b}")
                nc.tensor.transpose(out=etp[:], in_=ebf[:],
                                    identity=ident[:, :])
                ssum = ap_.tile([T, 1], F32, tag="ssum")
                nc.vector.tensor_reduce(out=ssum[:], in_=etp[:],
                                        axis=mybir.AxisListType.X, op=ALU.add)
                rsum = ap_.tile([T, 1], F32, tag="rsum")
                nc.vector.reciprocal(out=rsum[:], in_=ssum[:])
                abf = ap_.tile([T, S], BF, tag="abf")
                nc.vector.tensor_scalar_mul(out=abf[:], in0=etp[:],
                                            scalar1=rsum[:])
                atp = ppw.tile([S, T], BF, tag="ws", name=f"atp{b}")
                nc.tensor.transpose(out=atp[:], in_=abf[:],
                                    identity=ident[0:T, 0:T])
                atb = ap_.tile([S, T], BF, tag="atb")
                nc.vector.tensor_copy(out=atb[:], in_=atp[:])
                for dc in range(KC):
                    cps = ppg.tile([128, T], F32, tag="gps",
                                   name=f"cps{b}_{dc}")
                    nc.tensor.matmul(out=cps[:],
                                     lhsT=enc[:, b * D + dc * 128:
                                              b * D + (dc + 1) * 128],
                                     rhs=atb[:], start=True, stop=True)
                    nc.vector.tensor_copy(
                        out=ctxT[:, (b * KC + dc) * T:(b * KC + dc + 1) * T],
                        in_=cps[:])
                for ec in range(KC):
                    ops = ppg.tile([128, T], F32, tag="gps",
                                   name=f"ops{b}_{ec}")
                    for kc in range(KC):
                        nc.tensor.matmul(
                            out=ops[:],
                            lhsT=vw1[:, kc * D + ec * 128:
                                     kc * D + (ec + 1) * 128],
                            rhs=ctxT[:, (b * KC + kc) * T:
                                     (b * KC + kc + 1) * T],
                            start=(kc == 0), stop=False)
                    for kc in range(KC):
                        nc.tensor.matmul(
                            out=ops[:],
                            lhsT=vw2[:, kc * D + ec * 128:
                                     kc * D + (ec + 1) * 128],
                            rhs=outT[:, (kc * BL + b) * T:
                                     (kc * BL + b + 1) * T],
                            start=False, stop=(kc == KC - 1))
                    nc.vector.tensor_scalar(
                        out=o2T[:, (ec * BL + b) * T:(ec * BL + b) * T + T],
                        in0=ops[:], scalar1=vb[:, ec:ec + 1], scalar2=None,
                        op0=ALU.add)
                _hp2.close()

            # drain: all remaining su first (PE), then tanh/eps; each
            # batch's softmax/out2 is emitted the moment its scores finish,
            # overlapping the other batch's tanh drain
            sm_done = set()
            while pend_a:
                emit_su(*pend_a.pop(0))
            while pend_th or pend_b:
                pump()
                if stage >= 2:
                    for b_ in range(BL):
                        if eps_done[b_] and b_ not in sm_done:
                            sm_b(b_)
                            sm_done.add(b_)
            if stage >= 2:
                for b_ in range(BL):
                    if b_ not in sm_done:
                        sm_b(b_)

            # ================= vocab projection (bf16) ==========
            o2r = o2T[:].rearrange("p (e c) -> p e c", e=KC, c=128)
            # Recycle dead phase-1 SBUF slots as stream buffers for the tail
            # vocab blocks: their DMAs fire as soon as the old tiles' readers
            # retire, moving DMA out of the saturated vocab tail.
            GRP2 = ("whha", "whhb", "xg")   # 8KB slots: 2 blocks each
            NRG = 2 * len(GRP2)
            RS1 = ["whT", "wsT", "vw1", "vw2"]
            # 2KB slots that free when the last score task retires; pairs
            # stage one block each as two [128,2,512] half-tiles
            RS2 = [("th0", "th1"), ("th2", "th3"), ("th0", "th1"),
                   ("th2", "th3"), ("selv", "encT")]
            rg0 = NBLK - NRG - len(RS1)  # whh group covers rg0..rg0+3
            rs2_0 = rg0 - len(RS2)
            rs3_0 = rs2_0 - 2            # wsu-quad + outT/ctxT/wsb8 blocks
            rsrc = {}
            if stage >= 3:
                # 8 KB slots -> two blocks each
                for gi, wtag in enumerate(GRP2):
                    ib_g = rg0 + gi * 2
                    vg = cp.tile([128, KC, 2 * VBLK], BF, tag=wtag,
                                 name=f"vgrp{gi}")
                    nc.sync.dma_start(
                        out=vg[:, :, :],
                        in_=vpt4[:, :, ib_g * VBLK:(ib_g + 2) * VBLK])
                    rsrc[ib_g] = (vg, 0)
                    rsrc[ib_g + 1] = (vg, VBLK)
                # quad block from the four 1KB wsu slots (kc0..3)
                ibx = rs3_0
                vqs = []
                for b_ in range(BL):
                    for par in range(2):
                        vq = sp.tile([128, 1, VBLK], BF,
                                     tag=f"wsu{b_}_{par}",
                                     name=f"vq{b_}_{par}")
                        kcq = b_ * 2 + par
                        nc.sync.dma_start(
                            out=vq[:, :, :],
                            in_=vpt4[:, kcq:kcq + 1,
                                     ibx * VBLK:(ibx + 1) * VBLK])
                        vqs.append((vq, 0))
                rsrc[ibx] = ("quad", vqs)
                # quad block from outT (1KB) + ctxT (1KB) + enc (2KB) slots
                iby = rs3_0 + 1
                vq_o = sp.tile([128, 1, VBLK], BF, tag="outT", name="vqo")
                nc.sync.dma_start(out=vq_o[:, :, :],
                                  in_=vpt4[:, 0:1, iby * VBLK:
                                           (iby + 1) * VBLK])
                vq_c = sp.tile([128, 1, VBLK], BF, tag="ctxT", name="vqc")
                nc.sync.dma_start(out=vq_c[:, :, :],
                                  in_=vpt4[:, 1:2, iby * VBLK:
                                           (iby + 1) * VBLK])
                vq_e2 = cp.tile([128, 2, VBLK], BF, tag="enc", name="vqe2")
                nc.sync.dma_start(out=vq_e2[:, :, :],
                                  in_=vpt4[:, 2:4, iby * VBLK:
                                           (iby + 1) * VBLK])
                rsrc[iby] = ("quad", [(vq_o, 0), (vq_c, 0),
                                      (vq_e2, 0), (vq_e2, 1)])
                for i, (tga, tgb) in enumerate(RS2):
                    ib_r = rs2_0 + i
                    v0r = ib_r * VBLK
                    pa = thp if tga.startswith("th") else cp
                    pb = thp if tgb.startswith("th") else cp
                    ta = pa.tile([128, 2, VBLK], BF, tag=tga,
                                 name=f"vspl{ib_r}a")
                    nc.sync.dma_start(out=ta[:, :, :],
                                      in_=vpt4[:, 0:2, v0r:v0r + VBLK])
                    tb = pb.tile([128, 2, VBLK], BF, tag=tgb,
                                 name=f"vspl{ib_r}b")
                    nc.sync.dma_start(out=tb[:, :, :],
                                      in_=vpt4[:, 2:4, v0r:v0r + VBLK])
                    rsrc[ib_r] = ("split", ta, tb)
                for i, tg in enumerate(RS1):
                    ib_r = rg0 + NRG + i
                    v0r = ib_r * VBLK
                    wr = min(VBLK, V - v0r)
                    vrt = cp.tile([128, KC, VBLK], BF, tag=tg,
                                  name=f"vrt{ib_r}")
                    nc.sync.dma_start(out=vrt[:, :, :wr],
                                      in_=vpt4[:, :, v0r:v0r + wr])
                    rsrc[ib_r] = (vrt, 0)
            # consumption order: alternate streamed/prefetched so stream DMAs
            # never stall; recycled-slot blocks go last (data arrives
            # mid-phase)
            tail0 = rs3_0 - (rs3_0 % 4)   # align tail to lsb store groups
            order = []
            si, pi = NPRE, 0
            while si < tail0 or pi < NPRE:
                if si < tail0:
                    order.append(si)
                    si += 1
                if pi < NPRE:
                    order.append(pi)
                    pi += 1
            order += list(range(tail0, NBLK))
            lsb_tiles = {}
            if stage < 3:
                order = []
            for nb, ib in enumerate(order):
                v0 = ib * VBLK
                w = min(VBLK, V - v0)
                if ib < NPRE:
                    def rhs_of(kc, v0=v0, w=w):
                        return vpre4[:, kc, v0:v0 + w]
                elif ib in rsrc:
                    ent = rsrc[ib]
                    if ent[0] == "split":
                        def rhs_of(kc, ta=ent[1], tb=ent[2], w=w):
                            return (ta if kc < 2 else tb)[:, kc % 2, 0:w]
                    elif ent[0] == "quad":
                        def rhs_of(kc, lst=ent[1], w=w):
                            t_, ix = lst[kc]
                            return t_[:, ix, 0:w]
                    else:
                        def rhs_of(kc, vs3=ent[0], voff=ent[1], w=w):
                            return vs3[:, kc, voff:voff + w]
                else:
                    vst = vp.tile([128, KC, VBLK], BF, tag="vs", bufs=NSTRB,
                                  name=f"vst{ib}")
                    nc.sync.dma_start(out=vst[:, :, :w],
                                      in_=vpt4[:, :, v0:v0 + w])
                    def rhs_of(kc, vst=vst, w=w):
                        return vst[:, kc, 0:w]
                grp = ib // 4
                if grp not in lsb_tiles:
                    lsb_tiles[grp] = [lp.tile([128, 4 * VBLK], BF, tag="lsb",
                                              name=f"lsb{grp}"), 0]
                lsb_e = lsb_tiles[grp]
                lps = ppg.tile([128, VBLK], F32, tag="gps", name=f"lps{ib}")
                for kc in range(KC):
                    nc.tensor.matmul(out=lps[:, :w],
                                     lhsT=o2r[:, kc, :],
                                     rhs=rhs_of(kc),
                                     start=(kc == 0), stop=(kc == KC - 1))
                dst = lsb_e[0][:, (ib % 4) * VBLK:(ib % 4) * VBLK + w]
                if nb % 2 == 0:
                    nc.scalar.copy(out=dst, in_=lps[:, :w])
                else:
                    nc.vector.tensor_copy(out=dst, in_=lps[:, :w])
                lsb_e[1] += 1
                nblk_grp = min(4, NBLK - grp * 4)
                if lsb_e[1] == nblk_grp:
                    gv0 = grp * 4 * VBLK
                    wlen = min(4 * VBLK, V - gv0)
                    nc.sync.dma_start(out=t_out[:, gv0:gv0 + wlen],
                                      in_=lsb_e[0][:, :wlen])

    nc.compile()
    return nc


def _prep_in_maps(inputs):
    inp = {k: np.asarray(v) for k, v in inputs.items()}
    words = inp["words"].astype(np.int64)
    enc = inp["encoder_output"].astype(np.float32)
    pre_h, cell = inp["pre_h"], inp["cell"]
    emb = inp["emb"]
    W_ih, W_hh = inp["W_ih"], inp["W_hh"]
    b_ih, b_hh = inp["b_ih"], inp["b_hh"]
    Wh_w = inp["Wh_w"]
    Ws_w, Ws_b = inp["Ws_w"], inp["Ws_b"]
    vt_w = inp["vt_w"]
    V_w, V_b = inp["V_w"], inp["V_b"]
    Vp_w, Vp_b = inp["Vp_w"], inp["Vp_b"]

    def re_lhsT(m, dt=BF16):  # [512, N] -> [128, 4*N] chunk-major
        n = m.shape[1]
        return np.ascontiguousarray(
            m.reshape(4, 128, n).transpose(1, 0, 2).reshape(128, 4 * n)
        ).astype(dt)

    # gate reorder (i,f,g,o) -> (i,f,o,g); g rows doubled so a single
    # sigmoid yields sig(2g) and tanh(g) = 2*sig(2g) - 1 on device
    perm = np.r_[0:512, 512:1024, 1536:2048, 1024:1536]
    W_ih_p = W_ih[perm].copy()
    W_hh_p = W_hh[perm].copy()
    b2 = (b_ih + b_hh)[perm].astype(np.float32)
    W_ih_p[1536:] *= 2.0
    W_hh_p[1536:] *= 2.0
    b2[1536:] *= 2.0

    whh_re = re_lhsT(np.ascontiguousarray(W_hh_p.T))     # [p,(kc,g)]
    # -> j-major [p,(j,kc,128)]
    whh_j = np.ascontiguousarray(
        whh_re.reshape(128, KC, 16, 128).transpose(0, 2, 1, 3)
        .reshape(128, 16 * KC * 128))
    whha_re = np.ascontiguousarray(whh_j[:, :8 * KC * 128])
    whhb_re = np.ascontiguousarray(whh_j[:, 8 * KC * 128:])
    whT_re = re_lhsT(np.ascontiguousarray(Wh_w.T))
    wsT_re = re_lhsT(np.ascontiguousarray(Ws_w.T))
    vw1_re = re_lhsT(np.ascontiguousarray(V_w[:, :D].T))
    vw2_re = re_lhsT(np.ascontiguousarray(V_w[:, D:].T))
    vpt_re = re_lhsT(np.ascontiguousarray(Vp_w.T))
    wsb8_re = np.tile(Ws_b.reshape(1, D), (TS, 1)).astype(BF16)
    vb_re = np.ascontiguousarray(V_b.reshape(4, 128).T).astype(np.float32)
    vt_re = np.ascontiguousarray(vt_w.reshape(4, 128).T).astype(BF16)
    ident_re = np.eye(128, dtype=np.float32).astype(BF16)
    # DR selector: slot0[p,(tl,s)] = [p==s], slot1[p,(tl,s)] = [p==tl]
    sel0 = np.tile(np.eye(128, dtype=np.float32), (1, TS))
    sel1 = np.zeros((128, TS * S), dtype=np.float32)
    for tl in range(TS):
        sel1[tl, tl * S:(tl + 1) * S] = 1.0
    selv_re = np.concatenate([sel0, sel1], axis=1).astype(FP8)

    x_all = emb[words]                                   # [B,T,D]
    xg_all = x_all @ W_ih_p.T.astype(np.float32) + b2    # [B,T,4D]

    in_maps = []
    for k in range(NC):
        bs = slice(k * BL, (k + 1) * BL)
        xgl = xg_all[bs]                                 # [2,T,2048]
        xg_re = np.ascontiguousarray(
            xgl.reshape(BL, T, 16, 128).transpose(3, 1, 2, 0)
            .reshape(128, T * 16 * BL)).astype(BF16)     # [p,(t,j,b)]
        h0 = np.ascontiguousarray(
            pre_h[bs].reshape(BL, 4, 128).transpose(2, 1, 0)
            .reshape(128, 4 * BL)).astype(BF16)
        c0 = np.ascontiguousarray(
            cell[bs].reshape(BL, 4, 128).transpose(2, 1, 0)
            .reshape(128, 4 * BL)).astype(np.float32)
        encl = enc[bs]                                   # [2,S,D]
        encT_re = np.ascontiguousarray(
            encl.reshape(BL, S, 4, 128).transpose(3, 2, 0, 1)
            .reshape(128, 4 * BL * S)).astype(BF16)
        enc_re = np.ascontiguousarray(
            encl.transpose(1, 0, 2).reshape(S, BL * D)).astype(BF16)
        in_maps.append({
            "xg": xg_re, "whha": whha_re, "whhb": whhb_re, "h0": h0,
            "c0": c0, "encT": encT_re, "enc": enc_re, "whT": whT_re,
            "wsT": wsT_re, "wsb8": wsb8_re, "vt": vt_re, "selv": selv_re,
            "vw1": vw1_re, "vw2": vw2_re, "vb": vb_re, "vpt": vpt_re,
            "ident": ident_re,
        })
    return in_maps


def kernel(**inputs):
    in_maps = _prep_in_maps(inputs)
    if "nc" not in _cached:
        _cached["nc"] = _build_nc()
    res = bass_utils.run_bass_kernel_spmd(_cached["nc"], in_maps,
                                          core_ids=list(range(NC)))
    vpb = np.asarray(inputs["Vp_b"]).astype(np.float32)
    outs = [np.asarray(res.results[k]["out"]).astype(np.float32)
            .reshape(BL, T, V) for k in range(NC)]
    return np.concatenate(outs, axis=0) + vpb[None, None, :]


if __name__ == "__main__":
    pass


# revision 42
# speedup vs baseline: 1.1993x; 1.0101x over previous
"""AttnOutputDecoder Trainium2 kernel.

Sharding: data-parallel over batch B=16 across 8 cores (2 batches/core).
Per core: LSTM (gate order i,f,o,g; host-precomputed x@W_ih injected into
PSUM via identity matmul) overlapped with Bahdanau attention scores.
Score tanh-args wh[s]+ws[t] are built as a single fp8 DoubleRow matmul per
256-col chunk: lhsT packs [wh-chunk ; ws-rows] in fp8 (accuracy verified:
adds ~2e-3 rel err), rhs is an exact 0/1 selector matrix in fp8, so each
element is written once at 0.5 cyc/row instead of twice at 1.0. Ws_b is
folded into the ws tile copy. Output projection streams Vp_w.T in bf16
(fp8 fails the 2e-2 gate); logits stored bf16; Vp_b added on host.
"""

import numpy as np
import ml_dtypes

import concourse.bass as bass
import concourse.mybir as mybir
import concourse.tile as tile
from concourse import bacc
from concourse import bass_utils

BF16 = ml_dtypes.bfloat16
FP8 = ml_dtypes.float8_e4m3
F32 = mybir.dt.float32
BF = mybir.dt.bfloat16
F8 = mybir.dt.float8e4
AF = mybir.ActivationFunctionType
ALU = mybir.AluOpType
DR = mybir.MatmulPerfMode.DoubleRow

B, T, S, D, V = 16, 64, 128, 512, 32000
NC = 8
BL = B // NC          # local batches per core = 2
R = BL * T            # local rows = 128
G4 = 4 * D            # 2048 gates
KC = D // 128         # 4 contraction chunks
TS = 8                # score t-sub-block
CC = 256              # su DoubleRow column chunk
VBLK = 512
NBLK = (V + VBLK - 1) // VBLK   # 63
NPRE = 24             # prefetched vocab blocks
NSTRB = 6             # streamed-vocab buffer slots (1 block each)
WAITP = 2350          # pacing period hint (ns/step) for score tasks
WAITB = 8500          # pacing base offset (ns)
WAITD = 700           # extra tanh release delay past its su matmuls (ns)

_cached = {}


def _build_nc(stage=3):
    # stage 1: LSTM only; 2: + scores/softmax/out2; 3: full (vocab)
    nc = bacc.Bacc("TRN2", target_bir_lowering=False, debug=False,
                   num_devices=NC)

    def din(name, shape, dt):
        return nc.dram_tensor(name, shape, dt, kind="ExternalInput").ap()

    t_ident = din("ident", [128, 128], BF)
    t_h0 = din("h0", [128, KC * BL], BF)
    t_c0 = din("c0", [128, KC * BL], F32)
    t_whha = din("whha", [128, 8 * KC * 128], BF)    # [p,(j<8,kc,g)]
    t_whhb = din("whhb", [128, 8 * KC * 128], BF)    # [p,(j>=8,kc,g)]
    t_xg = din("xg", [128, T * 16 * BL], BF)         # [p,(t,j,b)]
    t_encT = din("encT", [128, KC * BL * S], BF)     # [p,(kc,b,s)]
    t_whT = din("whT", [128, KC * D], BF)            # Wh_w.T re
    t_wsT = din("wsT", [128, KC * D], BF)            # Ws_w.T re
    t_wsb8 = din("wsb8", [TS, D], BF)                # Ws_b row x TS
    t_vt = din("vt", [128, KC], BF)
    t_selv = din("selv", [128, 2 * TS * S], F8)      # DR selectors
    t_enc = din("enc", [128, BL * D], BF)            # [s,(b,d)]
    t_vw1 = din("vw1", [128, KC * D], BF)
    t_vw2 = din("vw2", [128, KC * D], BF)
    t_vb = din("vb", [128, KC], F32)
    t_vpt = din("vpt", [128, KC * V], BF)            # [p,(kc,v)] Vp_w.T re
    t_out = nc.dram_tensor("out", [R, V], BF, kind="ExternalOutput").ap()

    with tile.TileContext(nc) as tc:
        with (
            tc.tile_pool(name="const", bufs=1) as cp,
            tc.tile_pool(name="state", bufs=1) as sp,
            tc.tile_pool(name="gates", bufs=8) as gp,
            tc.tile_pool(name="attn", bufs=2) as ap_,
            tc.tile_pool(name="thp", bufs=2) as thp,
            tc.tile_pool(name="voc", bufs=2) as vp,
            tc.tile_pool(name="lsbp", bufs=3) as lp,
            tc.tile_pool(name="ps_g", bufs=2, space="PSUM") as ppg,
            tc.tile_pool(name="ps_sum", bufs=2, space="PSUM") as pps,
            tc.tile_pool(name="ps_e", bufs=1, space="PSUM") as ppe,
            tc.tile_pool(name="ps_w", bufs=1, space="PSUM") as ppw,
        ):
            # ---- LSTM-critical loads first (serial DMA device) ----
            ident = cp.tile([128, 128], BF)
            nc.sync.dma_start(out=ident[:], in_=t_ident[:])
            h0b = sp.tile([128, KC * BL], BF)
            nc.sync.dma_start(out=h0b[:], in_=t_h0[:])
            c = sp.tile([128, KC * BL], F32)
            nc.sync.dma_start(out=c[:], in_=t_c0[:])
            whha = cp.tile([128, 8 * KC * 128], BF)
            nc.sync.dma_start(out=whha[:], in_=t_whha[:])
            xg = cp.tile([128, T * 16 * BL], BF)
            nc.sync.dma_start(out=xg[:], in_=t_xg[:])
            whhb = cp.tile([128, 8 * KC * 128], BF)
            nc.sync.dma_start(out=whhb[:], in_=t_whhb[:])
            # ---- wh deps, then remaining constants ----
            encT = cp.tile([128, KC * BL * S], BF)
            nc.sync.dma_start(out=encT[:], in_=t_encT[:])
            whT = cp.tile([128, KC * D], BF)
            nc.sync.dma_start(out=whT[:], in_=t_whT[:])
            wsT = cp.tile([128, KC * D], BF)
            nc.sync.dma_start(out=wsT[:], in_=t_wsT[:])
            wsb8 = cp.tile([TS, D], BF)
            nc.sync.dma_start(out=wsb8[:], in_=t_wsb8[:])
            vt = cp.tile([128, KC], BF)
            nc.sync.dma_start(out=vt[:], in_=t_vt[:])
            selv = cp.tile([128, 2 * TS * S], F8)
            nc.sync.dma_start(out=selv[:], in_=t_selv[:])
            enc = cp.tile([128, BL * D], BF)
            nc.sync.dma_start(out=enc[:], in_=t_enc[:])
            vw1 = cp.tile([128, KC * D], BF)
            nc.sync.dma_start(out=vw1[:], in_=t_vw1[:])
            vw2 = cp.tile([128, KC * D], BF)
            nc.sync.dma_start(out=vw2[:], in_=t_vw2[:])
            vb = cp.tile([128, KC], F32)
            nc.sync.dma_start(out=vb[:], in_=t_vb[:])

            encT4 = encT[:].rearrange("p (kc b s) -> p kc b s", kc=KC, b=BL)
            whT4 = whT[:].rearrange("p (kc d) -> p kc d", kc=KC)
            wsT4 = wsT[:].rearrange("p (kc d) -> p kc d", kc=KC)
            selv3 = selv[:].rearrange("p (i n) -> p i n", i=2)

            outT = sp.tile([128, KC * BL * T], BF)   # [p,(kc,b,t)] all h's
            outT4 = outT[:].rearrange("p (kc b t) -> p kc b t", kc=KC, b=BL,
                                      t=T)

            # ---- vocab weight prefetch: chunked large DMAs ----
            vpt4 = t_vpt[:].rearrange("p (kc v) -> p kc v", kc=KC, v=V)
            vpre = cp.tile([128, KC * NPRE * VBLK], BF)
            vpre4 = vpre[:].rearrange("p (kc v) -> p kc v", kc=KC,
                                      v=NPRE * VBLK)
            PCH = 7
            for pc in range(0, NPRE, PCH):
                pe_ = min(NPRE, pc + PCH)
                nc.sync.dma_start(out=vpre4[:, :, pc * VBLK:pe_ * VBLK],
                                  in_=vpt4[:, :, pc * VBLK:pe_ * VBLK])

            # ====== wh[s, d] chunks -> wsu slot0 (fp8), ws_b folded to ws ==
            # wsu[(b,par)]: [128, kc, 2, 128] fp8; slot0 = wh (s-partition),
            # slot1 rows 0..TS = ws rows of t-block (t-partition), rest zero.
            wsu_tiles = {}
            for b_ in range(BL):
                for par in range(2):
                    w_ = sp.tile([128, KC, 2, 128], F8, tag=f"wsu{b_}_{par}")
                    wsu_tiles[(b_, par)] = w_
                    nc.gpsimd.memset(w_[:, :, 1, :], 0)
            for b_ in range(BL):
                whp = ppw.tile([S, D], F32, tag="ws", name=f"whp{b_}")
                for kc in range(KC):
                    nc.tensor.matmul(out=whp[:], lhsT=encT4[:, kc, b_, :],
                                     rhs=whT4[:, kc, :],
                                     start=(kc == 0), stop=(kc == KC - 1))
                whp3 = whp[:].rearrange("p (kc d) -> p kc d", kc=KC)
                for par in range(2):
                    nc.vector.tensor_copy(
                        out=wsu_tiles[(b_, par)][:, :, 0, :], in_=whp3[:, :, :])

            eps = ppe.tile([S, BL * T], F32, tag="eps")   # scores [s,(b,t)]

            # ========== LSTM with interleaved score tasks ==========
            # task_a: fp8 DoubleRow outer-sum + ACT tanh (lags LSTM);
            # task_b: eps dot matmuls, emitted later still.
            pend_a = []
            pend_b = []
            th_tiles = {}

            su_tiles = {}

            def emit_su(b, tsub, dc):
                # PE part of a score task: no h-dependency, runs in PE idle
                su = pps.tile([128, TS * S], F32, tag="sum",
                              name=f"su{b}_{tsub}_{dc}")
                wsu = wsu_tiles[(b, tsub % 2)]
                for cc in range(TS * S // CC):
                    nc.tensor.matmul(out=su[:, cc * CC:(cc + 1) * CC],
                                     lhsT=wsu[:, dc, :, :],
                                     rhs=selv3[:, :, cc * CC:(cc + 1) * CC],
                                     start=True, stop=True, perf_mode=DR)
                su_tiles[(b, tsub, dc)] = su
                pend_th.append((b, tsub, dc))

            HH = TS * S // 2
            thA_done = {}

            def emit_thA(b, tsub, dc):
                # first tanh half: fills the ACT idle gap while DVE runs
                # the cell update, ahead of tanh(c)
                th = thp.tile([128, TS * S], BF, tag=f"th{dc}",
                              name=f"th{b}_{tsub}_{dc}")
                nc.scalar.activation(out=th[:, 0:HH],
                                     in_=su_tiles[(b, tsub, dc)][:, 0:HH],
                                     func=AF.Tanh)
                thA_done[(b, tsub, dc)] = th

            def emit_thB(b, tsub, dc):
                th = thA_done.pop((b, tsub, dc))
                nc.scalar.activation(out=th[:, HH:],
                                     in_=su_tiles.pop((b, tsub, dc))[:, HH:],
                                     func=AF.Tanh)
                th_tiles[(b, tsub, dc)] = th
                if dc == KC - 1:
                    pend_b.append((b, tsub))

            def emit_th(b, tsub, dc):
                # unsplit path for the post-loop drain (fewer overheads)
                th = thp.tile([128, TS * S], BF, tag=f"th{dc}",
                              name=f"th{b}_{tsub}_{dc}")
                nc.scalar.activation(out=th[:],
                                     in_=su_tiles.pop((b, tsub, dc))[:],
                                     func=AF.Tanh)
                th_tiles[(b, tsub, dc)] = th
                if dc == KC - 1:
                    pend_b.append((b, tsub))

            eps_done = [False] * BL

            def emit_b(b, tsub, half):
                for tl in range(half * (TS // 2), (half + 1) * (TS // 2)):
                    t = tsub * TS + tl
                    for d2 in range(KC):
                        nc.tensor.matmul(
                            out=eps[:, b * T + t: b * T + t + 1],
                            lhsT=th_tiles[(b, tsub, d2)][:,
                                tl * S:(tl + 1) * S],
                            rhs=vt[:, d2:d2 + 1],
                            start=(d2 == 0), stop=(d2 == KC - 1))
                if tsub == T // TS - 1 and half == 1:
                    eps_done[b] = True

            pend_th = []

            def pump(hint_ns=None):
                # su one step ahead of its tanh: PE part never blocks ACT.
                # thA was emitted inside the LSTM block; finish with thB.
                if pend_th:
                    head = pend_th[0]
                    if head in thA_done:
                        emit_thB(*pend_th.pop(0))
                    else:
                        emit_th(*pend_th.pop(0))
                if pend_a:
                    emit_su(*pend_a.pop(0))
                if pend_b:
                    b, tsub = pend_b[0]
                    half = pump.half
                    emit_b(b, tsub, half)
                    if half == 1:
                        pend_b.pop(0)
                    pump.half = 1 - half
            pump.half = 0

            from contextlib import ExitStack
            for t in range(T):
                _hp = ExitStack()
                _hp.enter_context(tc.high_priority())
                gps = ppg.tile([128, 16 * BL], F32, tag="gps",
                               name=f"gps{t}")
                nc.tensor.matmul(out=gps[:], lhsT=ident[:],
                                 rhs=xg[:, t * 32:(t + 1) * 32],
                                 start=True, stop=False,
                                 skip_group_check=True)
                for j in range(16):
                    wt = whha if j < 8 else whhb
                    jr = j % 8
                    for kc in range(KC):
                        if t == 0:
                            hsrc = h0b[:, kc * BL:(kc + 1) * BL]
                        else:
                            hsrc = outT4[:, kc, :, t - 1]
                        nc.tensor.matmul(
                            out=gps[:, j * BL:(j + 1) * BL],
                            lhsT=wt[:, (jr * KC + kc) * 128:
                                    (jr * KC + kc + 1) * 128],
                            rhs=hsrc, start=False, stop=(kc == KC - 1),
                            skip_group_check=True)
                # ONE sigmoid for all four gates (g rows host-doubled):
                # tanh(g) = 2*sigmoid(2g) - 1, absorbed into the DVE ops.
                # Each ACT instruction costs ~400 ns of pipeline hold on
                # top of its work, so fewer/larger ACT ops win.
                sio = gp.tile([128, 16 * BL], F32, tag="sio")
                nc.scalar.activation(out=sio[:],
                                     in_=gps[:], func=AF.Sigmoid)
                t1 = gp.tile([128, KC * BL], F32, tag="t1")
                t2 = gp.tile([128, KC * BL], F32, tag="t2")
                nc.vector.tensor_mul(out=t1[:], in0=sio[:, 4 * BL:8 * BL],
                                     in1=c[:])
                # t2 = (2*sig(2g)) * sig(i)
                nc.vector.scalar_tensor_tensor(
                    out=t2[:], in0=sio[:, 12 * BL:16 * BL], scalar=2.0,
                    in1=sio[:, 0:4 * BL], op0=ALU.mult, op1=ALU.mult)
                t3 = gp.tile([128, KC * BL], F32, tag="t3")
                nc.vector.tensor_add(out=t3[:], in0=t1[:], in1=t2[:])
                nc.vector.tensor_sub(out=c[:], in0=t3[:],
                                     in1=sio[:, 0:4 * BL])
                tc_ = gp.tile([128, KC * BL], F32, tag="tc")
                nc.scalar.activation(out=tc_[:], in_=c[:], func=AF.Tanh)
                nc.vector.tensor_mul(out=outT4[:, :, :, t],
                                     in0=sio[:, 8 * BL:12 * BL], in1=tc_[:])
                _hp.close()

                with tc.tile_wait_until((WAITB + WAITP * t) / 1e6):
                    pump(hint_ns=WAITB + WAITP * t)

                if stage >= 2 and t % TS == TS - 1:
                    tsub = t // TS
                    tc.tile_set_cur_wait((WAITB + WAITP * t) / 1e6)
                    for b in range(BL):
                        wps = ppw.tile([TS, D], F32, tag="ws",
                                       name=f"wps{b}_{tsub}")
                        for kc in range(KC):
                            nc.tensor.matmul(
                                out=wps[:],
                                lhsT=outT[:, (kc * BL + b) * T + tsub * TS:
                                          (kc * BL + b) * T + tsub * TS + TS],
                                rhs=wsT4[:, kc, :],
                                start=(kc == 0), stop=(kc == KC - 1))
                        wps3 = wps[:].rearrange("p (kc d) -> p kc d", kc=KC)
                        wsb3 = wsb8[:].rearrange("p (kc d) -> p kc d", kc=KC)
                        nc.vector.tensor_add(
                            out=wsu_tiles[(b, tsub % 2)][0:TS, :, 1, :],
                            in0=wps3[:, :, :], in1=wsb3[:, :, :])
                    tc.cur_wait_ts = 0
                    pend_a.extend((b, tsub, dc) for b in range(BL)
                                  for dc in range(KC))

            # ============ softmax, context, out2 (per batch) ============
            ctxT = sp.tile([128, BL * KC * T], BF)   # [p,(b,dc,t)]
            o2T = sp.tile([128, KC * BL * T], BF)    # [p,(ec,b,t)]

            def sm_b(b):
                _hp2 = ExitStack()
                _hp2.enter_context(tc.high_priority())
                ebf = ap_.tile([S, T], BF, tag="ebf")
                nc.scalar.activation(out=ebf[:],
                                     in_=eps[:, b * T:(b + 1) * T],
                                     func=AF.Exp)
                etp = ppw.tile([T, S], BF, tag="ws", name=f"etp{b}")
                nc.tensor.transpose(out=etp[:], in_=ebf[:],
                                    identity=ident[:, :])
                ssum = ap_.tile([T, 1], F32, tag="ssum")
                nc.vector.tensor_reduce(out=ssum[:], in_=etp[:],
                                        axis=mybir.AxisListType.X, op=ALU.add)
                rsum = ap_.tile([T, 1], F32, tag="rsum")
                nc.vector.reciprocal(out=rsum[:], in_=ssum[:])
                abf = ap_.tile([T, S], BF, tag="abf")
                nc.vector.tensor_scalar_mul(out=abf[:], in0=etp[:],
                                            scalar1=rsum[:])
                atp = ppw.tile([S, T], BF, tag="ws", name=f"atp{b}")
                nc.tensor.transpose(out=atp[:], in_=abf[:],
                                    identity=ident[0:T, 0:T])
                atb = ap_.tile([S, T], BF, tag="atb")
                nc.vector.tensor_copy(out=atb[:], in_=atp[:])
                for dc in range(KC):
                    cps = ppg.tile([128, T], F32, tag="gps",
                                   name=f"cps{b}_{dc}")
                    nc.tensor.matmul(out=cps[:],
                                     lhsT=enc[:, b * D + dc * 128:
                                              b * D + (dc + 1) * 128],
                                     rhs=atb[:], start=True, stop=True)
                    nc.vector.tensor_copy(
                        out=ctxT[:, (b * KC + dc) * T:(b * KC + dc + 1) * T],
                        in_=cps[:])
                for ec in range(KC):
                    ops = ppg.tile([128, T], F32, tag="gps",
                                   name=f"ops{b}_{ec}")
                    for kc in range(KC):
                        nc.tensor.matmul(
                            out=ops[:],
                            lhsT=vw1[:, kc * D + ec * 128:
                                     kc * D + (ec + 1) * 128],
                            rhs=ctxT[:, (b * KC + kc) * T:
                                     (b * KC + kc + 1) * T],
                            start=(kc == 0), stop=False)
                    for kc in range(KC):
                        nc.tensor.matmul(
                            out=ops[:],
                            lhsT=vw2[:, kc * D + ec * 128:
                                     kc * D + (ec + 1) * 128],
                            rhs=outT[:, (kc * BL + b) * T:
                                     (kc * BL + b + 1) * T],
                            start=False, stop=(kc == KC - 1))
                    nc.vector.tensor_scalar(
                        out=o2T[:, (ec * BL + b) * T:(ec * BL + b) * T + T],
                        in0=ops[:], scalar1=vb[:, ec:ec + 1], scalar2=None,
                        op0=ALU.add)
                _hp2.close()

            # drain: all remaining su first (PE), then tanh/eps; each
            # batch's softmax/out2 is emitted the moment its scores finish,
            # overlapping the other batch's tanh drain
            sm_done = set()
            while pend_a:
                emit_su(*pend_a.pop(0))
            while pend_th or pend_b:
                pump()
                if stage >= 2:
                    for b_ in range(BL):
                        if eps_done[b_] and b_ not in sm_done:
                            sm_b(b_)
                            sm_done.add(b_)
            if stage >= 2:
                for b_ in range(BL):
                    if b_ not in sm_done:
                        sm_b(b_)

            # ================= vocab projection (bf16) ==========
            o2r = o2T[:].rearrange("p (e c) -> p e c", e=KC, c=128)
            # Recycle dead phase-1 SBUF slots as stream buffers for the tail
            # vocab blocks: their DMAs fire as soon as the old tiles' readers
            # retire, moving DMA out of the saturated vocab tail.
            GRP2 = ("whha", "whhb", "xg")   # 8KB slots: 2 blocks each
            NRG = 2 * len(GRP2)
            RS1 = ["whT", "wsT", "vw1", "vw2"]
            # 2KB slots that free when the last score task retires; pairs
            # stage one block each as two [128,2,512] half-tiles
            RS2 = [("th0", "th1"), ("th2", "th3"), ("th0", "th1"),
                   ("th2", "th3"), ("selv", "encT")]
            rg0 = NBLK - NRG - len(RS1)  # whh group covers rg0..rg0+3
            rs2_0 = rg0 - len(RS2)
            rs3_0 = rs2_0 - 2            # wsu-quad + outT/ctxT/wsb8 blocks
            rsrc = {}
            if stage >= 3:
                # 8 KB slots -> two blocks each
                for gi, wtag in enumerate(GRP2):
                    ib_g = rg0 + gi * 2
                    vg = cp.tile([128, KC, 2 * VBLK], BF, tag=wtag,
                                 name=f"vgrp{gi}")
                    nc.sync.dma_start(
                        out=vg[:, :, :],
                        in_=vpt4[:, :, ib_g * VBLK:(ib_g + 2) * VBLK])
                    rsrc[ib_g] = (vg, 0)
                    rsrc[ib_g + 1] = (vg, VBLK)
                # quad block from the four 1KB wsu slots (kc0..3)
                ibx = rs3_0
                vqs = []
                for b_ in range(BL):
                    for par in range(2):
                        vq = sp.tile([128, 1, VBLK], BF,
                                     tag=f"wsu{b_}_{par}",
                                     name=f"vq{b_}_{par}")
                        kcq = b_ * 2 + par
                        nc.sync.dma_start(
                            out=vq[:, :, :],
                            in_=vpt4[:, kcq:kcq + 1,
                                     ibx * VBLK:(ibx + 1) * VBLK])
                        vqs.append((vq, 0))
                rsrc[ibx] = ("quad", vqs)
                # quad block from outT (1KB) + ctxT (1KB) + enc (2KB) slots
                iby = rs3_0 + 1
                vq_o = sp.tile([128, 1, VBLK], BF, tag="outT", name="vqo")
                nc.sync.dma_start(out=vq_o[:, :, :],
                                  in_=vpt4[:, 0:1, iby * VBLK:
                                           (iby + 1) * VBLK])
                vq_c = sp.tile([128, 1, VBLK], BF, tag="ctxT", name="vqc")
                nc.sync.dma_start(out=vq_c[:, :, :],
                                  in_=vpt4[:, 1:2, iby * VBLK:
                                           (iby + 1) * VBLK])
                vq_e2 = cp.tile([128, 2, VBLK], BF, tag="enc", name="vqe2")
                nc.sync.dma_start(out=vq_e2[:, :, :],
                                  in_=vpt4[:, 2:4, iby * VBLK:
                                           (iby + 1) * VBLK])
                rsrc[iby] = ("quad", [(vq_o, 0), (vq_c, 0),
                                      (vq_e2, 0), (vq_e2, 1)])
                for i, (tga, tgb) in enumerate(RS2):
                    ib_r = rs2_0 + i
                    v0r = ib_r * VBLK
                    pa = thp if tga.startswith("th") else cp
                    pb = thp if tgb.startswith("th") else cp
                    ta = pa.tile([128, 2, VBLK], BF, tag=tga,
                                 name=f"vspl{ib_r}a")
                    nc.sync.dma_start(out=ta[:, :, :],
                                      in_=vpt4[:, 0:2, v0r:v0r + VBLK])
                    tb = pb.tile([128, 2, VBLK], BF, tag=tgb,
                                 name=f"vspl{ib_r}b")
                    nc.sync.dma_start(out=tb[:, :, :],
                                      in_=vpt4[:, 2:4, v0r:v0r + VBLK])
                    rsrc[ib_r] = ("split", ta, tb)
                for i, tg in enumerate(RS1):
                    ib_r = rg0 + NRG + i
                    v0r = ib_r * VBLK
                    wr = min(VBLK, V - v0r)
                    vrt = cp.tile([128, KC, VBLK], BF, tag=tg,
                                  name=f"vrt{ib_r}")
                    nc.sync.dma_start(out=vrt[:, :, :wr],
                                      in_=vpt4[:, :, v0r:v0r + wr])
                    rsrc[ib_r] = (vrt, 0)
            # consumption order: alternate streamed/prefetched so stream DMAs
            # never stall; recycled-slot blocks go last (data arrives
            # mid-phase)
            tail0 = rs3_0 - (rs3_0 % 4)   # align tail to lsb store groups
            order = []
            si, pi = NPRE, 0
            while si < tail0 or pi < NPRE:
                if si < tail0:
                    order.append(si)
                    si += 1
                if pi < NPRE:
                    order.append(pi)
                    pi += 1
            order += list(range(tail0, NBLK))
            lsb_tiles = {}
            if stage < 3:
                order = []
            for nb, ib in enumerate(order):
                v0 = ib * VBLK
                w = min(VBLK, V - v0)
                if ib < NPRE:
                    def rhs_of(kc, v0=v0, w=w):
                        return vpre4[:, kc, v0:v0 + w]
                elif ib in rsrc:
                    ent = rsrc[ib]
                    if ent[0] == "split":
                        def rhs_of(kc, ta=ent[1], tb=ent[2], w=w):
                            return (ta if kc < 2 else tb)[:, kc % 2, 0:w]
                    elif ent[0] == "quad":
                        def rhs_of(kc, lst=ent[1], w=w):
                            t_, ix = lst[kc]
                            return t_[:, ix, 0:w]
                    else:
                        def rhs_of(kc, vs3=ent[0], voff=ent[1], w=w):
                            return vs3[:, kc, voff:voff + w]
                else:
                    vst = vp.tile([128, KC, VBLK], BF, tag="vs", bufs=NSTRB,
                                  name=f"vst{ib}")
                    nc.sync.dma_start(out=vst[:, :, :w],
                                      in_=vpt4[:, :, v0:v0 + w])
                    def rhs_of(kc, vst=vst, w=w):
                        return vst[:, kc, 0:w]
                grp = ib // 4
                if grp not in lsb_tiles:
                    lsb_tiles[grp] = [lp.tile([128, 4 * VBLK], BF, tag="lsb",
                                              name=f"lsb{grp}"), 0]
                lsb_e = lsb_tiles[grp]
                lps = ppg.tile([128, VBLK], F32, tag="gps", name=f"lps{ib}")
                for kc in range(KC):
                    nc.tensor.matmul(out=lps[:, :w],
                                     lhsT=o2r[:, kc, :],
                                     rhs=rhs_of(kc),
                                     start=(kc == 0), stop=(kc == KC - 1))
                dst = lsb_e[0][:, (ib % 4) * VBLK:(ib % 4) * VBLK + w]
                if nb % 2 == 0:
                    nc.scalar.copy(out=dst, in_=lps[:, :w])
                else:
                    nc.vector.tensor_copy(out=dst, in_=lps[:, :w])
                lsb_e[1] += 1
                nblk_grp = min(4, NBLK - grp * 4)
                if lsb_e[1] == nblk_grp:
                    gv0 = grp * 4 * VBLK
                    wlen = min(4 * VBLK, V - gv0)
                    nc.sync.dma_start(out=t_out[:, gv0:gv0 + wlen],
                                      in_=lsb_e[0][:, :wlen])

    nc.compile()
    return nc


def _prep_in_maps(inputs):
    inp = {k: np.asarray(v) for k, v in inputs.items()}
    words = inp["words"].astype(np.int64)
    enc = inp["encoder_output"].astype(np.float32)
    pre_h, cell = inp["pre_h"], inp["cell"]
    emb = inp["emb"]
    W_ih, W_hh = inp["W_ih"], inp["W_hh"]
    b_ih, b_hh = inp["b_ih"], inp["b_hh"]
    Wh_w = inp["Wh_w"]
    Ws_w, Ws_b = inp["Ws_w"], inp["Ws_b"]
    vt_w = inp["vt_w"]
    V_w, V_b = inp["V_w"], inp["V_b"]
    Vp_w, Vp_b = inp["Vp_w"], inp["Vp_b"]

    def re_lhsT(m, dt=BF16):  # [512, N] -> [128, 4*N] chunk-major
        n = m.shape[1]
        return np.ascontiguousarray(
            m.reshape(4, 128, n).transpose(1, 0, 2).reshape(128, 4 * n)
        ).astype(dt)

    # gate reorder (i,f,g,o) -> (i,f,o,g); g rows doubled so a single
    # sigmoid yields sig(2g) and tanh(g) = 2*sig(2g) - 1 on device
    perm = np.r_[0:512, 512:1024, 1536:2048, 1024:1536]
    W_ih_p = W_ih[perm].copy()
    W_hh_p = W_hh[perm].copy()
    b2 = (b_ih + b_hh)[perm].astype(np.float32)
    W_ih_p[1536:] *= 2.0
    W_hh_p[1536:] *= 2.0
    b2[1536:] *= 2.0

    whh_re = re_lhsT(np.ascontiguousarray(W_hh_p.T))     # [p,(kc,g)]
    # -> j-major [p,(j,kc,128)]
    whh_j = np.ascontiguousarray(
        whh_re.reshape(128, KC, 16, 128).transpose(0, 2, 1, 3)
        .reshape(128, 16 * KC * 128))
    whha_re = np.ascontiguousarray(whh_j[:, :8 * KC * 128])
    whhb_re = np.ascontiguousarray(whh_j[:, 8 * KC * 128:])
    whT_re = re_lhsT(np.ascontiguousarray(Wh_w.T))
    wsT_re = re_lhsT(np.ascontiguousarray(Ws_w.T))
    vw1_re = re_lhsT(np.ascontiguousarray(V_w[:, :D].T))
    vw2_re = re_lhsT(np.ascontiguousarray(V_w[:, D:].T))
    vpt_re = re_lhsT(np.ascontiguousarray(Vp_w.T))
    wsb8_re = np.tile(Ws_b.reshape(1, D), (TS, 1)).astype(BF16)
    vb_re = np.ascontiguousarray(V_b.reshape(4, 128).T).astype(np.float32)
    vt_re = np.ascontiguousarray(vt_w.reshape(4, 128).T).astype(BF16)
    ident_re = np.eye(128, dtype=np.float32).astype(BF16)
    # DR selector: slot0[p,(tl,s)] = [p==s], slot1[p,(tl,s)] = [p==tl]
    sel0 = np.tile(np.eye(128, dtype=np.float32), (1, TS))
    sel1 = np.zeros((128, TS * S), dtype=np.float32)
    for tl in range(TS):
        sel1[tl, tl * S:(tl + 1) * S] = 1.0
    selv_re = np.concatenate([sel0, sel1], axis=1).astype(FP8)

    x_all = emb[words]                                   # [B,T,D]
    xg_all = x_all @ W_ih_p.T.astype(np.float32) + b2    # [B,T,4D]

    in_maps = []
    for k in range(NC):
        bs = slice(k * BL, (k + 1) * BL)
        xgl = xg_all[bs]                                 # [2,T,2048]
        xg_re = np.ascontiguousarray(
            xgl.reshape(BL, T, 16, 128).transpose(3, 1, 2, 0)
            .reshape(128, T * 16 * BL)).astype(BF16)     # [p,(t,j,b)]
        h0 = np.ascontiguousarray(
            pre_h[bs].reshape(BL, 4, 128).transpose(2, 1, 0)
            .reshape(128, 4 * BL)).astype(BF16)
        c0 = np.ascontiguousarray(
            cell[bs].reshape(BL, 4, 128).transpose(2, 1, 0)
            .reshape(128, 4 * BL)).astype(np.float32)
        encl = enc[bs]                                   # [2,S,D]
        encT_re = np.ascontiguousarray(
            encl.reshape(BL, S, 4, 128).transpose(3, 2, 0, 1)
            .reshape(128, 4 * BL * S)).astype(BF16)
        enc_re = np.ascontiguousarray(
            encl.transpose(1, 0, 2).reshape(S, BL * D)).astype(BF16)
        in_maps.append({
            "xg": xg_re, "whha": whha_re, "whhb": whhb_re, "h0": h0,
            "c0": c0, "encT": encT_re, "enc": enc_re, "whT": whT_re,
            "wsT": wsT_re, "wsb8": wsb8_re, "vt": vt_re, "selv": selv_re,
            "vw1": vw1_re, "vw2": vw2_re, "vb": vb_re, "vpt": vpt_re,
            "ident": ident_re,
        })
    return in_maps


def kernel(**inputs):
    in_maps = _prep_in_maps(inputs)
    if "nc" not in _cached:
        _cached["nc"] = _build_nc()
    res = bass_utils.run_bass_kernel_spmd(_cached["nc"], in_maps,
                                          core_ids=list(range(NC)))
    vpb = np.asarray(inputs["Vp_b"]).astype(np.float32)
    outs = [np.asarray(res.results[k]["out"]).astype(np.float32)
            .reshape(BL, T, V) for k in range(NC)]
    return np.concatenate(outs, axis=0) + vpb[None, None, :]


if __name__ == "__main__":
    pass


# revision 57
# speedup vs baseline: 1.2062x; 1.0057x over previous
"""AttnOutputDecoder Trainium2 kernel.

Sharding: data-parallel over batch B=16 across 8 cores (2 batches/core).
Per core: LSTM (gate order i,f,o,g; host-precomputed x@W_ih injected into
PSUM via identity matmul) overlapped with Bahdanau attention scores.
Score tanh-args wh[s]+ws[t] are built as a single fp8 DoubleRow matmul per
256-col chunk: lhsT packs [wh-chunk ; ws-rows] in fp8 (accuracy verified:
adds ~2e-3 rel err), rhs is an exact 0/1 selector matrix in fp8, so each
element is written once at 0.5 cyc/row instead of twice at 1.0. Ws_b is
folded into the ws tile copy. Output projection streams Vp_w.T in bf16
(fp8 fails the 2e-2 gate); logits stored bf16; Vp_b added on host.
"""

import numpy as np
import ml_dtypes

import concourse.bass as bass
import concourse.mybir as mybir
import concourse.tile as tile
from concourse import bacc
from concourse import bass_utils

BF16 = ml_dtypes.bfloat16
FP8 = ml_dtypes.float8_e4m3
F32 = mybir.dt.float32
BF = mybir.dt.bfloat16
F8 = mybir.dt.float8e4
AF = mybir.ActivationFunctionType
ALU = mybir.AluOpType
DR = mybir.MatmulPerfMode.DoubleRow

B, T, S, D, V = 16, 64, 128, 512, 32000
NC = 8
BL = B // NC          # local batches per core = 2
R = BL * T            # local rows = 128
G4 = 4 * D            # 2048 gates
KC = D // 128         # 4 contraction chunks
TS = 8                # score t-sub-block
CC = 256              # su DoubleRow column chunk
VBLK = 512
NBLK = (V + VBLK - 1) // VBLK   # 63
NPRE = 24             # prefetched vocab blocks
NSTRB = 6             # streamed-vocab buffer slots (1 block each)
WAITP = 2350          # pacing period hint (ns/step) for score tasks
WAITB = 8500          # pacing base offset (ns)
WAITD = 700           # extra tanh release delay past its su matmuls (ns)

_cached = {}


def _build_nc(stage=3):
    # stage 1: LSTM only; 2: + scores/softmax/out2; 3: full (vocab)
    nc = bacc.Bacc("TRN2", target_bir_lowering=False, debug=False,
                   num_devices=NC)

    def din(name, shape, dt):
        return nc.dram_tensor(name, shape, dt, kind="ExternalInput").ap()

    t_ident = din("ident", [128, 128], BF)
    t_h0 = din("h0", [128, KC * BL], BF)
    t_c0 = din("c0", [128, KC * BL], F32)
    t_whha = din("whha", [128, 8 * KC * 128], BF)    # [p,(j<8,kc,g)]
    t_whhb = din("whhb", [128, 8 * KC * 128], BF)    # [p,(j>=8,kc,g)]
    t_xg = din("xg", [128, T * 16 * BL], BF)         # [p,(t,j,b)]
    t_encT = din("encT", [128, KC * BL * S], BF)     # [p,(kc,b,s)]
    t_whT = din("whT", [128, KC * D], BF)            # Wh_w.T re
    t_wsT = din("wsT", [128, KC * D], BF)            # Ws_w.T re
    t_wsb8 = din("wsb8", [TS, D], BF)                # Ws_b row x TS
    t_vt = din("vt", [128, KC], BF)
    t_selv = din("selv", [128, 2 * TS * S], F8)      # DR selectors
    t_enc = din("enc", [128, BL * D], BF)            # [s,(b,d)]
    t_vw1 = din("vw1", [128, KC * D], BF)
    t_vw2 = din("vw2", [128, KC * D], BF)
    t_vb = din("vb", [128, KC], F32)
    t_vpt = din("vpt", [128, KC * V], BF)            # [p,(kc,v)] Vp_w.T re
    t_out = nc.dram_tensor("out", [R, V], BF, kind="ExternalOutput").ap()

    with tile.TileContext(nc) as tc:
        with (
            tc.tile_pool(name="const", bufs=1) as cp,
            tc.tile_pool(name="state", bufs=1) as sp,
            tc.tile_pool(name="gates", bufs=8) as gp,
            tc.tile_pool(name="attn", bufs=2) as ap_,
            tc.tile_pool(name="thp", bufs=2) as thp,
            tc.tile_pool(name="voc", bufs=2) as vp,
            tc.tile_pool(name="lsbp", bufs=3) as lp,
            tc.tile_pool(name="ps_g", bufs=2, space="PSUM") as ppg,
            tc.tile_pool(name="ps_sum", bufs=2, space="PSUM") as pps,
            tc.tile_pool(name="ps_e", bufs=1, space="PSUM") as ppe,
            tc.tile_pool(name="ps_w", bufs=1, space="PSUM") as ppw,
        ):
            # ---- LSTM-critical loads first (serial DMA device) ----
            ident = cp.tile([128, 128], BF)
            nc.sync.dma_start(out=ident[:], in_=t_ident[:])
            h0b = sp.tile([128, KC * BL], BF)
            nc.sync.dma_start(out=h0b[:], in_=t_h0[:])
            c = sp.tile([128, KC * BL], F32)
            nc.sync.dma_start(out=c[:], in_=t_c0[:])
            whha = cp.tile([128, 8 * KC * 128], BF)
            nc.sync.dma_start(out=whha[:], in_=t_whha[:])
            xg = cp.tile([128, T * 16 * BL], BF)
            nc.sync.dma_start(out=xg[:], in_=t_xg[:])
            whhb = cp.tile([128, 8 * KC * 128], BF)
            nc.sync.dma_start(out=whhb[:], in_=t_whhb[:])
            # ---- wh deps, then remaining constants ----
            encT = cp.tile([128, KC * BL * S], BF)
            nc.sync.dma_start(out=encT[:], in_=t_encT[:])
            whT = cp.tile([128, KC * D], BF)
            nc.sync.dma_start(out=whT[:], in_=t_whT[:])
            wsT = cp.tile([128, KC * D], BF)
            nc.sync.dma_start(out=wsT[:], in_=t_wsT[:])
            wsb8 = cp.tile([TS, D], BF)
            nc.sync.dma_start(out=wsb8[:], in_=t_wsb8[:])
            vt = cp.tile([128, KC], BF)
            nc.sync.dma_start(out=vt[:], in_=t_vt[:])
            selv = cp.tile([128, 2 * TS * S], F8)
            nc.sync.dma_start(out=selv[:], in_=t_selv[:])
            enc = cp.tile([128, BL * D], BF)
            nc.sync.dma_start(out=enc[:], in_=t_enc[:])
            vw1 = cp.tile([128, KC * D], BF)
            nc.sync.dma_start(out=vw1[:], in_=t_vw1[:])
            vw2 = cp.tile([128, KC * D], BF)
            nc.sync.dma_start(out=vw2[:], in_=t_vw2[:])
            vb = cp.tile([128, KC], F32)
            nc.sync.dma_start(out=vb[:], in_=t_vb[:])

            encT4 = encT[:].rearrange("p (kc b s) -> p kc b s", kc=KC, b=BL)
            whT4 = whT[:].rearrange("p (kc d) -> p kc d", kc=KC)
            wsT4 = wsT[:].rearrange("p (kc d) -> p kc d", kc=KC)
            selv3 = selv[:].rearrange("p (i n) -> p i n", i=2)

            outT = sp.tile([128, KC * BL * T], BF)   # [p,(kc,b,t)] all h's
            outT4 = outT[:].rearrange("p (kc b t) -> p kc b t", kc=KC, b=BL,
                                      t=T)

            # ---- vocab weight prefetch: chunked large DMAs ----
            vpt4 = t_vpt[:].rearrange("p (kc v) -> p kc v", kc=KC, v=V)
            vpre = cp.tile([128, KC * NPRE * VBLK], BF)
            vpre4 = vpre[:].rearrange("p (kc v) -> p kc v", kc=KC,
                                      v=NPRE * VBLK)
            PCH = 7
            for pc in range(0, NPRE, PCH):
                pe_ = min(NPRE, pc + PCH)
                nc.sync.dma_start(out=vpre4[:, :, pc * VBLK:pe_ * VBLK],
                                  in_=vpt4[:, :, pc * VBLK:pe_ * VBLK])
            # 3 more blocks pre-staged in the lsb slots (dead until
            # phase 3); consumed first there, after which the lsb group
            # tiles rotate into these slots via the tag WAR
            NLSB = 3
            vlsb = []
            for i in range(NLSB):
                vt_ = lp.tile([128, KC, VBLK], BF, tag="lsb",
                              name=f"vlsb{i}")
                nc.sync.dma_start(
                    out=vt_[:, :, :],
                    in_=vpt4[:, :, (NPRE + i) * VBLK:(NPRE + i + 1) * VBLK])
                vlsb.append(vt_)

            # ====== wh[s, d] chunks -> wsu slot0 (fp8), ws_b folded to ws ==
            # wsu[(b,par)]: [128, kc, 2, 128] fp8; slot0 = wh (s-partition),
            # slot1 rows 0..TS = ws rows of t-block (t-partition), rest zero.
            wsu_tiles = {}
            for b_ in range(BL):
                for par in range(2):
                    w_ = sp.tile([128, KC, 2, 128], F8, tag=f"wsu{b_}_{par}")
                    wsu_tiles[(b_, par)] = w_
                    nc.gpsimd.memset(w_[:, :, 1, :], 0)
            for b_ in range(BL):
                whp = ppw.tile([S, D], F32, tag="ws", name=f"whp{b_}")
                for kc in range(KC):
                    nc.tensor.matmul(out=whp[:], lhsT=encT4[:, kc, b_, :],
                                     rhs=whT4[:, kc, :],
                                     start=(kc == 0), stop=(kc == KC - 1))
                whp3 = whp[:].rearrange("p (kc d) -> p kc d", kc=KC)
                for par in range(2):
                    nc.vector.tensor_copy(
                        out=wsu_tiles[(b_, par)][:, :, 0, :], in_=whp3[:, :, :])

            eps = ppe.tile([S, BL * T], F32, tag="eps")   # scores [s,(b,t)]

            # ========== LSTM with interleaved score tasks ==========
            # task_a: fp8 DoubleRow outer-sum + ACT tanh (lags LSTM);
            # task_b: eps dot matmuls, emitted later still.
            pend_a = []
            pend_b = []
            th_tiles = {}

            su_tiles = {}

            def emit_su(b, tsub, dc):
                # PE part of a score task: no h-dependency, runs in PE idle
                su = pps.tile([128, TS * S], F32, tag="sum",
                              name=f"su{b}_{tsub}_{dc}")
                wsu = wsu_tiles[(b, tsub % 2)]
                for cc in range(TS * S // CC):
                    nc.tensor.matmul(out=su[:, cc * CC:(cc + 1) * CC],
                                     lhsT=wsu[:, dc, :, :],
                                     rhs=selv3[:, :, cc * CC:(cc + 1) * CC],
                                     start=True, stop=True, perf_mode=DR)
                su_tiles[(b, tsub, dc)] = su
                pend_th.append((b, tsub, dc))

            HH = TS * S // 2
            thA_done = {}

            def emit_thA(b, tsub, dc):
                # first tanh half: fills the ACT idle gap while DVE runs
                # the cell update, ahead of tanh(c)
                th = thp.tile([128, TS * S], BF, tag=f"th{dc}",
                              name=f"th{b}_{tsub}_{dc}")
                nc.scalar.activation(out=th[:, 0:HH],
                                     in_=su_tiles[(b, tsub, dc)][:, 0:HH],
                                     func=AF.Tanh)
                thA_done[(b, tsub, dc)] = th

            def emit_thB(b, tsub, dc):
                th = thA_done.pop((b, tsub, dc))
                nc.scalar.activation(out=th[:, HH:],
                                     in_=su_tiles.pop((b, tsub, dc))[:, HH:],
                                     func=AF.Tanh)
                th_tiles[(b, tsub, dc)] = th
                if dc == KC - 1:
                    pend_b.append((b, tsub))

            def emit_th(b, tsub, dc):
                # unsplit path for the post-loop drain (fewer overheads)
                th = thp.tile([128, TS * S], BF, tag=f"th{dc}",
                              name=f"th{b}_{tsub}_{dc}")
                nc.scalar.activation(out=th[:],
                                     in_=su_tiles.pop((b, tsub, dc))[:],
                                     func=AF.Tanh)
                th_tiles[(b, tsub, dc)] = th
                if dc == KC - 1:
                    pend_b.append((b, tsub))

            eps_done = [False] * BL

            def emit_b(b, tsub, half):
                for tl in range(half * (TS // 2), (half + 1) * (TS // 2)):
                    t = tsub * TS + tl
                    for d2 in range(KC):
                        nc.tensor.matmul(
                            out=eps[:, b * T + t: b * T + t + 1],
                            lhsT=th_tiles[(b, tsub, d2)][:,
                                tl * S:(tl + 1) * S],
                            rhs=vt[:, d2:d2 + 1],
                            start=(d2 == 0), stop=(d2 == KC - 1))
                if tsub == T // TS - 1 and half == 1:
                    eps_done[b] = True

            pend_th = []

            def pump(hint_ns=None):
                # su one step ahead of its tanh: PE part never blocks ACT.
                # thA was emitted inside the LSTM block; finish with thB.
                if pend_th:
                    head = pend_th[0]
                    if head in thA_done:
                        emit_thB(*pend_th.pop(0))
                    else:
                        emit_th(*pend_th.pop(0))
                if pend_a:
                    emit_su(*pend_a.pop(0))
                if pend_b:
                    b, tsub = pend_b[0]
                    half = pump.half
                    emit_b(b, tsub, half)
                    if half == 1:
                        pend_b.pop(0)
                    pump.half = 1 - half
            pump.half = 0

            from contextlib import ExitStack
            for t in range(T):
                _hp = ExitStack()
                _hp.enter_context(tc.high_priority())
                gps = ppg.tile([128, 16 * BL], F32, tag="gps",
                               name=f"gps{t}")
                nc.tensor.matmul(out=gps[:], lhsT=ident[:],
                                 rhs=xg[:, t * 32:(t + 1) * 32],
                                 start=True, stop=False,
                                 skip_group_check=True)
                for j in range(16):
                    wt = whha if j < 8 else whhb
                    jr = j % 8
                    for kc in range(KC):
                        if t == 0:
                            hsrc = h0b[:, kc * BL:(kc + 1) * BL]
                        else:
                            hsrc = outT4[:, kc, :, t - 1]
                        nc.tensor.matmul(
                            out=gps[:, j * BL:(j + 1) * BL],
                            lhsT=wt[:, (jr * KC + kc) * 128:
                                    (jr * KC + kc + 1) * 128],
                            rhs=hsrc, start=False, stop=(kc == KC - 1),
                            skip_group_check=True)
                # ONE sigmoid for all four gates (g rows host-doubled):
                # tanh(g) = 2*sigmoid(2g) - 1, absorbed into the DVE ops.
                # Each ACT instruction costs ~400 ns of pipeline hold on
                # top of its work, so fewer/larger ACT ops win.
                sio = gp.tile([128, 16 * BL], F32, tag="sio")
                nc.scalar.activation(out=sio[:],
                                     in_=gps[:], func=AF.Sigmoid)
                t1 = gp.tile([128, KC * BL], F32, tag="t1")
                t2 = gp.tile([128, KC * BL], F32, tag="t2")
                nc.vector.tensor_mul(out=t1[:], in0=sio[:, 4 * BL:8 * BL],
                                     in1=c[:])
                # t2 = (2*sig(2g)) * sig(i)
                nc.vector.scalar_tensor_tensor(
                    out=t2[:], in0=sio[:, 12 * BL:16 * BL], scalar=2.0,
                    in1=sio[:, 0:4 * BL], op0=ALU.mult, op1=ALU.mult)
                t3 = gp.tile([128, KC * BL], F32, tag="t3")
                nc.vector.tensor_add(out=t3[:], in0=t1[:], in1=t2[:])
                nc.vector.tensor_sub(out=c[:], in0=t3[:],
                                     in1=sio[:, 0:4 * BL])
                tc_ = gp.tile([128, KC * BL], F32, tag="tc")
                nc.scalar.activation(out=tc_[:], in_=c[:], func=AF.Tanh)
                nc.vector.tensor_mul(out=outT4[:, :, :, t],
                                     in0=sio[:, 8 * BL:12 * BL], in1=tc_[:])
                _hp.close()

                with tc.tile_wait_until((WAITB + WAITP * t) / 1e6):
                    pump(hint_ns=WAITB + WAITP * t)

                if stage >= 2 and t % TS == TS - 1:
                    tsub = t // TS
                    tc.tile_set_cur_wait((WAITB + WAITP * t) / 1e6)
                    for b in range(BL):
                        wps = ppw.tile([TS, D], F32, tag="ws",
                                       name=f"wps{b}_{tsub}")
                        for kc in range(KC):
                            nc.tensor.matmul(
                                out=wps[:],
                                lhsT=outT[:, (kc * BL + b) * T + tsub * TS:
                                          (kc * BL + b) * T + tsub * TS + TS],
                                rhs=wsT4[:, kc, :],
                                start=(kc == 0), stop=(kc == KC - 1))
                        wps3 = wps[:].rearrange("p (kc d) -> p kc d", kc=KC)
                        wsb3 = wsb8[:].rearrange("p (kc d) -> p kc d", kc=KC)
                        nc.vector.tensor_add(
                            out=wsu_tiles[(b, tsub % 2)][0:TS, :, 1, :],
                            in0=wps3[:, :, :], in1=wsb3[:, :, :])
                    tc.cur_wait_ts = 0
                    pend_a.extend((b, tsub, dc) for b in range(BL)
                                  for dc in range(KC))

            # ============ softmax, context, out2 (per batch) ============
            ctxT = sp.tile([128, BL * KC * T], BF)   # [p,(b,dc,t)]
            o2T = sp.tile([128, KC * BL * T], BF)    # [p,(ec,b,t)]

            def sm_b(b):
                _hp2 = ExitStack()
                _hp2.enter_context(tc.high_priority())
                ebf = ap_.tile([S, T], BF, tag="ebf")
                nc.scalar.activation(out=ebf[:],
                                     in_=eps[:, b * T:(b + 1) * T],
                                     func=AF.Exp)
                etp = ppw.tile([T, S], BF, tag="ws", name=f"etp{b}")
                nc.tensor.transpose(out=etp[:], in_=ebf[:],
                                    identity=ident[:, :])
                ssum = ap_.tile([T, 1], F32, tag="ssum")
                nc.vector.tensor_reduce(out=ssum[:], in_=etp[:],
                                        axis=mybir.AxisListType.X, op=ALU.add)
                rsum = ap_.tile([T, 1], F32, tag="rsum")
                nc.vector.reciprocal(out=rsum[:], in_=ssum[:])
                abf = ap_.tile([T, S], BF, tag="abf")
                nc.vector.tensor_scalar_mul(out=abf[:], in0=etp[:],
                                            scalar1=rsum[:])
                atp = ppw.tile([S, T], BF, tag="ws", name=f"atp{b}")
                nc.tensor.transpose(out=atp[:], in_=abf[:],
                                    identity=ident[0:T, 0:T])
                atb = ap_.tile([S, T], BF, tag="atb")
                nc.vector.tensor_copy(out=atb[:], in_=atp[:])
                for dc in range(KC):
                    cps = ppg.tile([128, T], F32, tag="gps",
                                   name=f"cps{b}_{dc}")
                    nc.tensor.matmul(out=cps[:],
                                     lhsT=enc[:, b * D + dc * 128:
                                              b * D + (dc + 1) * 128],
                                     rhs=atb[:], start=True, stop=True)
                    nc.vector.tensor_copy(
                        out=ctxT[:, (b * KC + dc) * T:(b * KC + dc + 1) * T],
                        in_=cps[:])
                for ec in range(KC):
                    ops = ppg.tile([128, T], F32, tag="gps",
                                   name=f"ops{b}_{ec}")
                    for kc in range(KC):
                        nc.tensor.matmul(
                            out=ops[:],
                            lhsT=vw1[:, kc * D + ec * 128:
                                     kc * D + (ec + 1) * 128],
                            rhs=ctxT[:, (b * KC + kc) * T:
                                     (b * KC + kc + 1) * T],
                            start=(kc == 0), stop=False)
                    for kc in range(KC):
                        nc.tensor.matmul(
                            out=ops[:],
                            lhsT=vw2[:, kc * D + ec * 128:
                                     kc * D + (ec + 1) * 128],
                            rhs=outT[:, (kc * BL + b) * T:
                                     (kc * BL + b + 1) * T],
                            start=False, stop=(kc == KC - 1))
                    nc.vector.tensor_scalar(
                        out=o2T[:, (ec * BL + b) * T:(ec * BL + b) * T + T],
                        in0=ops[:], scalar1=vb[:, ec:ec + 1], scalar2=None,
                        op0=ALU.add)
                _hp2.close()

            # drain: all remaining su first (PE), then tanh/eps; each
            # batch's softmax/out2 is emitted the moment its scores finish,
            # overlapping the other batch's tanh drain
            sm_done = set()
            while pend_a:
                emit_su(*pend_a.pop(0))
            while pend_th or pend_b:
                pump()
                if stage >= 2:
                    for b_ in range(BL):
                        if eps_done[b_] and b_ not in sm_done:
                            sm_b(b_)
                            sm_done.add(b_)
            if stage >= 2:
                for b_ in range(BL):
                    if b_ not in sm_done:
                        sm_b(b_)

            # ================= vocab projection (bf16) ==========
            o2r = o2T[:].rearrange("p (e c) -> p e c", e=KC, c=128)
            # Recycle dead phase-1 SBUF slots as stream buffers for the tail
            # vocab blocks: their DMAs fire as soon as the old tiles' readers
            # retire, moving DMA out of the saturated vocab tail.
            GRP2 = ("whha", "whhb", "xg")   # 8KB slots: 2 blocks each
            NRG = 2 * len(GRP2)
            RS1 = ["whT", "wsT", "vw1", "vw2"]
            # 2KB slots that free when the last score task retires; pairs
            # stage one block each as two [128,2,512] half-tiles
            RS2 = [("th0", "th1"), ("th2", "th3"), ("th0", "th1"),
                   ("th2", "th3"), ("selv", "encT")]
            rg0 = NBLK - NRG - len(RS1)  # whh group covers rg0..rg0+3
            rs2_0 = rg0 - len(RS2)
            rs3_0 = rs2_0 - 2            # wsu-quad + outT/ctxT/wsb8 blocks
            rsrc = {}
            if stage >= 3:
                # 8 KB slots -> two blocks each
                for gi, wtag in enumerate(GRP2):
                    ib_g = rg0 + gi * 2
                    vg = cp.tile([128, KC, 2 * VBLK], BF, tag=wtag,
                                 name=f"vgrp{gi}")
                    nc.sync.dma_start(
                        out=vg[:, :, :],
                        in_=vpt4[:, :, ib_g * VBLK:(ib_g + 2) * VBLK])
                    rsrc[ib_g] = (vg, 0)
                    rsrc[ib_g + 1] = (vg, VBLK)
                # quad block from the four 1KB wsu slots (kc0..3)
                ibx = rs3_0
                vqs = []
                for b_ in range(BL):
                    for par in range(2):
                        vq = sp.tile([128, 1, VBLK], BF,
                                     tag=f"wsu{b_}_{par}",
                                     name=f"vq{b_}_{par}")
                        kcq = b_ * 2 + par
                        nc.sync.dma_start(
                            out=vq[:, :, :],
                            in_=vpt4[:, kcq:kcq + 1,
                                     ibx * VBLK:(ibx + 1) * VBLK])
                        vqs.append((vq, 0))
                rsrc[ibx] = ("quad", vqs)
                # quad block from outT (1KB) + ctxT (1KB) + enc (2KB) slots
                iby = rs3_0 + 1
                vq_o = sp.tile([128, 1, VBLK], BF, tag="outT", name="vqo")
                nc.sync.dma_start(out=vq_o[:, :, :],
                                  in_=vpt4[:, 0:1, iby * VBLK:
                                           (iby + 1) * VBLK])
                vq_c = sp.tile([128, 1, VBLK], BF, tag="ctxT", name="vqc")
                nc.sync.dma_start(out=vq_c[:, :, :],
                                  in_=vpt4[:, 1:2, iby * VBLK:
                                           (iby + 1) * VBLK])
                vq_e2 = cp.tile([128, 2, VBLK], BF, tag="enc", name="vqe2")
                nc.sync.dma_start(out=vq_e2[:, :, :],
                                  in_=vpt4[:, 2:4, iby * VBLK:
                                           (iby + 1) * VBLK])
                rsrc[iby] = ("quad", [(vq_o, 0), (vq_c, 0),
                                      (vq_e2, 0), (vq_e2, 1)])
                for i, (tga, tgb) in enumerate(RS2):
                    ib_r = rs2_0 + i
                    v0r = ib_r * VBLK
                    pa = thp if tga.startswith("th") else cp
                    pb = thp if tgb.startswith("th") else cp
                    ta = pa.tile([128, 2, VBLK], BF, tag=tga,
                                 name=f"vspl{ib_r}a")
                    nc.sync.dma_start(out=ta[:, :, :],
                                      in_=vpt4[:, 0:2, v0r:v0r + VBLK])
                    tb = pb.tile([128, 2, VBLK], BF, tag=tgb,
                                 name=f"vspl{ib_r}b")
                    nc.sync.dma_start(out=tb[:, :, :],
                                      in_=vpt4[:, 2:4, v0r:v0r + VBLK])
                    rsrc[ib_r] = ("split", ta, tb)
                for i, tg in enumerate(RS1):
                    ib_r = rg0 + NRG + i
                    v0r = ib_r * VBLK
                    wr = min(VBLK, V - v0r)
                    vrt = cp.tile([128, KC, VBLK], BF, tag=tg,
                                  name=f"vrt{ib_r}")
                    nc.sync.dma_start(out=vrt[:, :, :wr],
                                      in_=vpt4[:, :, v0r:v0r + wr])
                    rsrc[ib_r] = (vrt, 0)
            # consumption order: alternate streamed/prefetched so stream DMAs
            # never stall; recycled-slot blocks go last (data arrives
            # mid-phase)
            tail0 = rs3_0 - (rs3_0 % 4)   # align tail to lsb store groups
            # lsb-staged blocks first: their matmuls must retire before the
            # group tiles rotate into those slots
            order = list(range(NPRE, NPRE + NLSB))
            si, pi = NPRE + NLSB, 0
            while si < tail0 or pi < NPRE:
                if si < tail0:
                    order.append(si)
                    si += 1
                if pi < NPRE:
                    order.append(pi)
                    pi += 1
            order += list(range(tail0, NBLK))
            lsb_tiles = {}
            if stage < 3:
                order = []
            for nb, ib in enumerate(order):
                v0 = ib * VBLK
                w = min(VBLK, V - v0)
                if ib < NPRE:
                    def rhs_of(kc, v0=v0, w=w):
                        return vpre4[:, kc, v0:v0 + w]
                elif ib < NPRE + NLSB:
                    def rhs_of(kc, vt_=vlsb[ib - NPRE], w=w):
                        return vt_[:, kc, 0:w]
                elif ib in rsrc:
                    ent = rsrc[ib]
                    if ent[0] == "split":
                        def rhs_of(kc, ta=ent[1], tb=ent[2], w=w):
                            return (ta if kc < 2 else tb)[:, kc % 2, 0:w]
                    elif ent[0] == "quad":
                        def rhs_of(kc, lst=ent[1], w=w):
                            t_, ix = lst[kc]
                            return t_[:, ix, 0:w]
                    else:
                        def rhs_of(kc, vs3=ent[0], voff=ent[1], w=w):
                            return vs3[:, kc, voff:voff + w]
                else:
                    vst = vp.tile([128, KC, VBLK], BF, tag="vs", bufs=NSTRB,
                                  name=f"vst{ib}")
                    nc.sync.dma_start(out=vst[:, :, :w],
                                      in_=vpt4[:, :, v0:v0 + w])
                    def rhs_of(kc, vst=vst, w=w):
                        return vst[:, kc, 0:w]
                grp = ib // 4
                if grp not in lsb_tiles:
                    lsb_tiles[grp] = [lp.tile([128, 4 * VBLK], BF, tag="lsb",
                                              name=f"lsb{grp}"), 0]
                lsb_e = lsb_tiles[grp]
                lps = ppg.tile([128, VBLK], F32, tag="gps", name=f"lps{ib}")
                for kc in range(KC):
                    nc.tensor.matmul(out=lps[:, :w],
                                     lhsT=o2r[:, kc, :],
                                     rhs=rhs_of(kc),
                                     start=(kc == 0), stop=(kc == KC - 1))
                dst = lsb_e[0][:, (ib % 4) * VBLK:(ib % 4) * VBLK + w]
                if nb % 2 == 0:
                    nc.scalar.copy(out=dst, in_=lps[:, :w])
                else:
                    nc.vector.tensor_copy(out=dst, in_=lps[:, :w])
                lsb_e[1] += 1
                nblk_grp = min(4, NBLK - grp * 4)
                if lsb_e[1] == nblk_grp:
                    gv0 = grp * 4 * VBLK
                    wlen = min(4 * VBLK, V - gv0)
                    nc.sync.dma_start(out=t_out[:, gv0:gv0 + wlen],
                                      in_=lsb_e[0][:, :wlen])

    nc.compile()
    return nc


def _prep_in_maps(inputs):
    inp = {k: np.asarray(v) for k, v in inputs.items()}
    words = inp["words"].astype(np.int64)
    enc = inp["encoder_output"].astype(np.float32)
    pre_h, cell = inp["pre_h"], inp["cell"]
    emb = inp["emb"]
    W_ih, W_hh = inp["W_ih"], inp["W_hh"]
    b_ih, b_hh = inp["b_ih"], inp["b_hh"]
    Wh_w = inp["Wh_w"]
    Ws_w, Ws_b = inp["Ws_w"], inp["Ws_b"]
    vt_w = inp["vt_w"]
    V_w, V_b = inp["V_w"], inp["V_b"]
    Vp_w, Vp_b = inp["Vp_w"], inp["Vp_b"]

    def re_lhsT(m, dt=BF16):  # [512, N] -> [128, 4*N] chunk-major
        n = m.shape[1]
        return np.ascontiguousarray(
            m.reshape(4, 128, n).transpose(1, 0, 2).reshape(128, 4 * n)
        ).astype(dt)

    # gate reorder (i,f,g,o) -> (i,f,o,g); g rows doubled so a single
    # sigmoid yields sig(2g) and tanh(g) = 2*sig(2g) - 1 on device
    perm = np.r_[0:512, 512:1024, 1536:2048, 1024:1536]
    W_ih_p = W_ih[perm].copy()
    W_hh_p = W_hh[perm].copy()
    b2 = (b_ih + b_hh)[perm].astype(np.float32)
    W_ih_p[1536:] *= 2.0
    W_hh_p[1536:] *= 2.0
    b2[1536:] *= 2.0

    whh_re = re_lhsT(np.ascontiguousarray(W_hh_p.T))     # [p,(kc,g)]
    # -> j-major [p,(j,kc,128)]
    whh_j = np.ascontiguousarray(
        whh_re.reshape(128, KC, 16, 128).transpose(0, 2, 1, 3)
        .reshape(128, 16 * KC * 128))
    whha_re = np.ascontiguousarray(whh_j[:, :8 * KC * 128])
    whhb_re = np.ascontiguousarray(whh_j[:, 8 * KC * 128:])
    whT_re = re_lhsT(np.ascontiguousarray(Wh_w.T))
    wsT_re = re_lhsT(np.ascontiguousarray(Ws_w.T))
    vw1_re = re_lhsT(np.ascontiguousarray(V_w[:, :D].T))
    vw2_re = re_lhsT(np.ascontiguousarray(V_w[:, D:].T))
    vpt_re = re_lhsT(np.ascontiguousarray(Vp_w.T))
    wsb8_re = np.tile(Ws_b.reshape(1, D), (TS, 1)).astype(BF16)
    vb_re = np.ascontiguousarray(V_b.reshape(4, 128).T).astype(np.float32)
    vt_re = np.ascontiguousarray(vt_w.reshape(4, 128).T).astype(BF16)
    ident_re = np.eye(128, dtype=np.float32).astype(BF16)
    # DR selector: slot0[p,(tl,s)] = [p==s], slot1[p,(tl,s)] = [p==tl]
    sel0 = np.tile(np.eye(128, dtype=np.float32), (1, TS))
    sel1 = np.zeros((128, TS * S), dtype=np.float32)
    for tl in range(TS):
        sel1[tl, tl * S:(tl + 1) * S] = 1.0
    selv_re = np.concatenate([sel0, sel1], axis=1).astype(FP8)

    x_all = emb[words]                                   # [B,T,D]
    xg_all = x_all @ W_ih_p.T.astype(np.float32) + b2    # [B,T,4D]

    in_maps = []
    for k in range(NC):
        bs = slice(k * BL, (k + 1) * BL)
        xgl = xg_all[bs]                                 # [2,T,2048]
        xg_re = np.ascontiguousarray(
            xgl.reshape(BL, T, 16, 128).transpose(3, 1, 2, 0)
            .reshape(128, T * 16 * BL)).astype(BF16)     # [p,(t,j,b)]
        h0 = np.ascontiguousarray(
            pre_h[bs].reshape(BL, 4, 128).transpose(2, 1, 0)
            .reshape(128, 4 * BL)).astype(BF16)
        c0 = np.ascontiguousarray(
            cell[bs].reshape(BL, 4, 128).transpose(2, 1, 0)
            .reshape(128, 4 * BL)).astype(np.float32)
        encl = enc[bs]                                   # [2,S,D]
        encT_re = np.ascontiguousarray(
            encl.reshape(BL, S, 4, 128).transpose(3, 2, 0, 1)
            .reshape(128, 4 * BL * S)).astype(BF16)
        enc_re = np.ascontiguousarray(
            encl.transpose(1, 0, 2).reshape(S, BL * D)).astype(BF16)
        in_maps.append({
            "xg": xg_re, "whha": whha_re, "whhb": whhb_re, "h0": h0,
            "c0": c0, "encT": encT_re, "enc": enc_re, "whT": whT_re,
            "wsT": wsT_re, "wsb8": wsb8_re, "vt": vt_re, "selv": selv_re,
            "vw1": vw1_re, "vw2": vw2_re, "vb": vb_re, "vpt": vpt_re,
            "ident": ident_re,
        })
    return in_maps


def kernel(**inputs):
    in_maps = _prep_in_maps(inputs)
    if "nc" not in _cached:
        _cached["nc"] = _build_nc()
    res = bass_utils.run_bass_kernel_spmd(_cached["nc"], in_maps,
                                          core_ids=list(range(NC)))
    vpb = np.asarray(inputs["Vp_b"]).astype(np.float32)
    outs = [np.asarray(res.results[k]["out"]).astype(np.float32)
            .reshape(BL, T, V) for k in range(NC)]
    return np.concatenate(outs, axis=0) + vpb[None, None, :]


if __name__ == "__main__":
    pass


# revision 58
# speedup vs baseline: 1.2158x; 1.0080x over previous
"""AttnOutputDecoder Trainium2 kernel.

Sharding: data-parallel over batch B=16 across 8 cores (2 batches/core).
Per core: LSTM (gate order i,f,o,g; host-precomputed x@W_ih injected into
PSUM via identity matmul) overlapped with Bahdanau attention scores.
Score tanh-args wh[s]+ws[t] are built as a single fp8 DoubleRow matmul per
256-col chunk: lhsT packs [wh-chunk ; ws-rows] in fp8 (accuracy verified:
adds ~2e-3 rel err), rhs is an exact 0/1 selector matrix in fp8, so each
element is written once at 0.5 cyc/row instead of twice at 1.0. Ws_b is
folded into the ws tile copy. Output projection streams Vp_w.T in bf16
(fp8 fails the 2e-2 gate); logits stored bf16; Vp_b added on host.
"""

import numpy as np
import ml_dtypes

import concourse.bass as bass
import concourse.mybir as mybir
import concourse.tile as tile
from concourse import bacc
from concourse import bass_utils

BF16 = ml_dtypes.bfloat16
FP8 = ml_dtypes.float8_e4m3
F32 = mybir.dt.float32
BF = mybir.dt.bfloat16
F8 = mybir.dt.float8e4
AF = mybir.ActivationFunctionType
ALU = mybir.AluOpType
DR = mybir.MatmulPerfMode.DoubleRow

B, T, S, D, V = 16, 64, 128, 512, 32000
NC = 8
BL = B // NC          # local batches per core = 2
R = BL * T            # local rows = 128
G4 = 4 * D            # 2048 gates
KC = D // 128         # 4 contraction chunks
TS = 8                # score t-sub-block
CC = 256              # su DoubleRow column chunk
VBLK = 512
NBLK = (V + VBLK - 1) // VBLK   # 63
NPRE = 24             # prefetched vocab blocks
NSTRB = 6             # streamed-vocab buffer slots (1 block each)
WAITP = 2350          # pacing period hint (ns/step) for score tasks
WAITB = 8500          # pacing base offset (ns)
WAITD = 700           # extra tanh release delay past its su matmuls (ns)

_cached = {}


def _build_nc(stage=3):
    # stage 1: LSTM only; 2: + scores/softmax/out2; 3: full (vocab)
    nc = bacc.Bacc("TRN2", target_bir_lowering=False, debug=False,
                   num_devices=NC)

    def din(name, shape, dt):
        return nc.dram_tensor(name, shape, dt, kind="ExternalInput").ap()

    t_ident = din("ident", [128, 128], BF)
    t_h0 = din("h0", [128, KC * BL], BF)
    t_c0 = din("c0", [128, KC * BL], F32)
    t_whha = din("whha", [128, 8 * KC * 128], BF)    # [p,(j<8,kc,g)]
    t_whhb = din("whhb", [128, 8 * KC * 128], BF)    # [p,(j>=8,kc,g)]
    t_xg = din("xg", [128, T * 16 * BL], BF)         # [p,(t,j,b)]
    t_encT = din("encT", [128, KC * BL * S], BF)     # [p,(kc,b,s)]
    t_whT = din("whT", [128, KC * D], BF)            # Wh_w.T re
    t_wsT = din("wsT", [128, KC * D], BF)            # Ws_w.T re
    t_wsb8 = din("wsb8", [TS, D], BF)                # Ws_b row x TS
    t_vt = din("vt", [128, KC], BF)
    t_selv = din("selv", [128, 2 * TS * S], F8)      # DR selectors
    t_enc = din("enc", [128, BL * D], BF)            # [s,(b,d)]
    t_vw1 = din("vw1", [128, KC * D], BF)
    t_vw2 = din("vw2", [128, KC * D], BF)
    t_vb = din("vb", [128, KC], F32)
    t_vpt = din("vpt", [128, KC * V], BF)            # [p,(kc,v)] Vp_w.T re
    t_out = nc.dram_tensor("out", [R, V], BF, kind="ExternalOutput").ap()

    with tile.TileContext(nc) as tc:
        with (
            tc.tile_pool(name="const", bufs=1) as cp,
            tc.tile_pool(name="state", bufs=1) as sp,
            tc.tile_pool(name="gates", bufs=8) as gp,
            tc.tile_pool(name="attn", bufs=2) as ap_,
            tc.tile_pool(name="thp", bufs=2) as thp,
            tc.tile_pool(name="voc", bufs=2) as vp,
            tc.tile_pool(name="lsbp", bufs=3) as lp,
            tc.tile_pool(name="ps_g", bufs=2, space="PSUM") as ppg,
            tc.tile_pool(name="ps_sum", bufs=2, space="PSUM") as pps,
            tc.tile_pool(name="ps_e", bufs=1, space="PSUM") as ppe,
            tc.tile_pool(name="ps_w", bufs=1, space="PSUM") as ppw,
        ):
            # ---- LSTM-critical loads first (serial DMA device) ----
            ident = cp.tile([128, 128], BF)
            nc.sync.dma_start(out=ident[:], in_=t_ident[:])
            h0b = sp.tile([128, KC * BL], BF)
            nc.sync.dma_start(out=h0b[:], in_=t_h0[:])
            c = sp.tile([128, KC * BL], F32)
            nc.sync.dma_start(out=c[:], in_=t_c0[:])
            whha = cp.tile([128, 8 * KC * 128], BF)
            nc.sync.dma_start(out=whha[:], in_=t_whha[:])
            xg = cp.tile([128, T * 16 * BL], BF)
            nc.sync.dma_start(out=xg[:], in_=t_xg[:])
            whhb = cp.tile([128, 8 * KC * 128], BF)
            nc.sync.dma_start(out=whhb[:], in_=t_whhb[:])
            # ---- wh deps, then remaining constants ----
            encT = cp.tile([128, KC * BL * S], BF)
            nc.sync.dma_start(out=encT[:], in_=t_encT[:])
            whT = cp.tile([128, KC * D], BF)
            nc.sync.dma_start(out=whT[:], in_=t_whT[:])
            wsT = cp.tile([128, KC * D], BF)
            nc.sync.dma_start(out=wsT[:], in_=t_wsT[:])
            wsb8 = cp.tile([TS, D], BF)
            nc.sync.dma_start(out=wsb8[:], in_=t_wsb8[:])
            vt = cp.tile([128, KC], BF)
            nc.sync.dma_start(out=vt[:], in_=t_vt[:])
            selv = cp.tile([128, 2 * TS * S], F8)
            nc.sync.dma_start(out=selv[:], in_=t_selv[:])
            enc = cp.tile([128, BL * D], BF)
            nc.sync.dma_start(out=enc[:], in_=t_enc[:])
            vw1 = cp.tile([128, KC * D], BF)
            nc.sync.dma_start(out=vw1[:], in_=t_vw1[:])
            vw2 = cp.tile([128, KC * D], BF)
            nc.sync.dma_start(out=vw2[:], in_=t_vw2[:])
            vb = cp.tile([128, KC], F32)
            nc.sync.dma_start(out=vb[:], in_=t_vb[:])

            encT4 = encT[:].rearrange("p (kc b s) -> p kc b s", kc=KC, b=BL)
            whT4 = whT[:].rearrange("p (kc d) -> p kc d", kc=KC)
            wsT4 = wsT[:].rearrange("p (kc d) -> p kc d", kc=KC)
            selv3 = selv[:].rearrange("p (i n) -> p i n", i=2)

            outT = sp.tile([128, KC * BL * T], BF)   # [p,(kc,b,t)] all h's
            outT4 = outT[:].rearrange("p (kc b t) -> p kc b t", kc=KC, b=BL,
                                      t=T)

            # ---- vocab weight prefetch: chunked large DMAs ----
            vpt4 = t_vpt[:].rearrange("p (kc v) -> p kc v", kc=KC, v=V)
            vpre = cp.tile([128, KC * NPRE * VBLK], BF)
            vpre4 = vpre[:].rearrange("p (kc v) -> p kc v", kc=KC,
                                      v=NPRE * VBLK)
            PCH = 7
            for pc in range(0, NPRE, PCH):
                pe_ = min(NPRE, pc + PCH)
                nc.sync.dma_start(out=vpre4[:, :, pc * VBLK:pe_ * VBLK],
                                  in_=vpt4[:, :, pc * VBLK:pe_ * VBLK])
            # 3 more blocks pre-staged in the lsb slots (dead until
            # phase 3); consumed first there, after which the lsb group
            # tiles rotate into these slots via the tag WAR
            NLSB = 3
            vlsb = []
            for i in range(NLSB):
                vt_ = lp.tile([128, KC, VBLK], BF, tag="lsb",
                              name=f"vlsb{i}")
                nc.sync.dma_start(
                    out=vt_[:, :, :],
                    in_=vpt4[:, :, (NPRE + i) * VBLK:(NPRE + i + 1) * VBLK])
                vlsb.append(vt_)

            # ====== wh[s, d] chunks -> wsu slot0 (fp8), ws_b folded to ws ==
            # wsu[(b,par)]: [128, kc, 2, 128] fp8; slot0 = wh (s-partition),
            # slot1 rows 0..TS = ws rows of t-block (t-partition), rest zero.
            wsu_tiles = {}
            for b_ in range(BL):
                for par in range(2):
                    w_ = sp.tile([128, KC, 2, 128], F8, tag=f"wsu{b_}_{par}")
                    wsu_tiles[(b_, par)] = w_
                    nc.gpsimd.memset(w_[:, :, 1, :], 0)
            for b_ in range(BL):
                whp = ppw.tile([S, D], F32, tag="ws", name=f"whp{b_}")
                for kc in range(KC):
                    nc.tensor.matmul(out=whp[:], lhsT=encT4[:, kc, b_, :],
                                     rhs=whT4[:, kc, :],
                                     start=(kc == 0), stop=(kc == KC - 1))
                whp3 = whp[:].rearrange("p (kc d) -> p kc d", kc=KC)
                for par in range(2):
                    nc.vector.tensor_copy(
                        out=wsu_tiles[(b_, par)][:, :, 0, :], in_=whp3[:, :, :])

            eps = ppe.tile([S, BL * T], F32, tag="eps")   # scores [s,(b,t)]

            # ========== LSTM with interleaved score tasks ==========
            # task_a: fp8 DoubleRow outer-sum + ACT tanh (lags LSTM);
            # task_b: eps dot matmuls, emitted later still.
            pend_a = []
            pend_b = []
            th_tiles = {}

            su_tiles = {}

            def emit_su(b, tsub, dc):
                # PE part of a score task: no h-dependency, runs in PE idle
                su = pps.tile([128, TS * S], F32, tag="sum",
                              name=f"su{b}_{tsub}_{dc}")
                wsu = wsu_tiles[(b, tsub % 2)]
                for cc in range(TS * S // CC):
                    nc.tensor.matmul(out=su[:, cc * CC:(cc + 1) * CC],
                                     lhsT=wsu[:, dc, :, :],
                                     rhs=selv3[:, :, cc * CC:(cc + 1) * CC],
                                     start=True, stop=True, perf_mode=DR)
                su_tiles[(b, tsub, dc)] = su
                pend_th.append((b, tsub, dc))

            HH = TS * S // 2
            thA_done = {}

            def emit_thA(b, tsub, dc):
                # first tanh half: fills the ACT idle gap while DVE runs
                # the cell update, ahead of tanh(c)
                th = thp.tile([128, TS * S], BF, tag=f"th{dc}",
                              name=f"th{b}_{tsub}_{dc}")
                nc.scalar.activation(out=th[:, 0:HH],
                                     in_=su_tiles[(b, tsub, dc)][:, 0:HH],
                                     func=AF.Tanh)
                thA_done[(b, tsub, dc)] = th

            def emit_thB(b, tsub, dc):
                th = thA_done.pop((b, tsub, dc))
                nc.scalar.activation(out=th[:, HH:],
                                     in_=su_tiles.pop((b, tsub, dc))[:, HH:],
                                     func=AF.Tanh)
                th_tiles[(b, tsub, dc)] = th
                if dc == KC - 1:
                    pend_b.append((b, tsub))

            def emit_th(b, tsub, dc):
                # unsplit path for the post-loop drain (fewer overheads)
                th = thp.tile([128, TS * S], BF, tag=f"th{dc}",
                              name=f"th{b}_{tsub}_{dc}")
                nc.scalar.activation(out=th[:],
                                     in_=su_tiles.pop((b, tsub, dc))[:],
                                     func=AF.Tanh)
                th_tiles[(b, tsub, dc)] = th
                if dc == KC - 1:
                    pend_b.append((b, tsub))

            eps_done = [False] * BL

            def emit_b(b, tsub, half):
                for tl in range(half * (TS // 2), (half + 1) * (TS // 2)):
                    t = tsub * TS + tl
                    for d2 in range(KC):
                        nc.tensor.matmul(
                            out=eps[:, b * T + t: b * T + t + 1],
                            lhsT=th_tiles[(b, tsub, d2)][:,
                                tl * S:(tl + 1) * S],
                            rhs=vt[:, d2:d2 + 1],
                            start=(d2 == 0), stop=(d2 == KC - 1))
                if tsub == T // TS - 1 and half == 1:
                    eps_done[b] = True

            pend_th = []

            def pump(hint_ns=None):
                # su one step ahead of its tanh: PE part never blocks ACT.
                # thA was emitted inside the LSTM block; finish with thB.
                if pend_th:
                    head = pend_th[0]
                    if head in thA_done:
                        emit_thB(*pend_th.pop(0))
                    else:
                        emit_th(*pend_th.pop(0))
                if pend_a:
                    emit_su(*pend_a.pop(0))
                if pend_b:
                    b, tsub = pend_b[0]
                    half = pump.half
                    emit_b(b, tsub, half)
                    if half == 1:
                        pend_b.pop(0)
                    pump.half = 1 - half
            pump.half = 0

            from contextlib import ExitStack
            for t in range(T):
                _hp = ExitStack()
                _hp.enter_context(tc.high_priority())
                gps = ppg.tile([128, 16 * BL], F32, tag="gps",
                               name=f"gps{t}")
                nc.tensor.matmul(out=gps[:], lhsT=ident[:],
                                 rhs=xg[:, t * 32:(t + 1) * 32],
                                 start=True, stop=False,
                                 skip_group_check=True)
                for j in range(16):
                    wt = whha if j < 8 else whhb
                    jr = j % 8
                    for kc in range(KC):
                        if t == 0:
                            hsrc = h0b[:, kc * BL:(kc + 1) * BL]
                        else:
                            hsrc = outT4[:, kc, :, t - 1]
                        nc.tensor.matmul(
                            out=gps[:, j * BL:(j + 1) * BL],
                            lhsT=wt[:, (jr * KC + kc) * 128:
                                    (jr * KC + kc + 1) * 128],
                            rhs=hsrc, start=False, stop=(kc == KC - 1),
                            skip_group_check=True)
                # ONE sigmoid for all four gates (g rows host-doubled):
                # tanh(g) = 2*sigmoid(2g) - 1, absorbed into the DVE ops.
                # Each ACT instruction costs ~400 ns of pipeline hold on
                # top of its work, so fewer/larger ACT ops win.
                sio = gp.tile([128, 16 * BL], F32, tag="sio")
                nc.scalar.activation(out=sio[:],
                                     in_=gps[:], func=AF.Sigmoid)
                t1 = gp.tile([128, KC * BL], F32, tag="t1")
                t2 = gp.tile([128, KC * BL], F32, tag="t2")
                nc.vector.tensor_mul(out=t1[:], in0=sio[:, 4 * BL:8 * BL],
                                     in1=c[:])
                # t2 = (2*sig(2g)) * sig(i)
                nc.vector.scalar_tensor_tensor(
                    out=t2[:], in0=sio[:, 12 * BL:16 * BL], scalar=2.0,
                    in1=sio[:, 0:4 * BL], op0=ALU.mult, op1=ALU.mult)
                t3 = gp.tile([128, KC * BL], F32, tag="t3")
                nc.vector.tensor_add(out=t3[:], in0=t1[:], in1=t2[:])
                nc.vector.tensor_sub(out=c[:], in0=t3[:],
                                     in1=sio[:, 0:4 * BL])
                tc_ = gp.tile([128, KC * BL], F32, tag="tc")
                nc.scalar.activation(out=tc_[:], in_=c[:], func=AF.Tanh)
                nc.vector.tensor_mul(out=outT4[:, :, :, t],
                                     in0=sio[:, 8 * BL:12 * BL], in1=tc_[:])
                _hp.close()

                with tc.tile_wait_until((WAITB + WAITP * t) / 1e6):
                    pump(hint_ns=WAITB + WAITP * t)

                if stage >= 2 and t % TS == TS - 1:
                    tsub = t // TS
                    tc.tile_set_cur_wait((WAITB + WAITP * t) / 1e6)
                    for b in range(BL):
                        wps = ppw.tile([TS, D], F32, tag="ws",
                                       name=f"wps{b}_{tsub}")
                        for kc in range(KC):
                            nc.tensor.matmul(
                                out=wps[:],
                                lhsT=outT[:, (kc * BL + b) * T + tsub * TS:
                                          (kc * BL + b) * T + tsub * TS + TS],
                                rhs=wsT4[:, kc, :],
                                start=(kc == 0), stop=(kc == KC - 1))
                        wps3 = wps[:].rearrange("p (kc d) -> p kc d", kc=KC)
                        wsb3 = wsb8[:].rearrange("p (kc d) -> p kc d", kc=KC)
                        nc.vector.tensor_add(
                            out=wsu_tiles[(b, tsub % 2)][0:TS, :, 1, :],
                            in0=wps3[:, :, :], in1=wsb3[:, :, :])
                    tc.cur_wait_ts = 0
                    pend_a.extend((b, tsub, dc) for b in range(BL)
                                  for dc in range(KC))

            # ============ softmax, context, out2 (per batch) ============
            ctxT = sp.tile([128, BL * KC * T], BF)   # [p,(b,dc,t)]
            o2T = sp.tile([128, KC * BL * T], BF)    # [p,(ec,b,t)]

            def sm_b(b):
                _hp2 = ExitStack()
                _hp2.enter_context(tc.high_priority())
                ebf = ap_.tile([S, T], BF, tag="ebf")
                nc.scalar.activation(out=ebf[:],
                                     in_=eps[:, b * T:(b + 1) * T],
                                     func=AF.Exp)
                etp = ppw.tile([T, S], BF, tag="ws", name=f"etp{b}")
                nc.tensor.transpose(out=etp[:], in_=ebf[:],
                                    identity=ident[:, :])
                ssum = ap_.tile([T, 1], F32, tag="ssum")
                nc.vector.tensor_reduce(out=ssum[:], in_=etp[:],
                                        axis=mybir.AxisListType.X, op=ALU.add)
                rsum = ap_.tile([T, 1], F32, tag="rsum")
                nc.vector.reciprocal(out=rsum[:], in_=ssum[:])
                abf = ap_.tile([T, S], BF, tag="abf")
                nc.vector.tensor_scalar_mul(out=abf[:], in0=etp[:],
                                            scalar1=rsum[:])
                atp = ppw.tile([S, T], BF, tag="ws", name=f"atp{b}")
                nc.tensor.transpose(out=atp[:], in_=abf[:],
                                    identity=ident[0:T, 0:T])
                atb = ap_.tile([S, T], BF, tag="atb")
                nc.vector.tensor_copy(out=atb[:], in_=atp[:])
                for dc in range(KC):
                    cps = ppg.tile([128, T], F32, tag="gps",
                                   name=f"cps{b}_{dc}")
                    nc.tensor.matmul(out=cps[:],
                                     lhsT=enc[:, b * D + dc * 128:
                                              b * D + (dc + 1) * 128],
                                     rhs=atb[:], start=True, stop=True)
                    nc.vector.tensor_copy(
                        out=ctxT[:, (b * KC + dc) * T:(b * KC + dc + 1) * T],
                        in_=cps[:])
                for ec in range(KC):
                    ops = ppg.tile([128, T], F32, tag="gps",
                                   name=f"ops{b}_{ec}")
                    for kc in range(KC):
                        nc.tensor.matmul(
                            out=ops[:],
                            lhsT=vw1[:, kc * D + ec * 128:
                                     kc * D + (ec + 1) * 128],
                            rhs=ctxT[:, (b * KC + kc) * T:
                                     (b * KC + kc + 1) * T],
                            start=(kc == 0), stop=False)
                    for kc in range(KC):
                        nc.tensor.matmul(
                            out=ops[:],
                            lhsT=vw2[:, kc * D + ec * 128:
                                     kc * D + (ec + 1) * 128],
                            rhs=outT[:, (kc * BL + b) * T:
                                     (kc * BL + b + 1) * T],
                            start=False, stop=(kc == KC - 1))
                    nc.vector.tensor_scalar(
                        out=o2T[:, (ec * BL + b) * T:(ec * BL + b) * T + T],
                        in0=ops[:], scalar1=vb[:, ec:ec + 1], scalar2=None,
                        op0=ALU.add)
                _hp2.close()

            # drain: all remaining su first (PE), then tanh/eps; each
            # batch's softmax/out2 is emitted the moment its scores finish,
            # overlapping the other batch's tanh drain
            sm_done = set()
            while pend_a:
                emit_su(*pend_a.pop(0))
            while pend_th or pend_b:
                pump()
                if stage >= 2:
                    for b_ in range(BL):
                        if eps_done[b_] and b_ not in sm_done:
                            sm_b(b_)
                            sm_done.add(b_)
            if stage >= 2:
                for b_ in range(BL):
                    if b_ not in sm_done:
                        sm_b(b_)

            # ================= vocab projection (bf16) ==========
            o2r = o2T[:].rearrange("p (e c) -> p e c", e=KC, c=128)
            # Recycle dead phase-1 SBUF slots as stream buffers for the tail
            # vocab blocks: their DMAs fire as soon as the old tiles' readers
            # retire, moving DMA out of the saturated vocab tail.
            GRP2 = ("whha", "whhb", "xg")   # 8KB slots: 2 blocks each
            NRG = 2 * len(GRP2)
            RS1 = ["whT", "wsT", "vw1", "vw2"]
            # 2KB slots that free when the last score task retires; pairs
            # stage one block each as two [128,2,512] half-tiles
            RS2 = [("th0", "th1"), ("th2", "th3"), ("th0", "th1"),
                   ("th2", "th3"), ("selv", "encT")]
            rg0 = NBLK - NRG - len(RS1)  # whh group covers rg0..rg0+3
            rs2_0 = rg0 - len(RS2)
            rs3_0 = rs2_0 - 2            # wsu-quad + outT/ctxT/wsb8 blocks
            rsrc = {}
            if stage >= 3:
                # 8 KB slots -> two blocks each
                for gi, wtag in enumerate(GRP2):
                    ib_g = rg0 + gi * 2
                    vg = cp.tile([128, KC, 2 * VBLK], BF, tag=wtag,
                                 name=f"vgrp{gi}")
                    nc.sync.dma_start(
                        out=vg[:, :, :],
                        in_=vpt4[:, :, ib_g * VBLK:(ib_g + 2) * VBLK])
                    rsrc[ib_g] = (vg, 0)
                    rsrc[ib_g + 1] = (vg, VBLK)
                # quad block from the four 1KB wsu slots (kc0..3)
                ibx = rs3_0
                vqs = []
                for b_ in range(BL):
                    for par in range(2):
                        vq = sp.tile([128, 1, VBLK], BF,
                                     tag=f"wsu{b_}_{par}",
                                     name=f"vq{b_}_{par}")
                        kcq = b_ * 2 + par
                        nc.sync.dma_start(
                            out=vq[:, :, :],
                            in_=vpt4[:, kcq:kcq + 1,
                                     ibx * VBLK:(ibx + 1) * VBLK])
                        vqs.append((vq, 0))
                rsrc[ibx] = ("quad", vqs)
                # quad block from outT (1KB) + ctxT (1KB) + enc (2KB) slots
                iby = rs3_0 + 1
                vq_o = sp.tile([128, 1, VBLK], BF, tag="outT", name="vqo")
                nc.sync.dma_start(out=vq_o[:, :, :],
                                  in_=vpt4[:, 0:1, iby * VBLK:
                                           (iby + 1) * VBLK])
                vq_c = sp.tile([128, 1, VBLK], BF, tag="ctxT", name="vqc")
                nc.sync.dma_start(out=vq_c[:, :, :],
                                  in_=vpt4[:, 1:2, iby * VBLK:
                                           (iby + 1) * VBLK])
                vq_e2 = cp.tile([128, 2, VBLK], BF, tag="enc", name="vqe2")
                nc.sync.dma_start(out=vq_e2[:, :, :],
                                  in_=vpt4[:, 2:4, iby * VBLK:
                                           (iby + 1) * VBLK])
                rsrc[iby] = ("quad", [(vq_o, 0), (vq_c, 0),
                                      (vq_e2, 0), (vq_e2, 1)])
                for i, (tga, tgb) in enumerate(RS2):
                    ib_r = rs2_0 + i
                    v0r = ib_r * VBLK
                    pa = thp if tga.startswith("th") else cp
                    pb = thp if tgb.startswith("th") else cp
                    ta = pa.tile([128, 2, VBLK], BF, tag=tga,
                                 name=f"vspl{ib_r}a")
                    nc.sync.dma_start(out=ta[:, :, :],
                                      in_=vpt4[:, 0:2, v0r:v0r + VBLK])
                    tb = pb.tile([128, 2, VBLK], BF, tag=tgb,
                                 name=f"vspl{ib_r}b")
                    nc.sync.dma_start(out=tb[:, :, :],
                                      in_=vpt4[:, 2:4, v0r:v0r + VBLK])
                    rsrc[ib_r] = ("split", ta, tb)
                for i, tg in enumerate(RS1):
                    ib_r = rg0 + NRG + i
                    v0r = ib_r * VBLK
                    wr = min(VBLK, V - v0r)
                    vrt = cp.tile([128, KC, VBLK], BF, tag=tg,
                                  name=f"vrt{ib_r}")
                    nc.sync.dma_start(out=vrt[:, :, :wr],
                                      in_=vpt4[:, :, v0r:v0r + wr])
                    rsrc[ib_r] = (vrt, 0)
            # consumption order: alternate streamed/prefetched so stream DMAs
            # never stall; recycled-slot blocks go last (data arrives
            # mid-phase)
            tail0 = rs3_0 - (rs3_0 % 4)   # align tail to lsb store groups
            # lsb-staged blocks first (their matmuls must retire before the
            # group tiles rotate into those slots), then all ring pre-fills
            # in a burst so every slot frees early and the vst stream runs
            # AHEAD of consumption instead of chasing it
            order = list(range(NPRE, NPRE + NLSB))
            order += list(range(NPRE + NLSB,
                                min(NPRE + NLSB + NSTRB, tail0)))
            si, pi = min(NPRE + NLSB + NSTRB, tail0), 0
            while si < tail0 or pi < NPRE:
                for _ in range(2):
                    if pi < NPRE:
                        order.append(pi)
                        pi += 1
                if si < tail0:
                    order.append(si)
                    si += 1
            order += list(range(tail0, NBLK))
            lsb_tiles = {}
            if stage < 3:
                order = []
            for nb, ib in enumerate(order):
                v0 = ib * VBLK
                w = min(VBLK, V - v0)
                if ib < NPRE:
                    def rhs_of(kc, v0=v0, w=w):
                        return vpre4[:, kc, v0:v0 + w]
                elif ib < NPRE + NLSB:
                    def rhs_of(kc, vt_=vlsb[ib - NPRE], w=w):
                        return vt_[:, kc, 0:w]
                elif ib in rsrc:
                    ent = rsrc[ib]
                    if ent[0] == "split":
                        def rhs_of(kc, ta=ent[1], tb=ent[2], w=w):
                            return (ta if kc < 2 else tb)[:, kc % 2, 0:w]
                    elif ent[0] == "quad":
                        def rhs_of(kc, lst=ent[1], w=w):
                            t_, ix = lst[kc]
                            return t_[:, ix, 0:w]
                    else:
                        def rhs_of(kc, vs3=ent[0], voff=ent[1], w=w):
                            return vs3[:, kc, voff:voff + w]
                else:
                    vst = vp.tile([128, KC, VBLK], BF, tag="vs", bufs=NSTRB,
                                  name=f"vst{ib}")
                    nc.sync.dma_start(out=vst[:, :, :w],
                                      in_=vpt4[:, :, v0:v0 + w])
                    def rhs_of(kc, vst=vst, w=w):
                        return vst[:, kc, 0:w]
                grp = ib // 4
                if grp not in lsb_tiles:
                    lsb_tiles[grp] = [lp.tile([128, 4 * VBLK], BF, tag="lsb",
                                              name=f"lsb{grp}"), 0]
                lsb_e = lsb_tiles[grp]
                lps = ppg.tile([128, VBLK], F32, tag="gps", name=f"lps{ib}")
                for kc in range(KC):
                    nc.tensor.matmul(out=lps[:, :w],
                                     lhsT=o2r[:, kc, :],
                                     rhs=rhs_of(kc),
                                     start=(kc == 0), stop=(kc == KC - 1))
                dst = lsb_e[0][:, (ib % 4) * VBLK:(ib % 4) * VBLK + w]
                if nb % 2 == 0:
                    nc.scalar.copy(out=dst, in_=lps[:, :w])
                else:
                    nc.vector.tensor_copy(out=dst, in_=lps[:, :w])
                lsb_e[1] += 1
                nblk_grp = min(4, NBLK - grp * 4)
                if lsb_e[1] == nblk_grp:
                    gv0 = grp * 4 * VBLK
                    wlen = min(4 * VBLK, V - gv0)
                    nc.sync.dma_start(out=t_out[:, gv0:gv0 + wlen],
                                      in_=lsb_e[0][:, :wlen])

    nc.compile()
    return nc


def _prep_in_maps(inputs):
    inp = {k: np.asarray(v) for k, v in inputs.items()}
    words = inp["words"].astype(np.int64)
    enc = inp["encoder_output"].astype(np.float32)
    pre_h, cell = inp["pre_h"], inp["cell"]
    emb = inp["emb"]
    W_ih, W_hh = inp["W_ih"], inp["W_hh"]
    b_ih, b_hh = inp["b_ih"], inp["b_hh"]
    Wh_w = inp["Wh_w"]
    Ws_w, Ws_b = inp["Ws_w"], inp["Ws_b"]
    vt_w = inp["vt_w"]
    V_w, V_b = inp["V_w"], inp["V_b"]
    Vp_w, Vp_b = inp["Vp_w"], inp["Vp_b"]

    def re_lhsT(m, dt=BF16):  # [512, N] -> [128, 4*N] chunk-major
        n = m.shape[1]
        return np.ascontiguousarray(
            m.reshape(4, 128, n).transpose(1, 0, 2).reshape(128, 4 * n)
        ).astype(dt)

    # gate reorder (i,f,g,o) -> (i,f,o,g); g rows doubled so a single
    # sigmoid yields sig(2g) and tanh(g) = 2*sig(2g) - 1 on device
    perm = np.r_[0:512, 512:1024, 1536:2048, 1024:1536]
    W_ih_p = W_ih[perm].copy()
    W_hh_p = W_hh[perm].copy()
    b2 = (b_ih + b_hh)[perm].astype(np.float32)
    W_ih_p[1536:] *= 2.0
    W_hh_p[1536:] *= 2.0
    b2[1536:] *= 2.0

    whh_re = re_lhsT(np.ascontiguousarray(W_hh_p.T))     # [p,(kc,g)]
    # -> j-major [p,(j,kc,128)]
    whh_j = np.ascontiguousarray(
        whh_re.reshape(128, KC, 16, 128).transpose(0, 2, 1, 3)
        .reshape(128, 16 * KC * 128))
    whha_re = np.ascontiguousarray(whh_j[:, :8 * KC * 128])
    whhb_re = np.ascontiguousarray(whh_j[:, 8 * KC * 128:])
    whT_re = re_lhsT(np.ascontiguousarray(Wh_w.T))
    wsT_re = re_lhsT(np.ascontiguousarray(Ws_w.T))
    vw1_re = re_lhsT(np.ascontiguousarray(V_w[:, :D].T))
    vw2_re = re_lhsT(np.ascontiguousarray(V_w[:, D:].T))
    vpt_re = re_lhsT(np.ascontiguousarray(Vp_w.T))
    wsb8_re = np.tile(Ws_b.reshape(1, D), (TS, 1)).astype(BF16)
    vb_re = np.ascontiguousarray(V_b.reshape(4, 128).T).astype(np.float32)
    vt_re = np.ascontiguousarray(vt_w.reshape(4, 128).T).astype(BF16)
    ident_re = np.eye(128, dtype=np.float32).astype(BF16)
    # DR selector: slot0[p,(tl,s)] = [p==s], slot1[p,(tl,s)] = [p==tl]
    sel0 = np.tile(np.eye(128, dtype=np.float32), (1, TS))
    sel1 = np.zeros((128, TS * S), dtype=np.float32)
    for tl in range(TS):
        sel1[tl, tl * S:(tl + 1) * S] = 1.0
    selv_re = np.concatenate([sel0, sel1], axis=1).astype(FP8)

    x_all = emb[words]                                   # [B,T,D]
    xg_all = x_all @ W_ih_p.T.astype(np.float32) + b2    # [B,T,4D]

    in_maps = []
    for k in range(NC):
        bs = slice(k * BL, (k + 1) * BL)
        xgl = xg_all[bs]                                 # [2,T,2048]
        xg_re = np.ascontiguousarray(
            xgl.reshape(BL, T, 16, 128).transpose(3, 1, 2, 0)
            .reshape(128, T * 16 * BL)).astype(BF16)     # [p,(t,j,b)]
        h0 = np.ascontiguousarray(
            pre_h[bs].reshape(BL, 4, 128).transpose(2, 1, 0)
            .reshape(128, 4 * BL)).astype(BF16)
        c0 = np.ascontiguousarray(
            cell[bs].reshape(BL, 4, 128).transpose(2, 1, 0)
            .reshape(128, 4 * BL)).astype(np.float32)
        encl = enc[bs]                                   # [2,S,D]
        encT_re = np.ascontiguousarray(
            encl.reshape(BL, S, 4, 128).transpose(3, 2, 0, 1)
            .reshape(128, 4 * BL * S)).astype(BF16)
        enc_re = np.ascontiguousarray(
            encl.transpose(1, 0, 2).reshape(S, BL * D)).astype(BF16)
        in_maps.append({
            "xg": xg_re, "whha": whha_re, "whhb": whhb_re, "h0": h0,
            "c0": c0, "encT": encT_re, "enc": enc_re, "whT": whT_re,
            "wsT": wsT_re, "wsb8": wsb8_re, "vt": vt_re, "selv": selv_re,
            "vw1": vw1_re, "vw2": vw2_re, "vb": vb_re, "vpt": vpt_re,
            "ident": ident_re,
        })
    return in_maps


def kernel(**inputs):
    in_maps = _prep_in_maps(inputs)
    if "nc" not in _cached:
        _cached["nc"] = _build_nc()
    res = bass_utils.run_bass_kernel_spmd(_cached["nc"], in_maps,
                                          core_ids=list(range(NC)))
    vpb = np.asarray(inputs["Vp_b"]).astype(np.float32)
    outs = [np.asarray(res.results[k]["out"]).astype(np.float32)
            .reshape(BL, T, V) for k in range(NC)]
    return np.concatenate(outs, axis=0) + vpb[None, None, :]


if __name__ == "__main__":
    pass
